# revision 1
# baseline (speedup 1.0000x reference)
"""Trainium2 Bass kernel for EmbededNonLocalLayer.

Distribution: 8 cores = 4 batches x 2 query-halves. Each core holds its
batch's full keys/values; its query half arrives as a separate input
slice xq (1985 real columns padded to 2048), so all 8 cores run one
static SPMD program.

Per-core math (transposed-attention layout, softmax denominators folded):
  qk   = BN(Wk x + bk)                      [256, N]   (BN folded on host)
  qkq  = BN(Wk xq + bk)                     [256, 2048] (query columns)
  val  = Wv x                               [256, N]
  v2   = (Wv2/49) @ pooled_sum(val)         [256, 81]
  simv82[m, 0:81] = softmax_k(val^T v2 /16) [N, 81], col 81 = 1
  E[m, q]    = exp((qk^T qkq)/16)           (keys m on partitions)
  out82[k,q] = sum_m simv82[m,k] E[m,q]     (row 81 = softmax denom r1[q])
  out[o, q]  = (Ww @ (v2 out82[0:81])) / r1[q]
"""

import sys

sys.path.insert(0, "/opt/trn_rl_repo")

import numpy as np

import concourse.bacc as bacc
import concourse.bass as bass
import concourse.mybir as mybir
from concourse.bass_utils import run_bass_kernel_spmd
from concourse.tile import TileContext

F32 = mybir.dt.float32
F32R = mybir.dt.float32r
AF = mybir.ActivationFunctionType
AX = mybir.AxisListType

B, CIN, H, W = 4, 512, 63, 63
N = H * W            # 3969
NPAD = 4096
CI, CO = 256, 512
KK = 81              # 9*9 pooled positions
SCALE = 0.0625       # 1/sqrt(CI)
QCNT = 1985          # queries per core (1 overlap column)
QP = 2048            # padded query columns
Q0STEP = 1984        # query offset of the second half
MB = NPAD // 128     # 32 key blocks
N_CHUNKS = [(i * 512, 512) for i in range(NPAD // 512)]

_CACHE = {}


def _build_program(reps=1):
    nc = bacc.Bacc()

    x_d = nc.dram_tensor("x", [CIN, N], F32R, kind="ExternalInput")
    xp_d = nc.dram_tensor("xpool", [CIN, 82], F32R, kind="ExternalInput")
    wk_d = nc.dram_tensor("wkT", [CIN, CI], F32R, kind="ExternalInput")
    wv_d = nc.dram_tensor("wvT", [CIN, CI], F32R, kind="ExternalInput")
    wv2_d = nc.dram_tensor("wv2T", [CI, CI], F32R, kind="ExternalInput")
    ww_d = nc.dram_tensor("wwT", [CI, CO], F32R, kind="ExternalInput")
    bk2_d = nc.dram_tensor("bk2p", [128, 2], F32, kind="ExternalInput")
    cones_d = nc.dram_tensor("cones", [128, 128], F32R, kind="ExternalInput")
    czero_d = nc.dram_tensor("czero", [128, 128], F32R, kind="ExternalInput")
    out_d = nc.dram_tensor("out", [CO, QP], F32, kind="ExternalOutput")

    with TileContext(nc) as tc, \
         nc.allow_low_precision(reason="float32r is bit-identical to float32"):
      for _rep in range(reps):
        with tc.tile_pool(name=f"const{_rep}", bufs=1) as cpool:
          # ---- persistent tiles (live for the whole kernel) ----
          ones_sb = cpool.tile([1, 128], F32R)
          wk_sb = cpool.tile([128, 4 * CI], F32R)
          wv_sb = cpool.tile([128, 4 * CI], F32R)
          wv2_sb = cpool.tile([128, 2 * CI], F32R)
          ww_sb = cpool.tile([128, 2 * CO], F32R)
          bk2_sb = cpool.tile([128, 2], F32)
          qk_sb = cpool.tile([128, 2 * NPAD], F32R)
          xp_sb = cpool.tile([128, 4 * 82], F32R)
          x_sb = cpool.tile([128, 4 * NPAD], F32R)
          simv_sb = cpool.tile([128, MB * 82], F32R)
          pooled_sb = cpool.tile([128, 2 * 82], F32R)
          v2_sb = cpool.tile([128, 2 * 82], mybir.dt.bfloat16)
          v2t_sb = cpool.tile([82, CI], F32R)
          r2_sb = cpool.tile([128, MB], F32)
          r2i_sb = cpool.tile([128, MB], F32)
          mask_sb = cpool.tile([128, 1], F32)

          with tc.tile_pool(name="psA", bufs=4, space="PSUM") as psA:
              # ---- phase A: loads. Small tensors first so PE-feeding data
              # is ready early; the big x load streams behind them.
              nc.sync.dma_start(out=bk2_sb[:], in_=bk2_d[:])
              nc.sync.dma_start(out=wk_sb[:, 0:CI], in_=wk_d[0:128, :])
              nc.sync.dma_start(out=wv_sb[:, 0:CI], in_=wv_d[0:128, :])

              with tc.tile_pool(name="early", bufs=1) as epool:
                  val_sb = epool.tile([128, 2 * NPAD], mybir.dt.bfloat16)

                  # phases B+C: x arrives in 512-column slabs (all 4 cin
                  # chunks per slab); weight chunks ride between the first
                  # slabs; conv matmuls chase the wire.
                  def _slab_dma(ni):
                      n0 = ni * 512
                      rl = min(512, max(0, N - n0))
                      for cc in range(4):
                          if rl > 0:
                              nc.sync.dma_start(
                                  out=x_sb[:, cc * NPAD + n0:cc * NPAD + n0 + rl],
                                  in_=x_d[cc * 128:(cc + 1) * 128, n0:n0 + rl])
                          if rl < 512:
                              nc.sync.dma_start(
                                  out=x_sb[:, cc * NPAD + n0 + rl:
                                           cc * NPAD + n0 + 512],
                                  in_=czero_d[:, 0:512 - rl])

                  _slab_dma(0)
                  for c in range(1, 4):
                      nc.sync.dma_start(out=wk_sb[:, c * CI:(c + 1) * CI],
                                        in_=wk_d[c * 128:(c + 1) * 128, :])
                      nc.sync.dma_start(out=wv_sb[:, c * CI:(c + 1) * CI],
                                        in_=wv_d[c * 128:(c + 1) * 128, :])
                      _slab_dma(c)

                  for n0, sz in N_CHUNKS:
                      ni = n0 // 512
                      if ni + 4 <= 7:
                          _slab_dma(ni + 4)
                      if ni == 0:
                          for c in range(4):
                              nc.sync.dma_start(
                                  out=xp_sb[:, c * 82:(c + 1) * 82],
                                  in_=xp_d[c * 128:(c + 1) * 128, :])
                      if ni == 1:
                          for c in range(2):
                              nc.sync.dma_start(
                                  out=wv2_sb[:, c * CI:(c + 1) * CI],
                                  in_=wv2_d[c * 128:(c + 1) * 128, :])
                      if ni == 2:
                          for c in range(2):
                              nc.sync.dma_start(
                                  out=ww_sb[:, c * CO:(c + 1) * CO],
                                  in_=ww_d[c * 128:(c + 1) * 128, :])
                          nc.sync.dma_start(out=ones_sb[:],
                                            in_=cones_d[0:1, :])
                      for conv, blk in ((0, 0), (0, 1), (1, 0), (1, 1)):
                          w_sb = wv_sb if conv == 0 else wk_sb
                          ps = psA.tile([128, 512], F32, tag="ps",
                                        name=f"psBC_{conv}_{blk}_{n0}")
                          for cc in range(4):
                              nc.tensor.matmul(
                                  ps[:, :sz],
                                  w_sb[:, cc * CI + blk * 128:cc * CI + blk * 128 + 128],
                                  x_sb[:, cc * NPAD + n0:cc * NPAD + n0 + sz],
                                  start=(cc == 0), stop=(cc == 3))
                          if conv == 0:
                              nc.vector.tensor_copy(
                                  val_sb[:, blk * NPAD + n0:blk * NPAD + n0 + sz],
                                  ps[:, :sz])
                          else:
                              nc.scalar.activation(
                                  qk_sb[:, blk * NPAD + n0:blk * NPAD + n0 + sz],
                                  ps[:, :sz], AF.Identity,
                                  bias=bk2_sb[:, blk:blk + 1])
                  # phase D: zero pad columns (conv wrote junk there from x pads)
                  for blk in range(2):
                      nc.sync.dma_start(
                          out=qk_sb[:, blk * NPAD + N:(blk + 1) * NPAD],
                          in_=czero_d[:, 0:NPAD - N])
                      nc.gpsimd.memset(
                          val_sb[:, blk * NPAD + N:(blk + 1) * NPAD], 0.0)
                  # phase E: pooled value = Wv @ xpool (pooling commutes
                  # with the 1x1 conv; xpool col 0 is zero -> pooled col 0 = 0)
                  for blk in range(2):
                      ps = psA.tile([128, 512], F32, tag="ps", name=f"psE_{blk}")
                      for cc in range(4):
                          nc.tensor.matmul(
                              ps[:, :82],
                              wv_sb[:, cc * CI + blk * 128:cc * CI + blk * 128 + 128],
                              xp_sb[:, cc * 82:(cc + 1) * 82],
                              start=(cc == 0), stop=(cc == 3))
                      nc.vector.tensor_copy(pooled_sb[:, blk * 82:(blk + 1) * 82],
                                            ps[:, :82])
                  # phase F: v2 and v2^T
                  for blk in range(2):
                      ps = psA.tile([128, 512], F32, tag="ps", name=f"psF_{blk}")
                      for cc in range(2):
                          nc.tensor.matmul(
                              ps[:, :82],
                              wv2_sb[:, cc * CI + blk * 128:cc * CI + blk * 128 + 128],
                              pooled_sb[:, cc * 82:(cc + 1) * 82],
                              start=(cc == 0), stop=(cc == 1))
                      nc.vector.tensor_copy(v2_sb[:, blk * 82:(blk + 1) * 82],
                                            ps[:, :82])
                  ps = psA.tile([128, 512], F32, tag="ps", name="psFt")
                  for cc in range(2):
                      nc.tensor.matmul(ps[:82, :CI],
                                       pooled_sb[:, cc * 82:(cc + 1) * 82],
                                       wv2_sb[:, cc * CI:(cc + 1) * CI],
                                       start=(cc == 0), stop=(cc == 1))
                  nc.vector.tensor_copy(v2t_sb[:], ps[:82, :CI])
                  # phase G: simv82, 6 key blocks batched per bank/exp
                  for grp in [list(range(g, min(g + 6, MB)))
                              for g in range(0, MB, 6)]:
                      ps = psA.tile([128, 512], F32, tag="ps",
                                    name=f"psG_{grp[0]}")
                      for j, mb in enumerate(grp):
                          m0 = mb * 128
                          for cc in range(2):
                              nc.tensor.matmul(
                                  ps[:, j * 82:(j + 1) * 82],
                                  val_sb[:, cc * NPAD + m0:cc * NPAD + m0 + 128],
                                  v2_sb[:, cc * 82:(cc + 1) * 82],
                                  start=(cc == 0), stop=(cc == 1))
                      g6 = len(grp)
                      psv = ps[:, 0:g6 * 82].rearrange("p (g c) -> p g c",
                                                       g=g6, c=82)
                      sv = simv_sb[:, grp[0] * 82:(grp[0] + g6) * 82].rearrange(
                          "p (g c) -> p g c", g=g6, c=82)
                      nc.scalar.activation(sv[:, :, 1:82], psv[:, :, 1:82],
                                           AF.Exp, scale=SCALE)
                      nc.vector.reduce_sum(
                          r2_sb[:, grp[0]:grp[0] + g6], sv[:, :, 1:82],
                          axis=AX.X)
                  nc.vector.reciprocal(r2i_sb[:], r2_sb[:])
                  for mb in range(MB):
                      nc.vector.tensor_scalar_mul(
                          simv_sb[:, mb * 82 + 1:(mb + 1) * 82],
                          simv_sb[:, mb * 82 + 1:(mb + 1) * 82],
                          r2i_sb[:, mb:mb + 1])
                  nc.sync.dma_start(
                      out=simv_sb.rearrange("p (b c) -> p b c",
                                            b=MB, c=82)[:, :, 0:1],
                      in_=cones_d[:, 0:MB].rearrange("p (b c) -> p b c",
                                                     b=MB, c=1))
                  # kill the 127 fake key rows of the last block via a row mask
                  nc.gpsimd.memset(mask_sb[:], 0.0)
                  nc.gpsimd.memset(mask_sb[0:1, :], 1.0)
                  nc.vector.tensor_scalar_mul(
                      simv_sb[:, (MB - 1) * 82:MB * 82],
                      simv_sb[:, (MB - 1) * 82:MB * 82],
                      mask_sb[:, 0:1])

          # ---- phase J: attention main loop ----
          with tc.tile_pool(name="work", bufs=1) as wpool, \
               tc.tile_pool(name="psL", bufs=2, space="PSUM") as psL_pool, \
               tc.tile_pool(name="psO", bufs=2, space="PSUM") as psO_pool, \
               tc.tile_pool(name="psT", bufs=2, space="PSUM") as psT_pool:
              for qp in range(2):
                  o82ps = [psO_pool.tile([82, 512], F32, tag="o82",
                                         name=f"o82ps_{qp}_{h2}")
                           for h2 in range(2)]
                  for mb in range(MB):
                      m0 = mb * 128
                      psL = psL_pool.tile([128, 1024], F32, tag="psL",
                                          name=f"psL_{qp}_{mb}")
                      for cc in range(2):
                          lhsT = qk_sb[:, cc * NPAD + m0:cc * NPAD + m0 + 128]
                          for h in range(2):
                              nc.tensor.matmul(
                                  psL[:, h * 512:(h + 1) * 512],
                                  lhsT,
                                  qk_sb[:, cc * NPAD + qp * 1024 + h * 512:
                                        cc * NPAD + qp * 1024 + h * 512 + 512],
                                  start=(cc == 0), stop=(cc == 1))
                      E = wpool.tile([128, 1024], F32R, tag="E", bufs=3,
                                     name=f"E_{qp}_{mb}")
                      nc.scalar.activation(E[:], psL[:], AF.Exp, scale=SCALE)
                      for h in range(2):
                          nc.tensor.matmul(
                              o82ps[h][:],
                              simv_sb[:, mb * 82:(mb + 1) * 82],
                              E[:, h * 512:(h + 1) * 512],
                              start=(mb == 0), stop=(mb == MB - 1))
                  for h in range(2):
                      qc = qp * 2 + h
                      o82 = wpool.tile([82, 512], F32R, tag="o82sb", bufs=2,
                                       name=f"o82_{qc}")
                      nc.scalar.copy(o82[:], o82ps[h][:])
                      # row 0 of out82 is the softmax denominator r1; read it
                      # straight from PSUM so the reciprocal skips the copy
                      rc = wpool.tile([1, 512], F32R, tag="rc", bufs=2,
                                      name=f"rc_{qc}")
                      nc.vector.reciprocal(rc[:], o82ps[h][0:1, :])
                      bps = psT_pool.tile([128, 512], F32, tag="tail",
                                          name=f"bps_{qc}")
                      nc.tensor.matmul(bps[:], ones_sb[:], rc[:],
                                       start=True, stop=True)
                      bc = wpool.tile([128, 512], F32, tag="bc", bufs=2,
                                      name=f"bc_{qc}")
                      nc.scalar.copy(bc[:], bps[:])
                      ctx = wpool.tile([128, 2 * 512], F32R, tag="ctx", bufs=2,
                                       name=f"ctx_{qc}")
                      for c2 in range(2):
                          cps = psT_pool.tile([128, 512], F32, tag="tail",
                                              name=f"cps_{qc}_{c2}")
                          nc.tensor.matmul(cps[:],
                                           v2t_sb[:, c2 * 128:(c2 + 1) * 128],
                                           o82[0:82, :], start=True, stop=True)
                          nc.vector.tensor_copy(ctx[:, c2 * 512:(c2 + 1) * 512],
                                                cps[:])
                      for ob in range(4):
                          ops_ = psT_pool.tile([128, 512], F32, tag="tail",
                                               name=f"ops_{qc}_{ob}")
                          for cc in range(2):
                              nc.tensor.matmul(
                                  ops_[:],
                                  ww_sb[:, cc * CO + ob * 128:cc * CO + ob * 128 + 128],
                                  ctx[:, cc * 512:(cc + 1) * 512],
                                  start=(cc == 0), stop=(cc == 1))
                          outb = wpool.tile([128, 512], F32, tag="outb", bufs=3,
                                            name=f"outb_{qc}_{ob}")
                          nc.vector.tensor_mul(outb[:], ops_[:], bc[:])
                          nc.sync.dma_start(
                              out=out_d[ob * 128:(ob + 1) * 128,
                                        qc * 512:(qc + 1) * 512],
                              in_=outb[:])

    nc.finalize()
    return nc


def _get_program(reps=1):
    if ("nc", reps) not in _CACHE:
        _CACHE[("nc", reps)] = _build_program(reps)
    return _CACHE[("nc", reps)]


def kernel(data_input, Wk, bk, gamma, beta, Wv, bv, Wv2, bv2, Ww, bw):
    f = np.float32
    for name, bias in (("bv", bv), ("bv2", bv2), ("bw", bw)):
        if not np.allclose(np.asarray(bias), 0.0):
            raise NotImplementedError(f"{name} != 0 not supported")
    s = (np.asarray(gamma, f) / np.sqrt(f(1.0) + f(1e-5))).astype(f)
    wkT = np.ascontiguousarray((np.asarray(Wk, f) * s[:, None]).T)
    bk2 = (np.asarray(bk, f) * s + np.asarray(beta, f)).astype(f)
    bk2p = np.ascontiguousarray(bk2.reshape(2, 128).T)
    wvT = np.ascontiguousarray(np.asarray(Wv, f).T)
    wv2T = np.ascontiguousarray((np.asarray(Wv2, f) / f(49.0)).T)
    wwT = np.ascontiguousarray(np.asarray(Ww, f).T)
    xs = np.ascontiguousarray(np.asarray(data_input, f).reshape(B, CIN, N))
    cones = np.ones((128, 128), f)
    czero = np.zeros((128, 128), f)
    xpools = []
    for b in range(B):
        xp = np.zeros((CIN, 82), f)
        xp[:, 1:] = xs[b].reshape(CIN, 9, 7, 9, 7).sum(axis=(2, 4)).reshape(CIN, KK)
        xpools.append(xp)

    nc = _get_program()
    in_maps = []
    for c in range(8):
        b = c % 4
        q0 = (c // 4) * Q0STEP
        # roll so this core's queries sit at columns [0:QCNT); attention is
        # invariant to the key permutation, and the pooled path uses xpool
        xr = np.ascontiguousarray(np.roll(xs[b], -q0, axis=1))
        in_maps.append({
            "x": xr, "xpool": xpools[b], "wkT": wkT, "wvT": wvT,
            "wv2T": wv2T, "wwT": wwT, "bk2p": bk2p,
            "cones": cones, "czero": czero,
        })
    res = run_bass_kernel_spmd(nc, in_maps, list(range(8)))

    full = np.empty((B, CO, N), f)
    for b in range(B):
        full[b, :, :Q0STEP] = res.results[b]["out"][:, :Q0STEP]
        full[b, :, Q0STEP:] = res.results[4 + b]["out"][:, :QCNT]
    return full.reshape(B, CO, H, W)



# revision 5
# speedup vs baseline: 1.5655x; 1.5655x over previous
"""Trainium2 Bass kernel for EmbededNonLocalLayer (fp8 DoubleRow version).

Distribution: 8 cores = 4 batches x 2 query-halves. Each core holds its
batch's full keys; its query half sits at columns [0:2048) of a rolled x.

Math (per core), with host scales SK=16 (qk path), SW=64 (w2 path),
SS=32 (r1 ones), SS2=1024 (centered simv):
  qk8   = fp8(wk8^T x8 + bk2*SK)              [256, 4096]  (conv, DoubleRow)
  v2    = Wv2 @ (Wv @ xpool)/49               [256, 82]    (fp32, col0=0)
  w28   = fp8(Wv^T v2 * SW)                   [512, 82]    (val^T v2 == x^T w2)
  lgt2  = x8^T w28                            per 128-key block (DoubleRow)
  simv  = softmax_k(lgt2 * S/SW); dsimv8 = fp8((simv - 1/81)*SS2), col0 = SS
  E8    = fp8(exp(qk8^T qk8 * S/SK^2))        ACT exp or DVE Schraudolph bits
  o82   = dsimv8^T E8  (DoubleRow, PSUM accum over 32 key blocks)
          row0 = SS*r1 (softmax denom), rows 1:82 = SS2 * (dsimv^T E)
  out   = [(Ww*SS/SS2)^T (v2t^T o82) + wu^T x o82row0] * (1/o82row0)
          (wu = Ww @ v2.sum/81 restores the centered-simv mean term)
"""

import sys

sys.path.insert(0, "/opt/trn_rl_repo")

import numpy as np
import ml_dtypes

import concourse.bacc as bacc
import concourse.bass as bass
import concourse.mybir as mybir
from concourse.bass_utils import run_bass_kernel_spmd
from concourse.tile import TileContext

F32 = mybir.dt.float32
F32R = mybir.dt.float32r
FP8 = mybir.dt.float8e4
U8 = mybir.dt.uint8
AF = mybir.ActivationFunctionType
AX = mybir.AxisListType
DR = mybir.MatmulPerfMode.DoubleRow
OP = mybir.AluOpType
NPF8 = ml_dtypes.float8_e4m3

B, CIN, H, W = 4, 512, 63, 63
N = H * W            # 3969
NPAD = 4096
CI, CO = 256, 512
KK = 81
SCALE = 0.0625       # 1/sqrt(CI)
QCNT = 1985
QP = 2048
Q0STEP = 1984
MB = NPAD // 128     # 32 key blocks
SLABS = 8            # 512-column x slabs

SK = 16.0            # qk fp8 scale
SW = 64.0            # w2 fp8 scale
SS = 32.0            # ones column scale (r1 row)
SS2 = 1024.0         # centered-simv scale
EXP_SCALE = SCALE / (SK * SK)
LG2_SCALE = SCALE / SW
SIGMA = 0.35
A_SCH = 8.0 / np.log(2.0) * EXP_SCALE
B_SCH = 8.0 * 7.0 + SIGMA

# (qp, mb) units whose exp runs on DVE via Schraudolph bits; rest on ACT.
DVE_EXP = {(qp, mb) for qp in range(2) for mb in range(MB) if mb % 3 == 1}

_CACHE = {}


def _build_program(reps=1):
    nc = bacc.Bacc()

    x8_d = nc.dram_tensor("x8", [CIN, N], FP8, kind="ExternalInput")
    xp_d = nc.dram_tensor("xpool", [CIN, 82], F32R, kind="ExternalInput")
    wk8_d = nc.dram_tensor("wk8", [128, 1024], FP8, kind="ExternalInput")
    wv_d = nc.dram_tensor("wvT", [CIN, CI], F32R, kind="ExternalInput")
    wv2_d = nc.dram_tensor("wv2T", [CI, CI], F32R, kind="ExternalInput")
    wvO_d = nc.dram_tensor("wvO", [CI, CIN], F32R, kind="ExternalInput")
    wws_d = nc.dram_tensor("wws", [CI, CO], F32R, kind="ExternalInput")
    bk2s_d = nc.dram_tensor("bk2s", [128, 2], F32, kind="ExternalInput")
    ones_d = nc.dram_tensor("ones1", [1, 128], F32R, kind="ExternalInput")
    c8_d = nc.dram_tensor("c8ones", [128, 128], FP8, kind="ExternalInput")
    out_d = nc.dram_tensor("out", [CO, QP], F32, kind="ExternalOutput")

    with TileContext(nc) as tc, \
         nc.allow_low_precision(reason="fp8 attention validated numerically"):
      for _rep in range(reps):
        with tc.tile_pool(name=f"const{_rep}", bufs=1) as cpool, \
             tc.tile_pool(name=f"work{_rep}", bufs=1) as wpool:
          ones_sb = cpool.tile([1, 128], F32R)
          wk8_sb = cpool.tile([128, 1024], FP8)
          wv_sb = cpool.tile([128, 4 * CI], F32R)
          wv2_sb = cpool.tile([128, 2 * CI], F32R)
          wvO_sb = cpool.tile([128, 2 * CIN], F32R)
          wws_sb = cpool.tile([128, 2 * CO], F32R)
          bk2s_sb = cpool.tile([128, 2], F32)
          xp_sb = cpool.tile([128, 4 * 82], F32R)
          x8_sb = cpool.tile([128, 4 * NPAD], FP8)
          qk8_sb = cpool.tile([128, 2 * NPAD], FP8)
          dsimv8_sb = cpool.tile([128, MB * 82], FP8)
          pooled_sb = cpool.tile([128, 2 * 82], F32R)
          v2_sb = cpool.tile([128, 2 * 82], F32R)
          v2t_sb = cpool.tile([82, CI], F32R)
          w28_sb = cpool.tile([128, 4 * 82], FP8)
          r2_sb = cpool.tile([128, MB], F32)
          r2i2_sb = cpool.tile([128, MB], F32)
          v2s_sb = cpool.tile([128, 2], F32R)
          wu_sb = cpool.tile([1, CO], F32R)
          mask_sb = cpool.tile([128, 1], F32)

          x8_4 = x8_sb.rearrange("p (c n) -> p c n", c=4)
          qk3 = qk8_sb.rearrange("p (t n) -> p t n", t=2)
          w28_4 = w28_sb.rearrange("p (c k) -> p c k", c=4)
          dsim3 = dsimv8_sb.rearrange("p (m c) -> p m c", m=MB)

          # ---------- emission helpers ----------
          def emit_loads():
              nc.sync.dma_start(out=bk2s_sb[:], in_=bk2s_d[:])
              nc.sync.dma_start(out=wk8_sb[:], in_=wk8_d[:])
              for c in range(4):
                  nc.sync.dma_start(out=xp_sb[:, c * 82:(c + 1) * 82],
                                    in_=xp_d[c * 128:(c + 1) * 128, :])
              for c in range(4):
                  nc.sync.dma_start(out=wv_sb[:, c * CI:(c + 1) * CI],
                                    in_=wv_d[c * 128:(c + 1) * 128, :])
              for c in range(2):
                  nc.sync.dma_start(out=wv2_sb[:, c * CI:(c + 1) * CI],
                                    in_=wv2_d[c * 128:(c + 1) * 128, :])
                  nc.sync.dma_start(out=wvO_sb[:, c * CIN:(c + 1) * CIN],
                                    in_=wvO_d[c * 128:(c + 1) * 128, :])
              nc.sync.dma_start(out=ones_sb[:], in_=ones_d[:])

              def _slab_dma(s):
                  n0 = s * 512
                  rl = min(512, N - n0)
                  for cc in range(4):
                      nc.sync.dma_start(
                          out=x8_4[:, cc:cc + 1, n0:n0 + rl],
                          in_=x8_d[cc * 128:(cc + 1) * 128,
                                   n0:n0 + rl].rearrange(
                              "p (o n) -> p o n", o=1))

              for s in range(4):
                  _slab_dma(s)
              for c in range(2):
                  nc.sync.dma_start(out=wws_sb[:, c * CO:(c + 1) * CO],
                                    in_=wws_d[c * 128:(c + 1) * 128, :])
              nc.sync.dma_start(
                  out=dsim3[:, :, 0:1],
                  in_=c8_d[:, 0:MB].rearrange("p (m c) -> p m c", m=MB))
              for s in range(4, 8):
                  _slab_dma(s)
              for cc in range(4):
                  nc.gpsimd.memset(x8_4[:, cc:cc + 1, N:NPAD], 0.0)
              nc.gpsimd.memset(mask_sb[:], 0.0)
              nc.gpsimd.memset(mask_sb[0:1, :], 1.0)

          def emit_pooled_path(psH):
              for blk in range(2):
                  ps = psH.tile([128, 512], F32, tag="ps", name=f"pooled{blk}")
                  for cc in range(4):
                      nc.tensor.matmul(
                          ps[:, :82],
                          wv_sb[:, cc * CI + blk * 128:
                                cc * CI + blk * 128 + 128],
                          xp_sb[:, cc * 82:(cc + 1) * 82],
                          start=(cc == 0), stop=(cc == 3))
                  nc.vector.tensor_copy(pooled_sb[:, blk * 82:(blk + 1) * 82],
                                        ps[:, :82])
              for blk in range(2):
                  ps = psH.tile([128, 512], F32, tag="ps", name=f"v2_{blk}")
                  for cc in range(2):
                      nc.tensor.matmul(
                          ps[:, :82],
                          wv2_sb[:, cc * CI + blk * 128:
                                 cc * CI + blk * 128 + 128],
                          pooled_sb[:, cc * 82:(cc + 1) * 82],
                          start=(cc == 0), stop=(cc == 1))
                  nc.vector.tensor_copy(v2_sb[:, blk * 82:(blk + 1) * 82],
                                        ps[:, :82])
              ps = psH.tile([128, 512], F32, tag="ps", name="v2t")
              for cc in range(2):
                  nc.tensor.matmul(ps[:82, :CI],
                                   pooled_sb[:, cc * 82:(cc + 1) * 82],
                                   wv2_sb[:, cc * CI:(cc + 1) * CI],
                                   start=(cc == 0), stop=(cc == 1))
              nc.vector.tensor_copy(v2t_sb[:], ps[:82, :CI])
              # w2 = Wv^T v2 (contract ci), scaled into fp8
              for oc4 in range(4):
                  ps = psH.tile([128, 512], F32, tag="ps", name=f"w2_{oc4}")
                  for cc in range(2):
                      nc.tensor.matmul(
                          ps[:, :82],
                          wvO_sb[:, cc * CIN + oc4 * 128:
                                 cc * CIN + oc4 * 128 + 128],
                          v2_sb[:, cc * 82:(cc + 1) * 82],
                          start=(cc == 0), stop=(cc == 1))
                  nc.vector.tensor_scalar(
                      out=w28_sb[:, oc4 * 82:(oc4 + 1) * 82], in0=ps[:, :82],
                      scalar1=SW, scalar2=None, op0=OP.mult)
              # v2s = rowsum(v2) * SS2/(SS*81); wu = v2s^T wws  -> [1, CO]
              nc.vector.reduce_sum(
                  v2s_sb[:], v2_sb.rearrange("p (c k) -> p c k", c=2),
                  axis=AX.X)
              nc.vector.tensor_scalar(
                  out=v2s_sb[:], in0=v2s_sb[:],
                  scalar1=float(SS2 / (SS * 81.0)), scalar2=None, op0=OP.mult)
              ps = psH.tile([128, 512], F32, tag="ps", name="wu")
              for cc in range(2):
                  nc.tensor.matmul(ps[0:1, :CO], v2s_sb[:, cc:cc + 1],
                                   wws_sb[:, cc * CO:(cc + 1) * CO],
                                   start=(cc == 0), stop=(cc == 1))
              nc.vector.tensor_copy(wu_sb[:], ps[0:1, :CO])

          def emit_head_slab(psH, s):
              n0 = s * 512
              for blk in range(2):
                  ps = psH.tile([128, 512], F32, tag="ps", name=f"cv{s}_{blk}")
                  for pr in range(2):
                      lhsT = wk8_sb[:, blk * 512 + pr * 256:
                                    blk * 512 + pr * 256 + 256].rearrange(
                          "p (s o) -> p s o", s=2)
                      nc.tensor.matmul(
                          ps[:], lhsT,
                          x8_4[:, 2 * pr:2 * pr + 2, n0:n0 + 512],
                          start=(pr == 0), stop=(pr == 1), perf_mode=DR)
                  qslice = qk8_sb[:, blk * NPAD + n0:blk * NPAD + n0 + 512]
                  if blk == 0:
                      nc.scalar.activation(qslice, ps[:], AF.Identity,
                                           bias=bk2s_sb[:, blk:blk + 1])
                  else:
                      nc.vector.tensor_scalar(
                          out=qslice, in0=ps[:],
                          scalar1=bk2s_sb[:, blk:blk + 1], scalar2=None,
                          op0=OP.add)
              # logits2 / dsimv8 for the slab's 4 key blocks
              ps2 = psH.tile([128, 512], F32, tag="ps", name=f"lg{s}")
              for j in range(4):
                  mb = 4 * s + j
                  for pr in range(2):
                      nc.tensor.matmul(
                          ps2[:, j * 82:j * 82 + 82],
                          x8_4[:, 2 * pr:2 * pr + 2, mb * 128:mb * 128 + 128],
                          w28_4[:, 2 * pr:2 * pr + 2, :],
                          start=(pr == 0), stop=(pr == 1), perf_mode=DR)
              ex2 = wpool.tile([128, 4 * 82], F32, tag="ex2", bufs=2,
                               name=f"ex2_{s}")
              ps2v = ps2[:, 0:328].rearrange("p (g c) -> p g c", g=4)
              ex2v = ex2.rearrange("p (g c) -> p g c", g=4)
              nc.scalar.activation(ex2v[:, :, 1:82], ps2v[:, :, 1:82],
                                   AF.Exp, scale=LG2_SCALE)
              nc.vector.reduce_sum(r2_sb[:, 4 * s:4 * s + 4],
                                   ex2v[:, :, 1:82], axis=AX.X)
              nc.vector.reciprocal(r2i2_sb[:, 4 * s:4 * s + 4],
                                   r2_sb[:, 4 * s:4 * s + 4])
              nc.vector.tensor_scalar(
                  out=r2i2_sb[:, 4 * s:4 * s + 4],
                  in0=r2i2_sb[:, 4 * s:4 * s + 4],
                  scalar1=SS2, scalar2=None, op0=OP.mult)
              for j in range(4):
                  mb = 4 * s + j
                  nc.gpsimd.tensor_scalar(
                      out=dsim3[:, mb:mb + 1, 1:82],
                      in0=ex2v[:, j:j + 1, 1:82],
                      scalar1=r2i2_sb[:, mb:mb + 1],
                      scalar2=float(SS2 / 81.0),
                      op0=OP.mult, op1=OP.subtract)

          def emit_mask():
              nc.gpsimd.tensor_scalar(
                  out=dsim3[:, MB - 1:MB, :], in0=dsim3[:, MB - 1:MB, :],
                  scalar1=mask_sb[:, 0:1], scalar2=None, op0=OP.mult)

          def emit_pair(psJ, o82ps, qp, pairi):
              E8 = wpool.tile([128, 2048], FP8, tag="E8", bufs=3,
                              name=f"E8_{qp}_{pairi}")
              E83 = E8.rearrange("p (t n) -> p t n", t=2)
              for j in range(2):
                  mb = 2 * pairi + j
                  psL = psJ.tile([128, 1024], F32, tag="psL",
                                 name=f"psL_{qp}_{mb}")
                  for h in range(2):
                      nc.tensor.matmul(
                          psL[:, h * 512:h * 512 + 512],
                          qk3[:, :, mb * 128:mb * 128 + 128],
                          qk3[:, :, qp * 1024 + h * 512:
                              qp * 1024 + h * 512 + 512],
                          start=True, stop=True, perf_mode=DR)
                  dst = E8[:, j * 1024:j * 1024 + 1024]
                  if (qp, mb) in DVE_EXP:
                      nc.vector.tensor_scalar(
                          out=dst.bitcast(U8), in0=psL[:],
                          scalar1=float(A_SCH), scalar2=float(B_SCH),
                          op0=OP.mult, op1=OP.add)
                  else:
                      nc.scalar.activation(dst, psL[:], AF.Exp,
                                           scale=EXP_SCALE)
              for h in range(2):
                  nc.tensor.matmul(
                      o82ps[h][:],
                      dsim3[:, 2 * pairi:2 * pairi + 2, :],
                      E83[:, :, h * 512:h * 512 + 512],
                      start=(pairi == 0), stop=(pairi == 15), perf_mode=DR)

          def emit_tail(psT, o82ps, qp):
              for h in range(2):
                  qc = qp * 2 + h
                  o82 = wpool.tile([82, 512], F32R, tag="o82sb", bufs=2,
                                   name=f"o82_{qc}")
                  nc.scalar.copy(o82[:], o82ps[h][:])
                  rc = wpool.tile([1, 512], F32R, tag="rc", bufs=2,
                                  name=f"rc_{qc}")
                  nc.vector.reciprocal(rc[:], o82ps[h][0:1, :])
                  bps = psT.tile([128, 512], F32, tag="tail", name=f"bps_{qc}")
                  nc.tensor.matmul(bps[:], ones_sb[:], rc[:],
                                   start=True, stop=True)
                  bc = wpool.tile([128, 512], F32, tag="bc", bufs=2,
                                  name=f"bc_{qc}")
                  nc.scalar.copy(bc[:], bps[:])
                  ctx = wpool.tile([128, 2 * 512], F32R, tag="ctx", bufs=2,
                                   name=f"ctx_{qc}")
                  for c2 in range(2):
                      cps = psT.tile([128, 512], F32, tag="tail",
                                     name=f"cps_{qc}_{c2}")
                      nc.tensor.matmul(cps[:],
                                       v2t_sb[:, c2 * 128:(c2 + 1) * 128],
                                       o82[0:82, :], start=True, stop=True)
                      nc.vector.tensor_copy(ctx[:, c2 * 512:(c2 + 1) * 512],
                                            cps[:])
                  for ob in range(4):
                      ops_ = psT.tile([128, 512], F32, tag="tail",
                                      name=f"ops_{qc}_{ob}")
                      for cc in range(2):
                          nc.tensor.matmul(
                              ops_[:],
                              wws_sb[:, cc * CO + ob * 128:
                                     cc * CO + ob * 128 + 128],
                              ctx[:, cc * 512:(cc + 1) * 512],
                              start=(cc == 0), stop=False)
                      # rank-1: + wu^T x row0 so *1/row0 leaves +Ww@v2sum/81
                      nc.tensor.matmul(
                          ops_[:], wu_sb[:, ob * 128:(ob + 1) * 128],
                          o82[0:1, :], start=False, stop=True)
                      outb = wpool.tile([128, 512], F32, tag="outb", bufs=3,
                                        name=f"outb_{qc}_{ob}")
                      nc.vector.tensor_tensor(outb[:], ops_[:], bc[:],
                                              op=OP.mult)
                      nc.sync.dma_start(
                          out=out_d[ob * 128:(ob + 1) * 128,
                                    qc * 512:(qc + 1) * 512],
                          in_=outb[:])

          # ---------- emission schedule ----------
          with tc.tile_pool(name="psJ", bufs=2, space="PSUM") as psJ, \
               tc.tile_pool(name="psO", bufs=2, space="PSUM") as psO:
              o82_qp0 = [psO.tile([82, 512], F32, tag="o82",
                                  name=f"o82ps_0_{h}") for h in range(2)]
              with tc.tile_pool(name="psHead", bufs=2, space="PSUM") as psH:
                  emit_loads()
                  emit_pooled_path(psH)
                  emit_head_slab(psH, 0)
                  emit_head_slab(psH, 1)
                  # lag-one interleave: after slab s, pairs 2(s-1), 2(s-1)+1
                  for s in range(2, 8):
                      emit_pair(psJ, o82_qp0, 0, 2 * (s - 2))
                      emit_pair(psJ, o82_qp0, 0, 2 * (s - 2) + 1)
                      emit_head_slab(psH, s)
                  emit_mask()
                  for pairi in range(12, 16):
                      emit_pair(psJ, o82_qp0, 0, pairi)
              with tc.tile_pool(name="psT", bufs=2, space="PSUM") as psT:
                  emit_tail(psT, o82_qp0, 0)
                  o82_qp1 = [psO.tile([82, 512], F32, tag="o82",
                                      name=f"o82ps_1_{h}") for h in range(2)]
                  for pairi in range(16):
                      emit_pair(psJ, o82_qp1, 1, pairi)
                  emit_tail(psT, o82_qp1, 1)

    nc.finalize()
    return nc


def _get_program(reps=1):
    if ("nc", reps) not in _CACHE:
        _CACHE[("nc", reps)] = _build_program(reps)
    return _CACHE[("nc", reps)]


def _host_inputs(data_input, Wk, bk, gamma, beta, Wv, bv, Wv2, bv2, Ww, bw):
    f = np.float32
    for name, bias in (("bv", bv), ("bv2", bv2), ("bw", bw)):
        if not np.allclose(np.asarray(bias), 0.0):
            raise NotImplementedError(f"{name} != 0 not supported")
    s = (np.asarray(gamma, f) / np.sqrt(f(1.0) + f(1e-5))).astype(f)
    wk_s = (np.asarray(Wk, f) * s[:, None]) * f(SK)     # [CI, CIN]
    bk2s = ((np.asarray(bk, f) * s + np.asarray(beta, f)) * f(SK)).astype(f)

    # wk8 packed layout: [p, blk*512 + pair*256 + slot*128 + oc]
    # cin = pair*256 + slot*128 + p ; oc_global = blk*128 + oc
    wk8 = np.zeros((128, 1024), NPF8)
    wkT = np.ascontiguousarray(wk_s.T)                  # [CIN, CI]
    for blk in range(2):
        for pr in range(2):
            for sl in range(2):
                cin0 = pr * 256 + sl * 128
                col0 = blk * 512 + pr * 256 + sl * 128
                wk8[:, col0:col0 + 128] = wkT[
                    cin0:cin0 + 128, blk * 128:blk * 128 + 128].astype(NPF8)

    wvT = np.ascontiguousarray(np.asarray(Wv, f).T)
    wv2T = np.ascontiguousarray((np.asarray(Wv2, f) / f(49.0)).T)
    wvO = np.ascontiguousarray(np.asarray(Wv, f))
    wws = np.ascontiguousarray(np.asarray(Ww, f).T * f(SS / SS2))
    xs = np.ascontiguousarray(np.asarray(data_input, f).reshape(B, CIN, N))
    ones1 = np.ones((1, 128), f)
    c8 = np.full((128, 128), SS, NPF8)
    x8s = [np.ascontiguousarray(xs[b].astype(NPF8)) for b in range(B)]
    xpools = []
    for b in range(B):
        xp = np.zeros((CIN, 82), f)
        xp[:, 1:] = xs[b].reshape(CIN, 9, 7, 9, 7).sum(axis=(2, 4)).reshape(
            CIN, KK)
        xpools.append(xp)
    bk2p = np.ascontiguousarray(bk2s.reshape(2, 128).T)

    in_maps = []
    for c in range(8):
        b = c % 4
        q0 = (c // 4) * Q0STEP
        xr = np.ascontiguousarray(np.roll(x8s[b], -q0, axis=1))
        in_maps.append({
            "x8": xr, "xpool": xpools[b], "wk8": wk8, "wvT": wvT,
            "wv2T": wv2T, "wvO": wvO, "wws": wws, "bk2s": bk2p,
            "ones1": ones1, "c8ones": c8,
        })
    return in_maps


def kernel(data_input, Wk, bk, gamma, beta, Wv, bv, Wv2, bv2, Ww, bw):
    f = np.float32
    in_maps = _host_inputs(data_input, Wk, bk, gamma, beta, Wv, bv, Wv2,
                           bv2, Ww, bw)
    nc = _get_program()
    res = run_bass_kernel_spmd(nc, in_maps, list(range(8)))

    full = np.empty((B, CO, N), f)
    for b in range(B):
        full[b, :, :Q0STEP] = res.results[b]["out"][:, :Q0STEP]
        full[b, :, Q0STEP:] = res.results[4 + b]["out"][:, :QCNT]
    return full.reshape(B, CO, H, W)


# revision 7
# speedup vs baseline: 1.8077x; 1.1547x over previous
"""Trainium2 Bass kernel for EmbededNonLocalLayer (fp8 DoubleRow version).

Distribution: 8 cores = 4 batches x 2 query-halves. Each core holds its
batch's full keys; its query half sits at columns [0:2048) of a rolled x.

Math (per core), with host scales SK=16 (qk path), SW=64 (w2 path),
SS=32 (r1 ones), SS2=1024 (centered simv):
  qk8   = fp8(wk8^T x8 + bk2*SK)              [256, 4096]  (conv, DoubleRow)
  v2    = Wv2 @ (Wv @ xpool)/49               [256, 82]    (fp32, col0=0)
  w28   = fp8(Wv^T v2 * SW)                   [512, 82]    (val^T v2 == x^T w2)
  lgt2  = x8^T w28                            per 128-key block (DoubleRow)
  simv  = softmax_k(lgt2 * S/SW); dsimv8 = fp8((simv - 1/81)*SS2), col0 = SS
  E8    = fp8(exp(qk8^T qk8 * S/SK^2))        ACT exp or DVE Schraudolph bits
  o82   = dsimv8^T E8  (DoubleRow, PSUM accum over 32 key blocks)
          row0 = SS*r1 (softmax denom), rows 1:82 = SS2 * (dsimv^T E)
  ctx   = (v2t^T o82) * (1/row0 bcast);  out = (Ww*SS/SS2)^T ctx + wu^T x 1s
          (wu = Ww @ v2.sum/81 restores the centered-simv mean term)
"""

import sys

sys.path.insert(0, "/opt/trn_rl_repo")

import numpy as np
import ml_dtypes

import concourse.bacc as bacc
import concourse.bass as bass
import concourse.mybir as mybir
from concourse.bass_utils import run_bass_kernel_spmd
from concourse.tile import TileContext

F32 = mybir.dt.float32
F32R = mybir.dt.float32r
FP8 = mybir.dt.float8e4
U8 = mybir.dt.uint8
AF = mybir.ActivationFunctionType
AX = mybir.AxisListType
DR = mybir.MatmulPerfMode.DoubleRow
OP = mybir.AluOpType
NPF8 = ml_dtypes.float8_e4m3

B, CIN, H, W = 4, 512, 63, 63
N = H * W            # 3969
NPAD = 4096
CI, CO = 256, 512
KK = 81
SCALE = 0.0625       # 1/sqrt(CI)
QCNT = 1985
QP = 2048
Q0STEP = 1984
MB = NPAD // 128     # 32 key blocks
SLABS = 8            # 512-column x slabs

SK = 16.0            # qk fp8 scale
SW = 64.0            # w2 fp8 scale
SS = 32.0            # ones column scale (r1 row)
SS2 = 1024.0         # centered-simv scale
EXP_SCALE = SCALE / (SK * SK)
LG2_SCALE = SCALE / SW
SIGMA = 0.35
A_SCH = 8.0 / np.log(2.0) * EXP_SCALE
B_SCH = 8.0 * 7.0 + SIGMA

# (qp, mb) units whose exp runs on DVE via Schraudolph bits; rest on ACT.
DVE_EXP = {(qp, mb) for qp in range(2) for mb in range(MB) if mb % 3 == 1}

_CACHE = {}


def _build_program(reps=1):
    nc = bacc.Bacc()

    x8_d = nc.dram_tensor("x8", [CIN, N], FP8, kind="ExternalInput")
    xp_d = nc.dram_tensor("xpool", [CIN, 82], F32R, kind="ExternalInput")
    wk8_d = nc.dram_tensor("wk8", [128, 1024], FP8, kind="ExternalInput")
    wv_d = nc.dram_tensor("wvT", [CIN, CI], F32R, kind="ExternalInput")
    wv2_d = nc.dram_tensor("wv2T", [CI, CI], F32R, kind="ExternalInput")
    wvO_d = nc.dram_tensor("wvO", [CI, CIN], F32R, kind="ExternalInput")
    wws_d = nc.dram_tensor("wws", [CI, CO], F32R, kind="ExternalInput")
    bk2s_d = nc.dram_tensor("bk2s", [128, 2], F32, kind="ExternalInput")
    ones_d = nc.dram_tensor("ones1", [1, 512], F32R, kind="ExternalInput")
    c8_d = nc.dram_tensor("c8ones", [128, 128], FP8, kind="ExternalInput")
    out_d = nc.dram_tensor("out", [CO, QP], F32, kind="ExternalOutput")

    with TileContext(nc) as tc, \
         nc.allow_low_precision(reason="fp8 attention validated numerically"):
      for _rep in range(reps):
        with tc.tile_pool(name=f"const{_rep}", bufs=1) as cpool, \
             tc.tile_pool(name=f"work{_rep}", bufs=1) as wpool:
          ones_sb = cpool.tile([1, 512], F32R)
          wk8_sb = cpool.tile([128, 1024], FP8)
          wv_sb = cpool.tile([128, 4 * CI], F32R)
          wv2_sb = cpool.tile([128, 2 * CI], F32R)
          wvO_sb = cpool.tile([128, 2 * CIN], F32R)
          wws_sb = cpool.tile([128, 2 * CO], F32R)
          bk2s_sb = cpool.tile([128, 2], F32)
          xp_sb = cpool.tile([128, 4 * 82], F32R)
          x8_sb = cpool.tile([128, 4 * NPAD], FP8)
          qk8_sb = cpool.tile([128, 2 * NPAD], FP8)
          dsimv8_sb = cpool.tile([128, MB * 82], FP8)
          pooled_sb = cpool.tile([128, 2 * 82], F32R)
          v2_sb = cpool.tile([128, 2 * 82], F32R)
          v2t_sb = cpool.tile([82, CI], F32R)
          w28_sb = cpool.tile([128, 4 * 82], FP8)
          r2_sb = cpool.tile([128, MB], F32)
          r2i2_sb = cpool.tile([128, MB], F32)
          v2s_sb = cpool.tile([128, 2], F32R)
          wu_sb = cpool.tile([1, CO], F32R)
          mask_sb = cpool.tile([128, 1], F32)

          x8_4 = x8_sb.rearrange("p (c n) -> p c n", c=4)
          qk3 = qk8_sb.rearrange("p (t n) -> p t n", t=2)
          w28_4 = w28_sb.rearrange("p (c k) -> p c k", c=4)
          dsim3 = dsimv8_sb.rearrange("p (m c) -> p m c", m=MB)

          # ---------- emission helpers ----------
          def _slab_dma(s):
              n0 = s * 512
              rl = min(512, N - n0)
              nc.sync.dma_start(
                  out=x8_4[:, :, n0:n0 + rl],
                  in_=x8_d.rearrange("(c p) n -> p c n", c=4)[:, :, n0:n0 + rl])

          def emit_loads_early():
              nc.sync.dma_start(out=wk8_sb[:], in_=wk8_d[:])
              _slab_dma(0)
              _slab_dma(1)
              nc.sync.dma_start(out=bk2s_sb[:], in_=bk2s_d[:])
              nc.sync.dma_start(
                  out=xp_sb.rearrange("p (c k) -> p c k", c=4),
                  in_=xp_d.rearrange("(c p) k -> p c k", c=4))
              nc.sync.dma_start(
                  out=wv_sb.rearrange("p (c k) -> p c k", c=4),
                  in_=wv_d.rearrange("(c p) k -> p c k", c=4))
              nc.sync.dma_start(
                  out=wv2_sb.rearrange("p (c k) -> p c k", c=2),
                  in_=wv2_d.rearrange("(c p) k -> p c k", c=2))
              nc.sync.dma_start(
                  out=wvO_sb.rearrange("p (c k) -> p c k", c=2),
                  in_=wvO_d.rearrange("(c p) k -> p c k", c=2))
              _slab_dma(2)
              _slab_dma(3)
              nc.sync.dma_start(
                  out=wws_sb.rearrange("p (c k) -> p c k", c=2),
                  in_=wws_d.rearrange("(c p) k -> p c k", c=2))
              nc.sync.dma_start(out=ones_sb[:], in_=ones_d[:])
              nc.sync.dma_start(
                  out=dsim3[:, :, 0:1],
                  in_=c8_d[:, 0:MB].rearrange("p (m c) -> p m c", m=MB))
              for s in range(4, 8):
                  _slab_dma(s)
              for cc in range(4):
                  nc.gpsimd.memset(x8_4[:, cc:cc + 1, N:NPAD], 0.0)
              nc.gpsimd.memset(mask_sb[:], 0.0)
              nc.gpsimd.memset(mask_sb[0:1, :], 1.0)

          def emit_conv(psH, s):
              n0 = s * 512
              for blk in range(2):
                  ps = psH.tile([128, 512], F32, tag="ps", name=f"cv{s}_{blk}")
                  for pr in range(2):
                      lhsT = wk8_sb[:, blk * 512 + pr * 256:
                                    blk * 512 + pr * 256 + 256].rearrange(
                          "p (s o) -> p s o", s=2)
                      nc.tensor.matmul(
                          ps[:], lhsT,
                          x8_4[:, 2 * pr:2 * pr + 2, n0:n0 + 512],
                          start=(pr == 0), stop=(pr == 1), perf_mode=DR)
                  qslice = qk8_sb[:, blk * NPAD + n0:blk * NPAD + n0 + 512]
                  if blk == 0:
                      nc.scalar.activation(qslice, ps[:], AF.Identity,
                                           bias=bk2s_sb[:, blk:blk + 1])
                  else:
                      nc.vector.tensor_scalar(
                          out=qslice, in0=ps[:],
                          scalar1=bk2s_sb[:, blk:blk + 1], scalar2=None,
                          op0=OP.add)

          def emit_pooled_path(psH):
              for blk in range(2):
                  ps = psH.tile([128, 512], F32, tag="ps", name=f"pooled{blk}")
                  for cc in range(4):
                      nc.tensor.matmul(
                          ps[:, :82],
                          wv_sb[:, cc * CI + blk * 128:
                                cc * CI + blk * 128 + 128],
                          xp_sb[:, cc * 82:(cc + 1) * 82],
                          start=(cc == 0), stop=(cc == 3))
                  nc.vector.tensor_copy(pooled_sb[:, blk * 82:(blk + 1) * 82],
                                        ps[:, :82])
              for blk in range(2):
                  ps = psH.tile([128, 512], F32, tag="ps", name=f"v2_{blk}")
                  for cc in range(2):
                      nc.tensor.matmul(
                          ps[:, :82],
                          wv2_sb[:, cc * CI + blk * 128:
                                 cc * CI + blk * 128 + 128],
                          pooled_sb[:, cc * 82:(cc + 1) * 82],
                          start=(cc == 0), stop=(cc == 1))
                  nc.vector.tensor_copy(v2_sb[:, blk * 82:(blk + 1) * 82],
                                        ps[:, :82])
              # w2 = Wv^T v2 (contract ci), scaled into fp8
              for oc4 in range(4):
                  ps = psH.tile([128, 512], F32, tag="ps", name=f"w2_{oc4}")
                  for cc in range(2):
                      nc.tensor.matmul(
                          ps[:, :82],
                          wvO_sb[:, cc * CIN + oc4 * 128:
                                 cc * CIN + oc4 * 128 + 128],
                          v2_sb[:, cc * 82:(cc + 1) * 82],
                          start=(cc == 0), stop=(cc == 1))
                  nc.vector.tensor_scalar(
                      out=w28_sb[:, oc4 * 82:(oc4 + 1) * 82], in0=ps[:, :82],
                      scalar1=SW, scalar2=None, op0=OP.mult)

          def emit_tail_consts(psH):
              ps = psH.tile([128, 512], F32, tag="ps", name="v2t")
              for cc in range(2):
                  nc.tensor.matmul(ps[:82, :CI],
                                   pooled_sb[:, cc * 82:(cc + 1) * 82],
                                   wv2_sb[:, cc * CI:(cc + 1) * CI],
                                   start=(cc == 0), stop=(cc == 1))
              nc.vector.tensor_copy(v2t_sb[:], ps[:82, :CI])
              # v2s = rowsum(v2) * SS2/(SS*81); wu = v2s^T wws  -> [1, CO]
              nc.vector.reduce_sum(
                  v2s_sb[:], v2_sb.rearrange("p (c k) -> p c k", c=2),
                  axis=AX.X)
              nc.vector.tensor_scalar(
                  out=v2s_sb[:], in0=v2s_sb[:],
                  scalar1=float(SS2 / (SS * 81.0)), scalar2=None, op0=OP.mult)
              ps = psH.tile([128, 512], F32, tag="ps", name="wu")
              for cc in range(2):
                  nc.tensor.matmul(ps[0:1, :CO], v2s_sb[:, cc:cc + 1],
                                   wws_sb[:, cc * CO:(cc + 1) * CO],
                                   start=(cc == 0), stop=(cc == 1))
              nc.vector.tensor_copy(wu_sb[:], ps[0:1, :CO])

          def emit_lg2(psH, s):
              ps2 = psH.tile([128, 512], F32, tag="ps", name=f"lg{s}")
              for j in range(4):
                  mb = 4 * s + j
                  for pr in range(2):
                      nc.tensor.matmul(
                          ps2[:, j * 82:j * 82 + 82],
                          x8_4[:, 2 * pr:2 * pr + 2, mb * 128:mb * 128 + 128],
                          w28_4[:, 2 * pr:2 * pr + 2, :],
                          start=(pr == 0), stop=(pr == 1), perf_mode=DR)
              ex2 = wpool.tile([128, 4 * 82], F32, tag="ex2", bufs=2,
                               name=f"ex2_{s}")
              ps2v = ps2[:, 0:328].rearrange("p (g c) -> p g c", g=4)
              ex2v = ex2.rearrange("p (g c) -> p g c", g=4)
              nc.scalar.activation(ex2v[:, :, 1:82], ps2v[:, :, 1:82],
                                   AF.Exp, scale=LG2_SCALE)
              nc.vector.reduce_sum(r2_sb[:, 4 * s:4 * s + 4],
                                   ex2v[:, :, 1:82], axis=AX.X)
              nc.vector.reciprocal(r2i2_sb[:, 4 * s:4 * s + 4],
                                   r2_sb[:, 4 * s:4 * s + 4])
              nc.vector.tensor_scalar(
                  out=r2i2_sb[:, 4 * s:4 * s + 4],
                  in0=r2i2_sb[:, 4 * s:4 * s + 4],
                  scalar1=SS2, scalar2=None, op0=OP.mult)
              for j in range(4):
                  mb = 4 * s + j
                  nc.gpsimd.tensor_scalar(
                      out=dsim3[:, mb:mb + 1, 1:82],
                      in0=ex2v[:, j:j + 1, 1:82],
                      scalar1=r2i2_sb[:, mb:mb + 1],
                      scalar2=float(SS2 / 81.0),
                      op0=OP.mult, op1=OP.subtract)

          def emit_mask():
              nc.gpsimd.tensor_scalar(
                  out=dsim3[:, MB - 1:MB, :], in0=dsim3[:, MB - 1:MB, :],
                  scalar1=mask_sb[:, 0:1], scalar2=None, op0=OP.mult)

          def emit_pair(psJ, o82ps, qp, pairi):
              E8 = wpool.tile([128, 2048], FP8, tag="E8", bufs=3,
                              name=f"E8_{qp}_{pairi}")
              E83 = E8.rearrange("p (t n) -> p t n", t=2)
              for j in range(2):
                  mb = 2 * pairi + j
                  psL = psJ.tile([128, 1024], F32, tag="psL",
                                 name=f"psL_{qp}_{mb}")
                  for h in range(2):
                      nc.tensor.matmul(
                          psL[:, h * 512:h * 512 + 512],
                          qk3[:, :, mb * 128:mb * 128 + 128],
                          qk3[:, :, qp * 1024 + h * 512:
                              qp * 1024 + h * 512 + 512],
                          start=True, stop=True, perf_mode=DR)
                  dst = E8[:, j * 1024:j * 1024 + 1024]
                  if (qp, mb) in DVE_EXP:
                      nc.vector.tensor_scalar(
                          out=dst.bitcast(U8), in0=psL[:],
                          scalar1=float(A_SCH), scalar2=float(B_SCH),
                          op0=OP.mult, op1=OP.add)
                  else:
                      nc.scalar.activation(dst, psL[:], AF.Exp,
                                           scale=EXP_SCALE)
              for h in range(2):
                  nc.tensor.matmul(
                      o82ps[h][:],
                      dsim3[:, 2 * pairi:2 * pairi + 2, :],
                      E83[:, :, h * 512:h * 512 + 512],
                      start=(pairi == 0), stop=(pairi == 15), perf_mode=DR)

          def emit_tail(psT, o82ps, qp):
              for h in range(2):
                  qc = qp * 2 + h
                  o82 = wpool.tile([82, 512], F32R, tag="o82sb", bufs=2,
                                   name=f"o82_{qc}")
                  nc.scalar.copy(o82[:], o82ps[h][:])
                  rc = wpool.tile([1, 512], F32R, tag="rc", bufs=2,
                                  name=f"rc_{qc}")
                  nc.vector.reciprocal(rc[:], o82ps[h][0:1, :])
                  bps = psT.tile([128, 512], F32, tag="tail", name=f"bps_{qc}")
                  nc.tensor.matmul(bps[:], ones_sb[0:1, 0:128], rc[:],
                                   start=True, stop=True)
                  bc = wpool.tile([128, 512], F32, tag="bc", bufs=2,
                                  name=f"bc_{qc}")
                  nc.scalar.copy(bc[:], bps[:])
                  ctx = wpool.tile([128, 2 * 512], F32R, tag="ctx", bufs=2,
                                   name=f"ctx_{qc}")
                  for c2 in range(2):
                      cps = psT.tile([128, 512], F32, tag="tail",
                                     name=f"cps_{qc}_{c2}")
                      nc.tensor.matmul(cps[:],
                                       v2t_sb[:, c2 * 128:(c2 + 1) * 128],
                                       o82[0:82, :], start=True, stop=True)
                      # fold the softmax normalization into ctx
                      nc.vector.tensor_tensor(ctx[:, c2 * 512:(c2 + 1) * 512],
                                              cps[:], bc[:], op=OP.mult)
                  for ob in range(4):
                      ops_ = psT.tile([128, 512], F32, tag="tail",
                                      name=f"ops_{qc}_{ob}")
                      for cc in range(2):
                          nc.tensor.matmul(
                              ops_[:],
                              wws_sb[:, cc * CO + ob * 128:
                                     cc * CO + ob * 128 + 128],
                              ctx[:, cc * 512:(cc + 1) * 512],
                              start=(cc == 0), stop=False)
                      # rank-1 mean restore: + wu^T x ones
                      nc.tensor.matmul(
                          ops_[:], wu_sb[:, ob * 128:(ob + 1) * 128],
                          ones_sb[:], start=False, stop=True)
                      outb = wpool.tile([128, 512], F32, tag="outb", bufs=3,
                                        name=f"outb_{qc}_{ob}")
                      if ob % 2 == 0:
                          nc.scalar.copy(outb[:], ops_[:])
                      else:
                          nc.vector.tensor_copy(outb[:], ops_[:])
                      nc.sync.dma_start(
                          out=out_d[ob * 128:(ob + 1) * 128,
                                    qc * 512:(qc + 1) * 512],
                          in_=outb[:])

          # ---------- emission schedule ----------
          with tc.tile_pool(name="psJ", bufs=2, space="PSUM") as psJ, \
               tc.tile_pool(name="psO", bufs=2, space="PSUM") as psO:
              o82_qp0 = [psO.tile([82, 512], F32, tag="o82",
                                  name=f"o82ps_0_{h}") for h in range(2)]
              with tc.tile_pool(name="psHead", bufs=2, space="PSUM") as psH:
                  emit_loads_early()
                  emit_conv(psH, 0)
                  emit_conv(psH, 1)
                  emit_pooled_path(psH)
                  emit_lg2(psH, 0)
                  emit_lg2(psH, 1)
                  # lag-one interleave: after slab s, pairs 2(s-1), 2(s-1)+1
                  for s in range(2, 8):
                      emit_pair(psJ, o82_qp0, 0, 2 * (s - 2))
                      emit_pair(psJ, o82_qp0, 0, 2 * (s - 2) + 1)
                      emit_conv(psH, s)
                      emit_lg2(psH, s)
                  emit_mask()
                  emit_pair(psJ, o82_qp0, 0, 12)
                  emit_pair(psJ, o82_qp0, 0, 13)
                  emit_tail_consts(psH)
                  emit_pair(psJ, o82_qp0, 0, 14)
                  emit_pair(psJ, o82_qp0, 0, 15)
              with tc.tile_pool(name="psT", bufs=2, space="PSUM") as psT:
                  emit_tail(psT, o82_qp0, 0)
                  o82_qp1 = [psO.tile([82, 512], F32, tag="o82",
                                      name=f"o82ps_1_{h}") for h in range(2)]
                  for pairi in range(16):
                      emit_pair(psJ, o82_qp1, 1, pairi)
                  emit_tail(psT, o82_qp1, 1)

    nc.finalize()
    return nc


def _get_program(reps=1):
    if ("nc", reps) not in _CACHE:
        _CACHE[("nc", reps)] = _build_program(reps)
    return _CACHE[("nc", reps)]


def _host_inputs(data_input, Wk, bk, gamma, beta, Wv, bv, Wv2, bv2, Ww, bw):
    f = np.float32
    for name, bias in (("bv", bv), ("bv2", bv2), ("bw", bw)):
        if not np.allclose(np.asarray(bias), 0.0):
            raise NotImplementedError(f"{name} != 0 not supported")
    s = (np.asarray(gamma, f) / np.sqrt(f(1.0) + f(1e-5))).astype(f)
    wk_s = (np.asarray(Wk, f) * s[:, None]) * f(SK)     # [CI, CIN]
    bk2s = ((np.asarray(bk, f) * s + np.asarray(beta, f)) * f(SK)).astype(f)

    # wk8 packed layout: [p, blk*512 + pair*256 + slot*128 + oc]
    # cin = pair*256 + slot*128 + p ; oc_global = blk*128 + oc
    wk8 = np.zeros((128, 1024), NPF8)
    wkT = np.ascontiguousarray(wk_s.T)                  # [CIN, CI]
    for blk in range(2):
        for pr in range(2):
            for sl in range(2):
                cin0 = pr * 256 + sl * 128
                col0 = blk * 512 + pr * 256 + sl * 128
                wk8[:, col0:col0 + 128] = wkT[
                    cin0:cin0 + 128, blk * 128:blk * 128 + 128].astype(NPF8)

    wvT = np.ascontiguousarray(np.asarray(Wv, f).T)
    wv2T = np.ascontiguousarray((np.asarray(Wv2, f) / f(49.0)).T)
    wvO = np.ascontiguousarray(np.asarray(Wv, f))
    wws = np.ascontiguousarray(np.asarray(Ww, f).T * f(SS / SS2))
    xs = np.ascontiguousarray(np.asarray(data_input, f).reshape(B, CIN, N))
    ones1 = np.ones((1, 512), f)
    c8 = np.full((128, 128), SS, NPF8)
    x8s = [np.ascontiguousarray(xs[b].astype(NPF8)) for b in range(B)]
    xpools = []
    for b in range(B):
        xp = np.zeros((CIN, 82), f)
        xp[:, 1:] = xs[b].reshape(CIN, 9, 7, 9, 7).sum(axis=(2, 4)).reshape(
            CIN, KK)
        xpools.append(xp)
    bk2p = np.ascontiguousarray(bk2s.reshape(2, 128).T)

    in_maps = []
    for c in range(8):
        b = c % 4
        q0 = (c // 4) * Q0STEP
        xr = np.ascontiguousarray(np.roll(x8s[b], -q0, axis=1))
        in_maps.append({
            "x8": xr, "xpool": xpools[b], "wk8": wk8, "wvT": wvT,
            "wv2T": wv2T, "wvO": wvO, "wws": wws, "bk2s": bk2p,
            "ones1": ones1, "c8ones": c8,
        })
    return in_maps


def kernel(data_input, Wk, bk, gamma, beta, Wv, bv, Wv2, bv2, Ww, bw):
    f = np.float32
    in_maps = _host_inputs(data_input, Wk, bk, gamma, beta, Wv, bv, Wv2,
                           bv2, Ww, bw)
    nc = _get_program()
    res = run_bass_kernel_spmd(nc, in_maps, list(range(8)))

    full = np.empty((B, CO, N), f)
    for b in range(B):
        full[b, :, :Q0STEP] = res.results[b]["out"][:, :Q0STEP]
        full[b, :, Q0STEP:] = res.results[4 + b]["out"][:, :QCNT]
    return full.reshape(B, CO, H, W)


# revision 10
# speedup vs baseline: 1.8207x; 1.0072x over previous
"""Trainium2 Bass kernel for EmbededNonLocalLayer (fp8 DoubleRow version).

Distribution: 8 cores = 4 batches x 2 query-halves. Each core holds its
batch's full keys; its query half sits at columns [0:2048) of a rolled x.

Math (per core), with host scales SK=16 (qk path), SW=64 (w2 path),
SS=32 (r1 ones), SS2=1024 (centered simv):
  qk8   = fp8(wk8^T x8 + bk2*SK)              [256, 4096]  (conv, DoubleRow)
  v2    = Wv2 @ (Wv @ xpool)/49               [256, 82]    (fp32, col0=0)
  w28   = fp8(Wv^T v2 * SW)                   [512, 82]    (val^T v2 == x^T w2)
  lgt2  = x8^T w28                            per 128-key block (DoubleRow)
  simv  = softmax_k(lgt2 * S/SW); dsimv8 = fp8((simv - 1/81)*SS2), col0 = SS
  E8    = fp8(exp(qk8^T qk8 * S/SK^2))        ACT exp or DVE Schraudolph bits
  o82   = dsimv8^T E8  (DoubleRow, PSUM accum over 32 key blocks)
          row0 = SS*r1 (softmax denom), rows 1:82 = SS2 * (dsimv^T E)
  ctx   = (v2t^T o82) * (1/row0 bcast);  out = (Ww*SS/SS2)^T ctx + wu^T x 1s
          (wu = Ww @ v2.sum/81 restores the centered-simv mean term)
"""

import sys

sys.path.insert(0, "/opt/trn_rl_repo")

import numpy as np
import ml_dtypes

import concourse.bacc as bacc
import concourse.bass as bass
import concourse.mybir as mybir
from concourse.bass_utils import run_bass_kernel_spmd
from concourse.tile import TileContext

F32 = mybir.dt.float32
F32R = mybir.dt.float32r
FP8 = mybir.dt.float8e4
U8 = mybir.dt.uint8
AF = mybir.ActivationFunctionType
AX = mybir.AxisListType
DR = mybir.MatmulPerfMode.DoubleRow
OP = mybir.AluOpType
NPF8 = ml_dtypes.float8_e4m3

B, CIN, H, W = 4, 512, 63, 63
N = H * W            # 3969
NPAD = 4096
CI, CO = 256, 512
KK = 81
SCALE = 0.0625       # 1/sqrt(CI)
QCNT = 1985
QP = 2048
Q0STEP = 1984
MB = NPAD // 128     # 32 key blocks
SLABS = 8            # 512-column x slabs

SK = 16.0            # qk fp8 scale
SW = 64.0            # w2 fp8 scale
SS = 32.0            # ones column scale (r1 row)
SS2 = 1024.0         # centered-simv scale
EXP_SCALE = SCALE / (SK * SK)
LG2_SCALE = SCALE / SW
SIGMA = 0.35
A_SCH = 8.0 / np.log(2.0) * EXP_SCALE
B_SCH = 8.0 * 7.0 + SIGMA

# (qp, mb) units whose exp runs on DVE via Schraudolph bits; rest on ACT.
DVE_EXP = {(qp, mb) for qp in range(2) for mb in range(MB)
           if mb % 5 in (1, 3)}

_CACHE = {}


def _build_program(reps=1):
    nc = bacc.Bacc()

    x8_d = nc.dram_tensor("x8", [CIN, N], FP8, kind="ExternalInput")
    xp_d = nc.dram_tensor("xpool", [CIN, 82], F32R, kind="ExternalInput")
    wk8_d = nc.dram_tensor("wk8", [128, 1024], FP8, kind="ExternalInput")
    wv_d = nc.dram_tensor("wvT", [CIN, CI], F32R, kind="ExternalInput")
    wv2_d = nc.dram_tensor("wv2T", [CI, CI], F32R, kind="ExternalInput")
    wvO_d = nc.dram_tensor("wvO", [CI, CIN], F32R, kind="ExternalInput")
    wws_d = nc.dram_tensor("wws", [CI, CO], F32R, kind="ExternalInput")
    bk2s_d = nc.dram_tensor("bk2s", [128, 2], F32, kind="ExternalInput")
    ones_d = nc.dram_tensor("ones1", [1, 512], F32R, kind="ExternalInput")
    c8_d = nc.dram_tensor("c8ones", [128, 128], FP8, kind="ExternalInput")
    out_d = nc.dram_tensor("out", [CO, QP], F32, kind="ExternalOutput")

    with TileContext(nc) as tc, \
         nc.allow_low_precision(reason="fp8 attention validated numerically"):
      for _rep in range(reps):
        with tc.tile_pool(name=f"const{_rep}", bufs=1) as cpool, \
             tc.tile_pool(name=f"work{_rep}", bufs=1) as wpool:
          ones_sb = cpool.tile([1, 512], F32R)
          wk8_sb = cpool.tile([128, 1024], FP8)
          wv_sb = cpool.tile([128, 4 * CI], F32R)
          wv2_sb = cpool.tile([128, 2 * CI], F32R)
          wvO_sb = cpool.tile([128, 2 * CIN], F32R)
          wws_sb = cpool.tile([128, 2 * CO], F32R)
          bk2s_sb = cpool.tile([128, 2], F32)
          xp_sb = cpool.tile([128, 4 * 82], F32R)
          x8_sb = cpool.tile([128, 4 * NPAD], FP8)
          qk8_sb = cpool.tile([128, 2 * NPAD], FP8)
          dsimv8_sb = cpool.tile([128, MB * 82], FP8)
          pooled_sb = cpool.tile([128, 2 * 82], F32R)
          v2_sb = cpool.tile([128, 2 * 82], F32R)
          v2t_sb = cpool.tile([82, CI], F32R)
          w28_sb = cpool.tile([128, 4 * 82], FP8)
          r2_sb = cpool.tile([128, MB], F32)
          r2i2_sb = cpool.tile([128, MB], F32)
          v2s_sb = cpool.tile([128, 2], F32R)
          wu_sb = cpool.tile([1, CO], F32R)
          mask_sb = cpool.tile([128, 1], F32)

          x8_4 = x8_sb.rearrange("p (c n) -> p c n", c=4)
          qk3 = qk8_sb.rearrange("p (t n) -> p t n", t=2)
          w28_4 = w28_sb.rearrange("p (c k) -> p c k", c=4)
          dsim3 = dsimv8_sb.rearrange("p (m c) -> p m c", m=MB)

          # ---------- emission helpers ----------
          def _slab_dma(s):
              n0 = s * 512
              rl = min(512, N - n0)
              nc.sync.dma_start(
                  out=x8_4[:, :, n0:n0 + rl],
                  in_=x8_d.rearrange("(c p) n -> p c n", c=4)[:, :, n0:n0 + rl])

          def emit_loads_early():
              nc.sync.dma_start(out=wk8_sb[:], in_=wk8_d[:])
              _slab_dma(0)
              _slab_dma(1)
              nc.sync.dma_start(out=bk2s_sb[:], in_=bk2s_d[:])
              nc.sync.dma_start(
                  out=xp_sb.rearrange("p (c k) -> p c k", c=4),
                  in_=xp_d.rearrange("(c p) k -> p c k", c=4))
              nc.sync.dma_start(
                  out=wv_sb.rearrange("p (c k) -> p c k", c=4),
                  in_=wv_d.rearrange("(c p) k -> p c k", c=4))
              nc.sync.dma_start(
                  out=wv2_sb.rearrange("p (c k) -> p c k", c=2),
                  in_=wv2_d.rearrange("(c p) k -> p c k", c=2))
              nc.sync.dma_start(
                  out=wvO_sb.rearrange("p (c k) -> p c k", c=2),
                  in_=wvO_d.rearrange("(c p) k -> p c k", c=2))
              _slab_dma(2)
              _slab_dma(3)
              nc.sync.dma_start(
                  out=wws_sb.rearrange("p (c k) -> p c k", c=2),
                  in_=wws_d.rearrange("(c p) k -> p c k", c=2))
              nc.sync.dma_start(out=ones_sb[:], in_=ones_d[:])
              nc.sync.dma_start(
                  out=dsim3[:, :, 0:1],
                  in_=c8_d[:, 0:MB].rearrange("p (m c) -> p m c", m=MB))
              for s in range(4, 8):
                  _slab_dma(s)
              for cc in range(4):
                  nc.gpsimd.memset(x8_4[:, cc:cc + 1, N:NPAD], 0.0)
              nc.gpsimd.memset(mask_sb[:], 0.0)
              nc.gpsimd.memset(mask_sb[0:1, :], 1.0)

          def emit_conv(psH, s):
              n0 = s * 512
              for blk in range(2):
                  ps = psH.tile([128, 512], F32, tag="ps", name=f"cv{s}_{blk}")
                  for pr in range(2):
                      lhsT = wk8_sb[:, blk * 512 + pr * 256:
                                    blk * 512 + pr * 256 + 256].rearrange(
                          "p (s o) -> p s o", s=2)
                      nc.tensor.matmul(
                          ps[:], lhsT,
                          x8_4[:, 2 * pr:2 * pr + 2, n0:n0 + 512],
                          start=(pr == 0), stop=(pr == 1), perf_mode=DR)
                  qslice = qk8_sb[:, blk * NPAD + n0:blk * NPAD + n0 + 512]
                  if blk == 0:
                      nc.scalar.activation(qslice, ps[:], AF.Identity,
                                           bias=bk2s_sb[:, blk:blk + 1])
                  else:
                      nc.vector.tensor_scalar(
                          out=qslice, in0=ps[:],
                          scalar1=bk2s_sb[:, blk:blk + 1], scalar2=None,
                          op0=OP.add)

          def emit_pooled_path(psH):
              for blk in range(2):
                  ps = psH.tile([128, 512], F32, tag="ps", name=f"pooled{blk}")
                  for cc in range(4):
                      nc.tensor.matmul(
                          ps[:, :82],
                          wv_sb[:, cc * CI + blk * 128:
                                cc * CI + blk * 128 + 128],
                          xp_sb[:, cc * 82:(cc + 1) * 82],
                          start=(cc == 0), stop=(cc == 3))
                  nc.vector.tensor_copy(pooled_sb[:, blk * 82:(blk + 1) * 82],
                                        ps[:, :82])
              for blk in range(2):
                  ps = psH.tile([128, 512], F32, tag="ps", name=f"v2_{blk}")
                  for cc in range(2):
                      nc.tensor.matmul(
                          ps[:, :82],
                          wv2_sb[:, cc * CI + blk * 128:
                                 cc * CI + blk * 128 + 128],
                          pooled_sb[:, cc * 82:(cc + 1) * 82],
                          start=(cc == 0), stop=(cc == 1))
                  nc.vector.tensor_copy(v2_sb[:, blk * 82:(blk + 1) * 82],
                                        ps[:, :82])
              # w2 = Wv^T v2 (contract ci), scaled into fp8
              for oc4 in range(4):
                  ps = psH.tile([128, 512], F32, tag="ps", name=f"w2_{oc4}")
                  for cc in range(2):
                      nc.tensor.matmul(
                          ps[:, :82],
                          wvO_sb[:, cc * CIN + oc4 * 128:
                                 cc * CIN + oc4 * 128 + 128],
                          v2_sb[:, cc * 82:(cc + 1) * 82],
                          start=(cc == 0), stop=(cc == 1))
                  nc.vector.tensor_scalar(
                      out=w28_sb[:, oc4 * 82:(oc4 + 1) * 82], in0=ps[:, :82],
                      scalar1=SW, scalar2=None, op0=OP.mult)

          def emit_tail_consts(psH):
              ps = psH.tile([128, 512], F32, tag="ps", name="v2t")
              for cc in range(2):
                  nc.tensor.matmul(ps[:82, :CI],
                                   pooled_sb[:, cc * 82:(cc + 1) * 82],
                                   wv2_sb[:, cc * CI:(cc + 1) * CI],
                                   start=(cc == 0), stop=(cc == 1))
              nc.vector.tensor_copy(v2t_sb[:], ps[:82, :CI])
              # v2s = rowsum(v2) * SS2/(SS*81); wu = v2s^T wws  -> [1, CO]
              nc.vector.reduce_sum(
                  v2s_sb[:], v2_sb.rearrange("p (c k) -> p c k", c=2),
                  axis=AX.X)
              nc.vector.tensor_scalar(
                  out=v2s_sb[:], in0=v2s_sb[:],
                  scalar1=float(SS2 / (SS * 81.0)), scalar2=None, op0=OP.mult)
              ps = psH.tile([128, 512], F32, tag="ps", name="wu")
              for cc in range(2):
                  nc.tensor.matmul(ps[0:1, :CO], v2s_sb[:, cc:cc + 1],
                                   wws_sb[:, cc * CO:(cc + 1) * CO],
                                   start=(cc == 0), stop=(cc == 1))
              nc.vector.tensor_copy(wu_sb[:], ps[0:1, :CO])

          def emit_lg2(psH, s):
              ps2 = psH.tile([128, 512], F32, tag="ps", name=f"lg{s}")
              for j in range(4):
                  mb = 4 * s + j
                  for pr in range(2):
                      nc.tensor.matmul(
                          ps2[:, j * 82:j * 82 + 82],
                          x8_4[:, 2 * pr:2 * pr + 2, mb * 128:mb * 128 + 128],
                          w28_4[:, 2 * pr:2 * pr + 2, :],
                          start=(pr == 0), stop=(pr == 1), perf_mode=DR)
              ex2 = wpool.tile([128, 4 * 82], F32, tag="ex2", bufs=2,
                               name=f"ex2_{s}")
              ps2v = ps2[:, 0:328].rearrange("p (g c) -> p g c", g=4)
              ex2v = ex2.rearrange("p (g c) -> p g c", g=4)
              nc.scalar.activation(ex2v[:, :, 1:82], ps2v[:, :, 1:82],
                                   AF.Exp, scale=LG2_SCALE)
              nc.vector.reduce_sum(r2_sb[:, 4 * s:4 * s + 4],
                                   ex2v[:, :, 1:82], axis=AX.X)
              nc.vector.reciprocal(r2i2_sb[:, 4 * s:4 * s + 4],
                                   r2_sb[:, 4 * s:4 * s + 4])
              nc.vector.tensor_scalar(
                  out=r2i2_sb[:, 4 * s:4 * s + 4],
                  in0=r2i2_sb[:, 4 * s:4 * s + 4],
                  scalar1=SS2, scalar2=None, op0=OP.mult)
              for j in range(4):
                  mb = 4 * s + j
                  nc.gpsimd.tensor_scalar(
                      out=dsim3[:, mb:mb + 1, 1:82],
                      in0=ex2v[:, j:j + 1, 1:82],
                      scalar1=r2i2_sb[:, mb:mb + 1],
                      scalar2=float(SS2 / 81.0),
                      op0=OP.mult, op1=OP.subtract)

          def emit_mask():
              nc.gpsimd.tensor_scalar(
                  out=dsim3[:, MB - 1:MB, :], in0=dsim3[:, MB - 1:MB, :],
                  scalar1=mask_sb[:, 0:1], scalar2=None, op0=OP.mult)

          def emit_pair(psJ, o82ps, qp, pairi):
              E8 = wpool.tile([128, 2048], FP8, tag="E8", bufs=3,
                              name=f"E8_{qp}_{pairi}")
              E83 = E8.rearrange("p (t n) -> p t n", t=2)
              for j in range(2):
                  mb = 2 * pairi + j
                  psL = psJ.tile([128, 1024], F32, tag="psL",
                                 name=f"psL_{qp}_{mb}")
                  for h in range(2):
                      nc.tensor.matmul(
                          psL[:, h * 512:h * 512 + 512],
                          qk3[:, :, mb * 128:mb * 128 + 128],
                          qk3[:, :, qp * 1024 + h * 512:
                              qp * 1024 + h * 512 + 512],
                          start=True, stop=True, perf_mode=DR)
                  dst = E8[:, j * 1024:j * 1024 + 1024]
                  if (qp, mb) in DVE_EXP:
                      nc.vector.tensor_scalar(
                          out=dst.bitcast(U8), in0=psL[:],
                          scalar1=float(A_SCH), scalar2=float(B_SCH),
                          op0=OP.mult, op1=OP.add)
                  else:
                      nc.scalar.activation(dst, psL[:], AF.Exp,
                                           scale=EXP_SCALE)
              for h in range(2):
                  nc.tensor.matmul(
                      o82ps[h][:],
                      dsim3[:, 2 * pairi:2 * pairi + 2, :],
                      E83[:, :, h * 512:h * 512 + 512],
                      start=(pairi == 0), stop=(pairi == 15), perf_mode=DR)

          def _cp(engine, out, in_):
              if engine == "act":
                  nc.scalar.copy(out, in_)
              else:
                  nc.vector.tensor_copy(out, in_)

          def emit_tail_h(psT, o82ps, qp, h, eng):
              """Tail for one 512-query column block; copies on `eng`."""
              qc = qp * 2 + h
              o82 = wpool.tile([82, 512], F32R, tag="o82sb", bufs=2,
                               name=f"o82_{qc}")
              _cp(eng, o82[:], o82ps[h][:])
              rc = wpool.tile([1, 512], F32R, tag="rc", bufs=2,
                              name=f"rc_{qc}")
              nc.vector.reciprocal(rc[:], o82ps[h][0:1, :])
              bps = psT.tile([128, 512], F32, tag="tail", name=f"bps_{qc}")
              nc.tensor.matmul(bps[:], ones_sb[0:1, 0:128], rc[:],
                               start=True, stop=True)
              bc = wpool.tile([128, 512], F32, tag="bc", bufs=2,
                              name=f"bc_{qc}")
              _cp(eng, bc[:], bps[:])
              ctx = wpool.tile([128, 2 * 512], F32R, tag="ctx", bufs=2,
                               name=f"ctx_{qc}")
              for c2 in range(2):
                  cps = psT.tile([128, 512], F32, tag="tail",
                                 name=f"cps_{qc}_{c2}")
                  nc.tensor.matmul(cps[:],
                                   v2t_sb[:, c2 * 128:(c2 + 1) * 128],
                                   o82[0:82, :], start=True, stop=True)
                  # fold the softmax normalization into ctx
                  nc.vector.tensor_tensor(ctx[:, c2 * 512:(c2 + 1) * 512],
                                          cps[:], bc[:], op=OP.mult)
              for ob in range(4):
                  ops_ = psT.tile([128, 512], F32, tag="tail",
                                  name=f"ops_{qc}_{ob}")
                  for cc in range(2):
                      nc.tensor.matmul(
                          ops_[:],
                          wws_sb[:, cc * CO + ob * 128:
                                 cc * CO + ob * 128 + 128],
                          ctx[:, cc * 512:(cc + 1) * 512],
                          start=(cc == 0), stop=False)
                  # rank-1 mean restore: + wu^T x ones
                  nc.tensor.matmul(
                      ops_[:], wu_sb[:, ob * 128:(ob + 1) * 128],
                      ones_sb[:], start=False, stop=True)
                  outb = wpool.tile([128, 512], F32, tag="outb", bufs=3,
                                    name=f"outb_{qc}_{ob}")
                  _cp(eng if ob % 2 == 0 else
                      ("dve" if eng == "act" else "act"), outb[:], ops_[:])
                  nc.sync.dma_start(
                      out=out_d[ob * 128:(ob + 1) * 128,
                                qc * 512:(qc + 1) * 512],
                      in_=outb[:])

          # ---------- emission schedule ----------
          with tc.tile_pool(name="psJ", bufs=2, space="PSUM") as psJ, \
               tc.tile_pool(name="psO", bufs=2, space="PSUM") as psO:
              o82_qp0 = [psO.tile([82, 512], F32, tag="o82",
                                  name=f"o82ps_0_{h}") for h in range(2)]
              with tc.tile_pool(name="psHead", bufs=2, space="PSUM") as psH:
                  emit_loads_early()
                  emit_conv(psH, 0)
                  emit_conv(psH, 1)
                  emit_pooled_path(psH)
                  emit_lg2(psH, 0)
                  emit_lg2(psH, 1)
                  # lag-one interleave: after slab s, pairs 2(s-1), 2(s-1)+1
                  for s in range(2, 8):
                      emit_pair(psJ, o82_qp0, 0, 2 * (s - 2))
                      emit_pair(psJ, o82_qp0, 0, 2 * (s - 2) + 1)
                      emit_conv(psH, s)
                      emit_lg2(psH, s)
                  emit_mask()
                  emit_pair(psJ, o82_qp0, 0, 12)
                  emit_pair(psJ, o82_qp0, 0, 13)
                  emit_tail_consts(psH)
                  emit_pair(psJ, o82_qp0, 0, 14)
                  emit_pair(psJ, o82_qp0, 0, 15)
              with tc.tile_pool(name="psT", bufs=2, space="PSUM") as psT:
                  o82_qp1 = [psO.tile([82, 512], F32, tag="o82",
                                      name=f"o82ps_1_{h}") for h in range(2)]
                  # qp0's tail rides between early qp1 pairs so the in-order
                  # ACT/DVE queues don't stall qp1's exps behind it
                  for pairi in range(3):
                      emit_pair(psJ, o82_qp1, 1, pairi)
                  emit_tail_h(psT, o82_qp0, 0, 0, "act")
                  for pairi in range(3, 6):
                      emit_pair(psJ, o82_qp1, 1, pairi)
                  emit_tail_h(psT, o82_qp0, 0, 1, "dve")
                  for pairi in range(6, 16):
                      emit_pair(psJ, o82_qp1, 1, pairi)
                  emit_tail_h(psT, o82_qp1, 1, 0, "act")
                  emit_tail_h(psT, o82_qp1, 1, 1, "dve")

    nc.finalize()
    return nc


def _get_program(reps=1):
    if ("nc", reps) not in _CACHE:
        _CACHE[("nc", reps)] = _build_program(reps)
    return _CACHE[("nc", reps)]


def _host_inputs(data_input, Wk, bk, gamma, beta, Wv, bv, Wv2, bv2, Ww, bw):
    f = np.float32
    for name, bias in (("bv", bv), ("bv2", bv2), ("bw", bw)):
        if not np.allclose(np.asarray(bias), 0.0):
            raise NotImplementedError(f"{name} != 0 not supported")
    s = (np.asarray(gamma, f) / np.sqrt(f(1.0) + f(1e-5))).astype(f)
    wk_s = (np.asarray(Wk, f) * s[:, None]) * f(SK)     # [CI, CIN]
    bk2s = ((np.asarray(bk, f) * s + np.asarray(beta, f)) * f(SK)).astype(f)

    # wk8 packed layout: [p, blk*512 + pair*256 + slot*128 + oc]
    # cin = pair*256 + slot*128 + p ; oc_global = blk*128 + oc
    wk8 = np.zeros((128, 1024), NPF8)
    wkT = np.ascontiguousarray(wk_s.T)                  # [CIN, CI]
    for blk in range(2):
        for pr in range(2):
            for sl in range(2):
                cin0 = pr * 256 + sl * 128
                col0 = blk * 512 + pr * 256 + sl * 128
                wk8[:, col0:col0 + 128] = wkT[
                    cin0:cin0 + 128, blk * 128:blk * 128 + 128].astype(NPF8)

    wvT = np.ascontiguousarray(np.asarray(Wv, f).T)
    wv2T = np.ascontiguousarray((np.asarray(Wv2, f) / f(49.0)).T)
    wvO = np.ascontiguousarray(np.asarray(Wv, f))
    wws = np.ascontiguousarray(np.asarray(Ww, f).T * f(SS / SS2))
    xs = np.ascontiguousarray(np.asarray(data_input, f).reshape(B, CIN, N))
    ones1 = np.ones((1, 512), f)
    c8 = np.full((128, 128), SS, NPF8)
    x8s = [np.ascontiguousarray(xs[b].astype(NPF8)) for b in range(B)]
    xpools = []
    for b in range(B):
        xp = np.zeros((CIN, 82), f)
        xp[:, 1:] = xs[b].reshape(CIN, 9, 7, 9, 7).sum(axis=(2, 4)).reshape(
            CIN, KK)
        xpools.append(xp)
    bk2p = np.ascontiguousarray(bk2s.reshape(2, 128).T)

    in_maps = []
    for c in range(8):
        b = c % 4
        q0 = (c // 4) * Q0STEP
        xr = np.ascontiguousarray(np.roll(x8s[b], -q0, axis=1))
        in_maps.append({
            "x8": xr, "xpool": xpools[b], "wk8": wk8, "wvT": wvT,
            "wv2T": wv2T, "wvO": wvO, "wws": wws, "bk2s": bk2p,
            "ones1": ones1, "c8ones": c8,
        })
    return in_maps


def kernel(data_input, Wk, bk, gamma, beta, Wv, bv, Wv2, bv2, Ww, bw):
    f = np.float32
    in_maps = _host_inputs(data_input, Wk, bk, gamma, beta, Wv, bv, Wv2,
                           bv2, Ww, bw)
    nc = _get_program()
    res = run_bass_kernel_spmd(nc, in_maps, list(range(8)))

    full = np.empty((B, CO, N), f)
    for b in range(B):
        full[b, :, :Q0STEP] = res.results[b]["out"][:, :Q0STEP]
        full[b, :, Q0STEP:] = res.results[4 + b]["out"][:, :QCNT]
    return full.reshape(B, CO, H, W)


# revision 13
# speedup vs baseline: 1.8471x; 1.0146x over previous
"""Trainium2 Bass kernel for EmbededNonLocalLayer (fp8 DoubleRow version).

Distribution: 8 cores = 4 batches x 2 query-halves. Each core holds its
batch's full keys; its query half sits at columns [0:2048) of a rolled x.

Math (per core), with host scales SK=16 (qk path), SW=64 (w2 path),
SS=32 (r1 ones), SS2=1024 (centered simv):
  qk8   = fp8(wk8^T x8 + bk2*SK)              [256, 4096]  (conv, DoubleRow)
  v2    = Wv2 @ (Wv @ xpool)/49               [256, 82]    (fp32, col0=0)
  w28   = fp8(Wv^T v2 * SW)                   [512, 82]    (val^T v2 == x^T w2)
  lgt2  = x8^T w28                            per 128-key block (DoubleRow)
  simv  = softmax_k(lgt2 * S/SW); dsimv8 = fp8((simv - 1/81)*SS2), col0 = SS
  E8    = fp8(exp(qk8^T qk8 * S/SK^2))        ACT exp or DVE Schraudolph bits
  o82   = dsimv8^T E8  (DoubleRow, PSUM accum over 32 key blocks)
          row0 = SS*r1 (softmax denom), rows 1:82 = SS2 * (dsimv^T E)
  ctx   = (v2t^T o82) * (1/row0 bcast);  out = (Ww*SS/SS2)^T ctx + wu^T x 1s
          (wu = Ww @ v2.sum/81 restores the centered-simv mean term)
"""

import sys

sys.path.insert(0, "/opt/trn_rl_repo")

import numpy as np
import ml_dtypes

import concourse.bacc as bacc
import concourse.bass as bass
import concourse.mybir as mybir
from concourse.bass_utils import run_bass_kernel_spmd
from concourse.tile import TileContext

F32 = mybir.dt.float32
F32R = mybir.dt.float32r
FP8 = mybir.dt.float8e4
U8 = mybir.dt.uint8
AF = mybir.ActivationFunctionType
AX = mybir.AxisListType
DR = mybir.MatmulPerfMode.DoubleRow
OP = mybir.AluOpType
NPF8 = ml_dtypes.float8_e4m3

B, CIN, H, W = 4, 512, 63, 63
N = H * W            # 3969
NPAD = 4096
CI, CO = 256, 512
KK = 81
SCALE = 0.0625       # 1/sqrt(CI)
QCNT = 1985
QP = 2048
Q0STEP = 1984
MB = NPAD // 128     # 32 key blocks
SLABS = 8            # 512-column x slabs

SK = 16.0            # qk fp8 scale
SW = 64.0            # w2 fp8 scale
SS = 32.0            # ones column scale (r1 row)
SS2 = 1024.0         # centered-simv scale
EXP_SCALE = SCALE / (SK * SK)
LG2_SCALE = SCALE / SW
SIGMA = 0.35
A_SCH = 8.0 / np.log(2.0) * EXP_SCALE
B_SCH = 8.0 * 7.0 + SIGMA

# (qp, mb) units whose exp runs on DVE via Schraudolph bits; rest on ACT.
DVE_EXP = {(qp, mb) for qp in range(2) for mb in range(MB)
           if mb % 5 in (1, 3)}

_CACHE = {}


def _build_program(reps=1):
    nc = bacc.Bacc()

    x8_d = nc.dram_tensor("x8", [CIN, N], FP8, kind="ExternalInput")
    xp_d = nc.dram_tensor("xpool", [CIN, 82], F32R, kind="ExternalInput")
    wk8_d = nc.dram_tensor("wk8", [128, 1024], FP8, kind="ExternalInput")
    wv_d = nc.dram_tensor("wvT", [CIN, CI], F32R, kind="ExternalInput")
    wv2_d = nc.dram_tensor("wv2T", [CI, CI], F32R, kind="ExternalInput")
    wvO_d = nc.dram_tensor("wvO", [CI, CIN], F32R, kind="ExternalInput")
    wws_d = nc.dram_tensor("wws", [CI, CO], F32R, kind="ExternalInput")
    bk2s_d = nc.dram_tensor("bk2s", [128, 2], F32, kind="ExternalInput")
    ones_d = nc.dram_tensor("ones1", [1, 512], F32R, kind="ExternalInput")
    c8_d = nc.dram_tensor("c8ones", [128, 128], FP8, kind="ExternalInput")
    out_d = nc.dram_tensor("out", [CO, QP], F32, kind="ExternalOutput")

    with TileContext(nc) as tc, \
         nc.allow_low_precision(reason="fp8 attention validated numerically"):
      for _rep in range(reps):
        with tc.tile_pool(name=f"const{_rep}", bufs=1) as cpool, \
             tc.tile_pool(name=f"work{_rep}", bufs=1) as wpool:
          ones_sb = cpool.tile([1, 512], F32R)
          wk8_sb = cpool.tile([128, 1024], FP8)
          wv_sb = cpool.tile([128, 4 * CI], F32R)
          wv2_sb = cpool.tile([128, 2 * CI], F32R)
          wvO_sb = cpool.tile([128, 2 * CIN], F32R)
          wws_sb = cpool.tile([128, 2 * CO], F32R)
          bk2s_sb = cpool.tile([128, 2], F32)
          xp_sb = cpool.tile([128, 4 * 82], F32R)
          x8_sb = cpool.tile([128, 4 * NPAD], FP8)
          qk8_sb = cpool.tile([128, 2 * NPAD], FP8)
          dsimv8_sb = cpool.tile([128, MB * 82], FP8)
          pooled_sb = cpool.tile([128, 2 * 82], F32R)
          v2_sb = cpool.tile([128, 2 * 82], F32R)
          v2t_sb = cpool.tile([82, CI], F32R)
          w28_sb = cpool.tile([128, 4 * 82], FP8)
          r2_sb = cpool.tile([128, MB], F32)
          r2i2_sb = cpool.tile([128, MB], F32)
          v2s_sb = cpool.tile([128, 2], F32R)
          wu_sb = cpool.tile([1, CO], F32R)
          mask_sb = cpool.tile([128, 1], F32)

          x8_4 = x8_sb.rearrange("p (c n) -> p c n", c=4)
          qk3 = qk8_sb.rearrange("p (t n) -> p t n", t=2)
          w28_4 = w28_sb.rearrange("p (c k) -> p c k", c=4)
          dsim3 = dsimv8_sb.rearrange("p (m c) -> p m c", m=MB)

          # ---------- emission helpers ----------
          def _slab_dma(s):
              n0 = s * 512
              rl = min(512, N - n0)
              nc.sync.dma_start(
                  out=x8_4[:, :, n0:n0 + rl],
                  in_=x8_d.rearrange("(c p) n -> p c n", c=4)[:, :, n0:n0 + rl])

          def emit_loads_early():
              nc.sync.dma_start(out=wk8_sb[:], in_=wk8_d[:])
              _slab_dma(0)
              _slab_dma(1)
              nc.sync.dma_start(out=bk2s_sb[:], in_=bk2s_d[:])
              nc.sync.dma_start(
                  out=xp_sb.rearrange("p (c k) -> p c k", c=4),
                  in_=xp_d.rearrange("(c p) k -> p c k", c=4))
              nc.sync.dma_start(
                  out=wv_sb.rearrange("p (c k) -> p c k", c=4),
                  in_=wv_d.rearrange("(c p) k -> p c k", c=4))
              nc.sync.dma_start(
                  out=wv2_sb.rearrange("p (c k) -> p c k", c=2),
                  in_=wv2_d.rearrange("(c p) k -> p c k", c=2))
              nc.sync.dma_start(
                  out=wvO_sb.rearrange("p (c k) -> p c k", c=2),
                  in_=wvO_d.rearrange("(c p) k -> p c k", c=2))
              _slab_dma(2)
              _slab_dma(3)
              nc.sync.dma_start(
                  out=wws_sb.rearrange("p (c k) -> p c k", c=2),
                  in_=wws_d.rearrange("(c p) k -> p c k", c=2))
              nc.sync.dma_start(out=ones_sb[:], in_=ones_d[:])
              nc.sync.dma_start(
                  out=dsim3[:, :, 0:1],
                  in_=c8_d[:, 0:MB].rearrange("p (m c) -> p m c", m=MB))
              for s in range(4, 8):
                  _slab_dma(s)
              for cc in range(4):
                  nc.gpsimd.memset(x8_4[:, cc:cc + 1, N:NPAD], 0.0)
              nc.gpsimd.memset(mask_sb[:], 0.0)
              nc.gpsimd.memset(mask_sb[0:1, :], 1.0)

          def emit_conv(psH, s):
              n0 = s * 512
              for blk in range(2):
                  ps = psH.tile([128, 512], F32, tag="ps", name=f"cv{s}_{blk}")
                  for pr in range(2):
                      lhsT = wk8_sb[:, blk * 512 + pr * 256:
                                    blk * 512 + pr * 256 + 256].rearrange(
                          "p (s o) -> p s o", s=2)
                      nc.tensor.matmul(
                          ps[:], lhsT,
                          x8_4[:, 2 * pr:2 * pr + 2, n0:n0 + 512],
                          start=(pr == 0), stop=(pr == 1), perf_mode=DR)
                  qslice = qk8_sb[:, blk * NPAD + n0:blk * NPAD + n0 + 512]
                  if blk == 0:
                      nc.scalar.activation(qslice, ps[:], AF.Identity,
                                           bias=bk2s_sb[:, blk:blk + 1])
                  else:
                      nc.vector.tensor_scalar(
                          out=qslice, in0=ps[:],
                          scalar1=bk2s_sb[:, blk:blk + 1], scalar2=None,
                          op0=OP.add)

          def emit_pooled_path(psH):
              for blk in range(2):
                  ps = psH.tile([128, 512], F32, tag="ps", name=f"pooled{blk}")
                  for cc in range(4):
                      nc.tensor.matmul(
                          ps[:, :82],
                          wv_sb[:, cc * CI + blk * 128:
                                cc * CI + blk * 128 + 128],
                          xp_sb[:, cc * 82:(cc + 1) * 82],
                          start=(cc == 0), stop=(cc == 3))
                  nc.vector.tensor_copy(pooled_sb[:, blk * 82:(blk + 1) * 82],
                                        ps[:, :82])
              for blk in range(2):
                  ps = psH.tile([128, 512], F32, tag="ps", name=f"v2_{blk}")
                  for cc in range(2):
                      nc.tensor.matmul(
                          ps[:, :82],
                          wv2_sb[:, cc * CI + blk * 128:
                                 cc * CI + blk * 128 + 128],
                          pooled_sb[:, cc * 82:(cc + 1) * 82],
                          start=(cc == 0), stop=(cc == 1))
                  nc.vector.tensor_copy(v2_sb[:, blk * 82:(blk + 1) * 82],
                                        ps[:, :82])
              # w2 = Wv^T v2 (contract ci), scaled into fp8
              for oc4 in range(4):
                  ps = psH.tile([128, 512], F32, tag="ps", name=f"w2_{oc4}")
                  for cc in range(2):
                      nc.tensor.matmul(
                          ps[:, :82],
                          wvO_sb[:, cc * CIN + oc4 * 128:
                                 cc * CIN + oc4 * 128 + 128],
                          v2_sb[:, cc * 82:(cc + 1) * 82],
                          start=(cc == 0), stop=(cc == 1))
                  nc.vector.tensor_scalar(
                      out=w28_sb[:, oc4 * 82:(oc4 + 1) * 82], in0=ps[:, :82],
                      scalar1=SW, scalar2=None, op0=OP.mult)

          def emit_tail_consts(psH):
              ps = psH.tile([128, 512], F32, tag="ps", name="v2t")
              for cc in range(2):
                  nc.tensor.matmul(ps[:82, :CI],
                                   pooled_sb[:, cc * 82:(cc + 1) * 82],
                                   wv2_sb[:, cc * CI:(cc + 1) * CI],
                                   start=(cc == 0), stop=(cc == 1))
              nc.vector.tensor_copy(v2t_sb[:], ps[:82, :CI])
              # v2s = rowsum(v2) * SS2/(SS*81); wu = v2s^T wws  -> [1, CO]
              nc.vector.reduce_sum(
                  v2s_sb[:], v2_sb.rearrange("p (c k) -> p c k", c=2),
                  axis=AX.X)
              nc.vector.tensor_scalar(
                  out=v2s_sb[:], in0=v2s_sb[:],
                  scalar1=float(SS2 / (SS * 81.0)), scalar2=None, op0=OP.mult)
              ps = psH.tile([128, 512], F32, tag="ps", name="wu")
              for cc in range(2):
                  nc.tensor.matmul(ps[0:1, :CO], v2s_sb[:, cc:cc + 1],
                                   wws_sb[:, cc * CO:(cc + 1) * CO],
                                   start=(cc == 0), stop=(cc == 1))
              nc.vector.tensor_copy(wu_sb[:], ps[0:1, :CO])

          def emit_lg2(psH, s):
              ps2 = psH.tile([128, 512], F32, tag="ps", name=f"lg{s}")
              for j in range(4):
                  mb = 4 * s + j
                  for pr in range(2):
                      nc.tensor.matmul(
                          ps2[:, j * 82:j * 82 + 82],
                          x8_4[:, 2 * pr:2 * pr + 2, mb * 128:mb * 128 + 128],
                          w28_4[:, 2 * pr:2 * pr + 2, :],
                          start=(pr == 0), stop=(pr == 1), perf_mode=DR)
              ex2 = wpool.tile([128, 4 * 82], F32, tag="ex2", bufs=2,
                               name=f"ex2_{s}")
              ps2v = ps2[:, 0:328].rearrange("p (g c) -> p g c", g=4)
              ex2v = ex2.rearrange("p (g c) -> p g c", g=4)
              nc.scalar.activation(ex2v[:, :, 1:82], ps2v[:, :, 1:82],
                                   AF.Exp, scale=LG2_SCALE)
              nc.vector.reduce_sum(r2_sb[:, 4 * s:4 * s + 4],
                                   ex2v[:, :, 1:82], axis=AX.X)
              nc.vector.reciprocal(r2i2_sb[:, 4 * s:4 * s + 4],
                                   r2_sb[:, 4 * s:4 * s + 4])
              nc.vector.tensor_scalar(
                  out=r2i2_sb[:, 4 * s:4 * s + 4],
                  in0=r2i2_sb[:, 4 * s:4 * s + 4],
                  scalar1=SS2, scalar2=None, op0=OP.mult)
              for j in range(4):
                  mb = 4 * s + j
                  nc.gpsimd.tensor_scalar(
                      out=dsim3[:, mb:mb + 1, 1:82],
                      in0=ex2v[:, j:j + 1, 1:82],
                      scalar1=r2i2_sb[:, mb:mb + 1],
                      scalar2=float(SS2 / 81.0),
                      op0=OP.mult, op1=OP.subtract)

          def emit_mask():
              nc.gpsimd.tensor_scalar(
                  out=dsim3[:, MB - 1:MB, :], in0=dsim3[:, MB - 1:MB, :],
                  scalar1=mask_sb[:, 0:1], scalar2=None, op0=OP.mult)

          def emit_pair(psJ, o82ps, qp, pairi):
              E8 = wpool.tile([128, 2048], FP8, tag="E8", bufs=3,
                              name=f"E8_{qp}_{pairi}")
              E83 = E8.rearrange("p (t n) -> p t n", t=2)
              for j in range(2):
                  mb = 2 * pairi + j
                  psL = psJ.tile([128, 1024], F32, tag="psL",
                                 name=f"psL_{qp}_{mb}")
                  for h in range(2):
                      nc.tensor.matmul(
                          psL[:, h * 512:h * 512 + 512],
                          qk3[:, :, mb * 128:mb * 128 + 128],
                          qk3[:, :, qp * 1024 + h * 512:
                              qp * 1024 + h * 512 + 512],
                          start=True, stop=True, perf_mode=DR)
                  dst = E8[:, j * 1024:j * 1024 + 1024]
                  if (qp, mb) in DVE_EXP:
                      nc.vector.tensor_scalar(
                          out=dst.bitcast(U8), in0=psL[:],
                          scalar1=float(A_SCH), scalar2=float(B_SCH),
                          op0=OP.mult, op1=OP.add)
                  else:
                      nc.scalar.activation(dst, psL[:], AF.Exp,
                                           scale=EXP_SCALE)
              for h in range(2):
                  nc.tensor.matmul(
                      o82ps[h][:],
                      dsim3[:, 2 * pairi:2 * pairi + 2, :],
                      E83[:, :, h * 512:h * 512 + 512],
                      start=(pairi == 0), stop=(pairi == 15), perf_mode=DR)

          def _cp(engine, out, in_):
              if engine == "act":
                  nc.scalar.copy(out, in_)
              else:
                  nc.vector.tensor_copy(out, in_)

          def emit_tail_h(psT, o82ps, qp, h, eng):
              """Tail for one 512-query column block; copies on `eng`."""
              qc = qp * 2 + h
              o82 = wpool.tile([82, 512], F32R, tag="o82sb", bufs=2,
                               name=f"o82_{qc}")
              _cp(eng, o82[:], o82ps[h][:])
              rc = wpool.tile([1, 512], F32R, tag="rc", bufs=2,
                              name=f"rc_{qc}")
              nc.vector.reciprocal(rc[:], o82ps[h][0:1, :])
              bps = psT.tile([128, 512], F32, tag="tail", name=f"bps_{qc}")
              nc.tensor.matmul(bps[:], ones_sb[0:1, 0:128], rc[:],
                               start=True, stop=True)
              bc = wpool.tile([128, 512], F32, tag="bc", bufs=2,
                              name=f"bc_{qc}")
              _cp(eng, bc[:], bps[:])
              ctx = wpool.tile([128, 2 * 512], F32R, tag="ctx", bufs=2,
                               name=f"ctx_{qc}")
              for c2 in range(2):
                  cps = psT.tile([128, 512], F32, tag="tail",
                                 name=f"cps_{qc}_{c2}")
                  nc.tensor.matmul(cps[:],
                                   v2t_sb[:, c2 * 128:(c2 + 1) * 128],
                                   o82[0:82, :], start=True, stop=True)
                  # fold the softmax normalization into ctx
                  nc.vector.tensor_tensor(ctx[:, c2 * 512:(c2 + 1) * 512],
                                          cps[:], bc[:], op=OP.mult)
              for ob in range(4):
                  ops_ = psT.tile([128, 512], F32, tag="tail",
                                  name=f"ops_{qc}_{ob}")
                  for cc in range(2):
                      nc.tensor.matmul(
                          ops_[:],
                          wws_sb[:, cc * CO + ob * 128:
                                 cc * CO + ob * 128 + 128],
                          ctx[:, cc * 512:(cc + 1) * 512],
                          start=(cc == 0), stop=False)
                  # rank-1 mean restore: + wu^T x ones
                  nc.tensor.matmul(
                      ops_[:], wu_sb[:, ob * 128:(ob + 1) * 128],
                      ones_sb[:], start=False, stop=True)
                  outb = wpool.tile([128, 512], F32, tag="outb", bufs=3,
                                    name=f"outb_{qc}_{ob}")
                  _cp(eng if ob % 2 == 0 else
                      ("dve" if eng == "act" else "act"), outb[:], ops_[:])
                  nc.sync.dma_start(
                      out=out_d[ob * 128:(ob + 1) * 128,
                                qc * 512:(qc + 1) * 512],
                      in_=outb[:])

          def emit_tail_final(psT, psJ, o82ps, qp):
              """Last tail: both column-blocks interleaved, 4 psum slots
              (psT's 2 plus the now-idle psJ's 2)."""
              slot_i = [0]

              def _slot(name):
                  slot_i[0] += 1
                  if slot_i[0] % 2 == 0:
                      return psT.tile([128, 512], F32, tag="tail", name=name)
                  t = psJ.tile([128, 1024], F32, tag="psL", name=name)
                  return t[:, 0:512]

              qcs = [qp * 2, qp * 2 + 1]
              rcs, o82s, bcs, ctxs = [], [], [], []
              for h in range(2):
                  rc = wpool.tile([1, 512], F32R, tag="rc", bufs=2,
                                  name=f"rc_{qcs[h]}")
                  nc.vector.reciprocal(rc[:], o82ps[h][0:1, :])
                  rcs.append(rc)
              for h in range(2):
                  o82 = wpool.tile([82, 512], F32R, tag="o82sb", bufs=2,
                                   name=f"o82_{qcs[h]}")
                  nc.scalar.copy(o82[:], o82ps[h][:])
                  o82s.append(o82)
              bpss = []
              for h in range(2):
                  bps = _slot(f"bps_{qcs[h]}")
                  nc.tensor.matmul(bps, ones_sb[0:1, 0:128], rcs[h][:],
                                   start=True, stop=True)
                  bpss.append(bps)
              for h in range(2):
                  bc = wpool.tile([128, 512], F32, tag="bc", bufs=2,
                                  name=f"bc_{qcs[h]}")
                  _cp("act" if h == 0 else "dve", bc[:], bpss[h])
                  bcs.append(bc)
              for h in range(2):
                  ctx = wpool.tile([128, 2 * 512], F32R, tag="ctx", bufs=2,
                                   name=f"ctx_{qcs[h]}")
                  ctxs.append(ctx)
              for c2 in range(2):
                  for h in range(2):
                      cps = _slot(f"cps_{qcs[h]}_{c2}")
                      nc.tensor.matmul(cps,
                                       v2t_sb[:, c2 * 128:(c2 + 1) * 128],
                                       o82s[h][0:82, :], start=True, stop=True)
                      nc.vector.tensor_tensor(
                          ctxs[h][:, c2 * 512:(c2 + 1) * 512], cps, bcs[h][:],
                          op=OP.mult)
              for ob in range(4):
                  for h in range(2):
                      qc = qcs[h]
                      ops_ = _slot(f"ops_{qc}_{ob}")
                      for cc in range(2):
                          nc.tensor.matmul(
                              ops_,
                              wws_sb[:, cc * CO + ob * 128:
                                     cc * CO + ob * 128 + 128],
                              ctxs[h][:, cc * 512:(cc + 1) * 512],
                              start=(cc == 0), stop=False)
                      nc.tensor.matmul(
                          ops_, wu_sb[:, ob * 128:(ob + 1) * 128],
                          ones_sb[:], start=False, stop=True)
                      outb = wpool.tile([128, 512], F32, tag="outb", bufs=3,
                                        name=f"outb_{qc}_{ob}")
                      _cp("act" if (ob + h) % 2 == 0 else "dve",
                          outb[:], ops_)
                      nc.sync.dma_start(
                          out=out_d[ob * 128:(ob + 1) * 128,
                                    qc * 512:(qc + 1) * 512],
                          in_=outb[:])

          # ---------- emission schedule ----------
          with tc.tile_pool(name="psJ", bufs=2, space="PSUM") as psJ, \
               tc.tile_pool(name="psO", bufs=2, space="PSUM") as psO:
              o82_qp0 = [psO.tile([82, 512], F32, tag="o82",
                                  name=f"o82ps_0_{h}") for h in range(2)]
              with tc.tile_pool(name="psHead", bufs=2, space="PSUM") as psH:
                  emit_loads_early()
                  emit_conv(psH, 0)
                  emit_conv(psH, 1)
                  emit_pooled_path(psH)
                  emit_lg2(psH, 0)
                  emit_lg2(psH, 1)
                  # lag-one interleave: after slab s, pairs 2(s-1), 2(s-1)+1
                  for s in range(2, 8):
                      emit_pair(psJ, o82_qp0, 0, 2 * (s - 2))
                      emit_pair(psJ, o82_qp0, 0, 2 * (s - 2) + 1)
                      emit_conv(psH, s)
                      emit_lg2(psH, s)
                  emit_mask()
                  emit_pair(psJ, o82_qp0, 0, 12)
                  emit_pair(psJ, o82_qp0, 0, 13)
                  emit_tail_consts(psH)
                  emit_pair(psJ, o82_qp0, 0, 14)
                  emit_pair(psJ, o82_qp0, 0, 15)
              with tc.tile_pool(name="psT", bufs=2, space="PSUM") as psT:
                  o82_qp1 = [psO.tile([82, 512], F32, tag="o82",
                                      name=f"o82ps_1_{h}") for h in range(2)]
                  # qp0's tail rides between early qp1 pairs so the in-order
                  # ACT/DVE queues don't stall qp1's exps behind it
                  for pairi in range(3):
                      emit_pair(psJ, o82_qp1, 1, pairi)
                  emit_tail_h(psT, o82_qp0, 0, 0, "act")
                  for pairi in range(3, 6):
                      emit_pair(psJ, o82_qp1, 1, pairi)
                  emit_tail_h(psT, o82_qp0, 0, 1, "dve")
                  for pairi in range(6, 16):
                      emit_pair(psJ, o82_qp1, 1, pairi)
                  emit_tail_final(psT, psJ, o82_qp1, 1)

    nc.finalize()
    return nc


def _get_program(reps=1):
    if ("nc", reps) not in _CACHE:
        _CACHE[("nc", reps)] = _build_program(reps)
    return _CACHE[("nc", reps)]


def _host_inputs(data_input, Wk, bk, gamma, beta, Wv, bv, Wv2, bv2, Ww, bw):
    f = np.float32
    for name, bias in (("bv", bv), ("bv2", bv2), ("bw", bw)):
        if not np.allclose(np.asarray(bias), 0.0):
            raise NotImplementedError(f"{name} != 0 not supported")
    s = (np.asarray(gamma, f) / np.sqrt(f(1.0) + f(1e-5))).astype(f)
    wk_s = (np.asarray(Wk, f) * s[:, None]) * f(SK)     # [CI, CIN]
    bk2s = ((np.asarray(bk, f) * s + np.asarray(beta, f)) * f(SK)).astype(f)

    # wk8 packed layout: [p, blk*512 + pair*256 + slot*128 + oc]
    # cin = pair*256 + slot*128 + p ; oc_global = blk*128 + oc
    wk8 = np.zeros((128, 1024), NPF8)
    wkT = np.ascontiguousarray(wk_s.T)                  # [CIN, CI]
    for blk in range(2):
        for pr in range(2):
            for sl in range(2):
                cin0 = pr * 256 + sl * 128
                col0 = blk * 512 + pr * 256 + sl * 128
                wk8[:, col0:col0 + 128] = wkT[
                    cin0:cin0 + 128, blk * 128:blk * 128 + 128].astype(NPF8)

    wvT = np.ascontiguousarray(np.asarray(Wv, f).T)
    wv2T = np.ascontiguousarray((np.asarray(Wv2, f) / f(49.0)).T)
    wvO = np.ascontiguousarray(np.asarray(Wv, f))
    wws = np.ascontiguousarray(np.asarray(Ww, f).T * f(SS / SS2))
    xs = np.ascontiguousarray(np.asarray(data_input, f).reshape(B, CIN, N))
    ones1 = np.ones((1, 512), f)
    c8 = np.full((128, 128), SS, NPF8)
    x8s = [np.ascontiguousarray(xs[b].astype(NPF8)) for b in range(B)]
    xpools = []
    for b in range(B):
        xp = np.zeros((CIN, 82), f)
        xp[:, 1:] = xs[b].reshape(CIN, 9, 7, 9, 7).sum(axis=(2, 4)).reshape(
            CIN, KK)
        xpools.append(xp)
    bk2p = np.ascontiguousarray(bk2s.reshape(2, 128).T)

    in_maps = []
    for c in range(8):
        b = c % 4
        q0 = (c // 4) * Q0STEP
        xr = np.ascontiguousarray(np.roll(x8s[b], -q0, axis=1))
        in_maps.append({
            "x8": xr, "xpool": xpools[b], "wk8": wk8, "wvT": wvT,
            "wv2T": wv2T, "wvO": wvO, "wws": wws, "bk2s": bk2p,
            "ones1": ones1, "c8ones": c8,
        })
    return in_maps


def kernel(data_input, Wk, bk, gamma, beta, Wv, bv, Wv2, bv2, Ww, bw):
    f = np.float32
    in_maps = _host_inputs(data_input, Wk, bk, gamma, beta, Wv, bv, Wv2,
                           bv2, Ww, bw)
    nc = _get_program()
    res = run_bass_kernel_spmd(nc, in_maps, list(range(8)))

    full = np.empty((B, CO, N), f)
    for b in range(B):
        full[b, :, :Q0STEP] = res.results[b]["out"][:, :Q0STEP]
        full[b, :, Q0STEP:] = res.results[4 + b]["out"][:, :QCNT]
    return full.reshape(B, CO, H, W)


# revision 15
# speedup vs baseline: 1.9394x; 1.0499x over previous
"""Trainium2 Bass kernel for EmbededNonLocalLayer (fp8 DoubleRow version).

Distribution: 8 cores = 4 batches x 2 query-halves. Each core holds its
batch's full keys; its query half sits at columns [0:2048) of a rolled x.

Math (per core), with host scales SK=16 (qk path), SW=64 (w2 path),
SS=32 (r1 ones), SS2=1024 (centered simv):
  qk8   = fp8(wk8^T x8 + bk2*SK)              [256, 4096]  (conv, DoubleRow)
  v2    = Wv2 @ (Wv @ xpool)/49               [256, 82]    (fp32, col0=0)
  w28   = fp8(Wv^T v2 * SW)                   [512, 82]    (val^T v2 == x^T w2)
  lgt2  = x8^T w28                            per 128-key block (DoubleRow)
  simv  = softmax_k(lgt2 * S/SW); dsimv8 = fp8((simv - 1/81)*SS2), col0 = SS
  E8    = fp8(exp(qk8^T qk8 * S/SK^2))        ACT exp or DVE Schraudolph bits
  o82   = dsimv8^T E8  (DoubleRow, PSUM accum over 32 key blocks)
          row0 = SS*r1 (softmax denom), rows 1:82 = SS2 * (dsimv^T E)
  ctx   = (v2t^T o82) * (1/row0 bcast);  out = (Ww*SS/SS2)^T ctx + wu^T x 1s
          (wu = Ww @ v2.sum/81 restores the centered-simv mean term)
"""

import sys

sys.path.insert(0, "/opt/trn_rl_repo")

import numpy as np
import ml_dtypes

import concourse.bacc as bacc
import concourse.bass as bass
import concourse.mybir as mybir
from concourse.bass_utils import run_bass_kernel_spmd
from concourse.tile import TileContext

F32 = mybir.dt.float32
F32R = mybir.dt.float32r
FP8 = mybir.dt.float8e4
U8 = mybir.dt.uint8
AF = mybir.ActivationFunctionType
AX = mybir.AxisListType
DR = mybir.MatmulPerfMode.DoubleRow
OP = mybir.AluOpType
NPF8 = ml_dtypes.float8_e4m3

B, CIN, H, W = 4, 512, 63, 63
N = H * W            # 3969
NPAD = 4096
CI, CO = 256, 512
KK = 81
SCALE = 0.0625       # 1/sqrt(CI)
QCNT = 1985
QP = 2048
Q0STEP = 1984
MB = NPAD // 128     # 32 key blocks
SLABS = 8            # 512-column x slabs

SK = 16.0            # qk fp8 scale
SW = 64.0            # w2 fp8 scale
SS = 32.0            # ones column scale (r1 row)
SS2 = 1024.0         # centered-simv scale
EXP_SCALE = SCALE / (SK * SK)
LG2_SCALE = SCALE / SW
SIGMA = 0.35
A_SCH = 8.0 / np.log(2.0) * EXP_SCALE
B_SCH = 8.0 * 7.0 + SIGMA

# (qp, mb) units whose exp runs on DVE via Schraudolph bits; rest on ACT.
DVE_EXP = {(qp, mb) for qp in range(2) for mb in range(MB)
           if mb % 5 in (1, 3)}

_CACHE = {}


def _build_program(reps=1):
    nc = bacc.Bacc()

    x8_d = nc.dram_tensor("x8", [CIN, N], FP8, kind="ExternalInput")
    xp_d = nc.dram_tensor("xpool", [CIN, 82], F32R, kind="ExternalInput")
    wk8_d = nc.dram_tensor("wk8", [128, 1024], FP8, kind="ExternalInput")
    wv_d = nc.dram_tensor("wvT", [CIN, CI], F32R, kind="ExternalInput")
    wv2_d = nc.dram_tensor("wv2T", [CI, CI], F32R, kind="ExternalInput")
    wvO_d = nc.dram_tensor("wvO", [CI, CIN], F32R, kind="ExternalInput")
    wws_d = nc.dram_tensor("wws", [CI, CO], F32R, kind="ExternalInput")
    bk2s_d = nc.dram_tensor("bk2s", [128, 2], F32, kind="ExternalInput")
    ones_d = nc.dram_tensor("ones1", [1, 512], F32R, kind="ExternalInput")
    c8_d = nc.dram_tensor("c8ones", [128, 128], FP8, kind="ExternalInput")
    out_d = nc.dram_tensor("out", [CO, QP], F32, kind="ExternalOutput")

    with TileContext(nc) as tc, \
         nc.allow_low_precision(reason="fp8 attention validated numerically"):
      for _rep in range(reps):
        with tc.tile_pool(name=f"const{_rep}", bufs=1) as cpool, \
             tc.tile_pool(name=f"work{_rep}", bufs=1) as wpool:
          ones_sb = cpool.tile([1, 512], F32R)
          wk8_sb = cpool.tile([128, 1024], FP8)
          wv_sb = cpool.tile([128, 4 * CI], F32R)
          wv2_sb = cpool.tile([128, 2 * CI], F32R)
          wvO_sb = cpool.tile([128, 2 * CIN], F32R)
          wws_sb = cpool.tile([128, 2 * CO], F32R)
          bk2s_sb = cpool.tile([128, 2], F32)
          xp_sb = cpool.tile([128, 4 * 82], F32R)
          x8_sb = cpool.tile([128, 4 * NPAD], FP8)
          qk8_sb = cpool.tile([128, 2 * NPAD], FP8)
          dsimv8_sb = cpool.tile([128, MB * 82], FP8)
          pooled_sb = cpool.tile([128, 2 * 82], F32R)
          v2_sb = cpool.tile([128, 2 * 82], F32R)
          v2t_sb = cpool.tile([82, CI], F32R)
          w28_sb = cpool.tile([128, 4 * 82], FP8)
          r2_sb = cpool.tile([128, MB], F32)
          r2i2_sb = cpool.tile([128, MB], F32)
          v2s_sb = cpool.tile([128, 2], F32R)
          wu_sb = cpool.tile([1, CO], F32R)
          mask_sb = cpool.tile([128, 1], F32)

          x8_4 = x8_sb.rearrange("p (c n) -> p c n", c=4)
          qk3 = qk8_sb.rearrange("p (t n) -> p t n", t=2)
          w28_4 = w28_sb.rearrange("p (c k) -> p c k", c=4)
          dsim3 = dsimv8_sb.rearrange("p (m c) -> p m c", m=MB)

          # ---------- emission helpers ----------
          def _slab_dma(s):
              n0 = s * 512
              rl = min(512, N - n0)
              nc.sync.dma_start(
                  out=x8_4[:, :, n0:n0 + rl],
                  in_=x8_d.rearrange("(c p) n -> p c n", c=4)[:, :, n0:n0 + rl])

          def emit_loads_early():
              nc.sync.dma_start(out=wk8_sb[:], in_=wk8_d[:])
              _slab_dma(0)
              _slab_dma(1)
              nc.sync.dma_start(out=bk2s_sb[:], in_=bk2s_d[:])
              nc.sync.dma_start(
                  out=xp_sb.rearrange("p (c k) -> p c k", c=4),
                  in_=xp_d.rearrange("(c p) k -> p c k", c=4))
              nc.sync.dma_start(
                  out=wv_sb.rearrange("p (c k) -> p c k", c=4),
                  in_=wv_d.rearrange("(c p) k -> p c k", c=4))
              nc.sync.dma_start(
                  out=wv2_sb.rearrange("p (c k) -> p c k", c=2),
                  in_=wv2_d.rearrange("(c p) k -> p c k", c=2))
              nc.sync.dma_start(
                  out=wvO_sb.rearrange("p (c k) -> p c k", c=2),
                  in_=wvO_d.rearrange("(c p) k -> p c k", c=2))
              _slab_dma(2)
              _slab_dma(3)
              nc.sync.dma_start(
                  out=wws_sb.rearrange("p (c k) -> p c k", c=2),
                  in_=wws_d.rearrange("(c p) k -> p c k", c=2))
              nc.sync.dma_start(out=ones_sb[:], in_=ones_d[:])
              nc.sync.dma_start(
                  out=dsim3[:, :, 0:1],
                  in_=c8_d[:, 0:MB].rearrange("p (m c) -> p m c", m=MB))
              for s in range(4, 8):
                  _slab_dma(s)
              for cc in range(4):
                  nc.gpsimd.memset(x8_4[:, cc:cc + 1, N:NPAD], 0.0)
              nc.gpsimd.memset(mask_sb[:], 0.0)
              nc.gpsimd.memset(mask_sb[0:1, :], 1.0)

          def emit_conv(psH, s):
              n0 = s * 512
              for blk in range(2):
                  ps = psH.tile([128, 512], F32, tag="ps", name=f"cv{s}_{blk}")
                  for pr in range(2):
                      lhsT = wk8_sb[:, blk * 512 + pr * 256:
                                    blk * 512 + pr * 256 + 256].rearrange(
                          "p (s o) -> p s o", s=2)
                      nc.tensor.matmul(
                          ps[:], lhsT,
                          x8_4[:, 2 * pr:2 * pr + 2, n0:n0 + 512],
                          start=(pr == 0), stop=(pr == 1), perf_mode=DR)
                  qslice = qk8_sb[:, blk * NPAD + n0:blk * NPAD + n0 + 512]
                  if blk == 0:
                      nc.scalar.activation(qslice, ps[:], AF.Identity,
                                           bias=bk2s_sb[:, blk:blk + 1])
                  else:
                      nc.vector.tensor_scalar(
                          out=qslice, in0=ps[:],
                          scalar1=bk2s_sb[:, blk:blk + 1], scalar2=None,
                          op0=OP.add)

          def emit_pooled_path(psH):
              for blk in range(2):
                  ps = psH.tile([128, 512], F32, tag="ps", name=f"pooled{blk}")
                  for cc in range(4):
                      nc.tensor.matmul(
                          ps[:, :82],
                          wv_sb[:, cc * CI + blk * 128:
                                cc * CI + blk * 128 + 128],
                          xp_sb[:, cc * 82:(cc + 1) * 82],
                          start=(cc == 0), stop=(cc == 3))
                  nc.vector.tensor_copy(pooled_sb[:, blk * 82:(blk + 1) * 82],
                                        ps[:, :82])
              for blk in range(2):
                  ps = psH.tile([128, 512], F32, tag="ps", name=f"v2_{blk}")
                  for cc in range(2):
                      nc.tensor.matmul(
                          ps[:, :82],
                          wv2_sb[:, cc * CI + blk * 128:
                                 cc * CI + blk * 128 + 128],
                          pooled_sb[:, cc * 82:(cc + 1) * 82],
                          start=(cc == 0), stop=(cc == 1))
                  nc.vector.tensor_copy(v2_sb[:, blk * 82:(blk + 1) * 82],
                                        ps[:, :82])
              # w2 = Wv^T v2 (contract ci), scaled into fp8
              for oc4 in range(4):
                  ps = psH.tile([128, 512], F32, tag="ps", name=f"w2_{oc4}")
                  for cc in range(2):
                      nc.tensor.matmul(
                          ps[:, :82],
                          wvO_sb[:, cc * CIN + oc4 * 128:
                                 cc * CIN + oc4 * 128 + 128],
                          v2_sb[:, cc * 82:(cc + 1) * 82],
                          start=(cc == 0), stop=(cc == 1))
                  nc.vector.tensor_scalar(
                      out=w28_sb[:, oc4 * 82:(oc4 + 1) * 82], in0=ps[:, :82],
                      scalar1=SW, scalar2=None, op0=OP.mult)

          def emit_tail_consts(psH):
              ps = psH.tile([128, 512], F32, tag="ps", name="v2t")
              for cc in range(2):
                  nc.tensor.matmul(ps[:82, :CI],
                                   pooled_sb[:, cc * 82:(cc + 1) * 82],
                                   wv2_sb[:, cc * CI:(cc + 1) * CI],
                                   start=(cc == 0), stop=(cc == 1))
              nc.vector.tensor_copy(v2t_sb[:], ps[:82, :CI])
              # v2s = rowsum(v2) * SS2/(SS*81); wu = v2s^T wws  -> [1, CO]
              nc.vector.reduce_sum(
                  v2s_sb[:], v2_sb.rearrange("p (c k) -> p c k", c=2),
                  axis=AX.X)
              nc.vector.tensor_scalar(
                  out=v2s_sb[:], in0=v2s_sb[:],
                  scalar1=float(SS2 / (SS * 81.0)), scalar2=None, op0=OP.mult)
              ps = psH.tile([128, 512], F32, tag="ps", name="wu")
              for cc in range(2):
                  nc.tensor.matmul(ps[0:1, :CO], v2s_sb[:, cc:cc + 1],
                                   wws_sb[:, cc * CO:(cc + 1) * CO],
                                   start=(cc == 0), stop=(cc == 1))
              nc.vector.tensor_copy(wu_sb[:], ps[0:1, :CO])

          def emit_lg2(psH, s):
              ps2 = psH.tile([128, 512], F32, tag="ps", name=f"lg{s}")
              for j in range(4):
                  mb = 4 * s + j
                  for pr in range(2):
                      nc.tensor.matmul(
                          ps2[:, j * 82:j * 82 + 82],
                          x8_4[:, 2 * pr:2 * pr + 2, mb * 128:mb * 128 + 128],
                          w28_4[:, 2 * pr:2 * pr + 2, :],
                          start=(pr == 0), stop=(pr == 1), perf_mode=DR)
              ex2 = wpool.tile([128, 4 * 82], F32, tag="ex2", bufs=2,
                               name=f"ex2_{s}")
              ps2v = ps2[:, 0:328].rearrange("p (g c) -> p g c", g=4)
              ex2v = ex2.rearrange("p (g c) -> p g c", g=4)
              nc.scalar.activation(ex2v[:, :, 1:82], ps2v[:, :, 1:82],
                                   AF.Exp, scale=LG2_SCALE)
              nc.vector.reduce_sum(r2_sb[:, 4 * s:4 * s + 4],
                                   ex2v[:, :, 1:82], axis=AX.X)
              nc.vector.reciprocal(r2i2_sb[:, 4 * s:4 * s + 4],
                                   r2_sb[:, 4 * s:4 * s + 4])
              nc.vector.tensor_scalar(
                  out=r2i2_sb[:, 4 * s:4 * s + 4],
                  in0=r2i2_sb[:, 4 * s:4 * s + 4],
                  scalar1=SS2, scalar2=None, op0=OP.mult)
              for j in range(4):
                  mb = 4 * s + j
                  nc.gpsimd.tensor_scalar(
                      out=dsim3[:, mb:mb + 1, 1:82],
                      in0=ex2v[:, j:j + 1, 1:82],
                      scalar1=r2i2_sb[:, mb:mb + 1],
                      scalar2=float(SS2 / 81.0),
                      op0=OP.mult, op1=OP.subtract)

          def emit_mask():
              nc.gpsimd.tensor_scalar(
                  out=dsim3[:, MB - 1:MB, :], in0=dsim3[:, MB - 1:MB, :],
                  scalar1=mask_sb[:, 0:1], scalar2=None, op0=OP.mult)

          pend_o82 = []

          def emit_o82(o82ps, pairi, E83):
              for h in range(2):
                  nc.tensor.matmul(
                      o82ps[h][:],
                      dsim3[:, 2 * pairi:2 * pairi + 2, :],
                      E83[:, :, h * 512:h * 512 + 512],
                      start=(pairi == 0), stop=(pairi == 15), perf_mode=DR)

          def emit_pair(psJ, o82ps, qp, pairi, last=False):
              """Emit psL+exp for pair `pairi`; the o82 accumulation is
              emitted one pair late so PE never stalls waiting on exp."""
              E8 = wpool.tile([128, 2048], FP8, tag="E8", bufs=4,
                              name=f"E8_{qp}_{pairi}")
              E83 = E8.rearrange("p (t n) -> p t n", t=2)
              for j in range(2):
                  mb = 2 * pairi + j
                  psL = psJ.tile([128, 1024], F32, tag="psL",
                                 name=f"psL_{qp}_{mb}")
                  for h in range(2):
                      nc.tensor.matmul(
                          psL[:, h * 512:h * 512 + 512],
                          qk3[:, :, mb * 128:mb * 128 + 128],
                          qk3[:, :, qp * 1024 + h * 512:
                              qp * 1024 + h * 512 + 512],
                          start=True, stop=True, perf_mode=DR)
                  dst = E8[:, j * 1024:j * 1024 + 1024]
                  if (qp, mb) in DVE_EXP:
                      nc.vector.tensor_scalar(
                          out=dst.bitcast(U8), in0=psL[:],
                          scalar1=float(A_SCH), scalar2=float(B_SCH),
                          op0=OP.mult, op1=OP.add)
                  else:
                      nc.scalar.activation(dst, psL[:], AF.Exp,
                                           scale=EXP_SCALE)
              pend_o82.append((pairi, E83))
              while len(pend_o82) > (0 if last else 1):
                  pi, e83 = pend_o82.pop(0)
                  emit_o82(o82ps, pi, e83)

          def _cp(engine, out, in_):
              if engine == "act":
                  nc.scalar.copy(out, in_)
              else:
                  nc.vector.tensor_copy(out, in_)

          def emit_tail_h(psT, o82ps, qp, h, eng):
              """Tail for one 512-query column block; copies on `eng`."""
              qc = qp * 2 + h
              o82 = wpool.tile([82, 512], F32R, tag="o82sb", bufs=2,
                               name=f"o82_{qc}")
              _cp(eng, o82[:], o82ps[h][:])
              rc = wpool.tile([1, 512], F32R, tag="rc", bufs=2,
                              name=f"rc_{qc}")
              nc.vector.reciprocal(rc[:], o82ps[h][0:1, :])
              bps = psT.tile([128, 512], F32, tag="tail", name=f"bps_{qc}")
              nc.tensor.matmul(bps[:], ones_sb[0:1, 0:128], rc[:],
                               start=True, stop=True)
              bc = wpool.tile([128, 512], F32, tag="bc", bufs=2,
                              name=f"bc_{qc}")
              _cp(eng, bc[:], bps[:])
              ctx = wpool.tile([128, 2 * 512], F32R, tag="ctx", bufs=2,
                               name=f"ctx_{qc}")
              for c2 in range(2):
                  cps = psT.tile([128, 512], F32, tag="tail",
                                 name=f"cps_{qc}_{c2}")
                  nc.tensor.matmul(cps[:],
                                   v2t_sb[:, c2 * 128:(c2 + 1) * 128],
                                   o82[0:82, :], start=True, stop=True)
                  # fold the softmax normalization into ctx
                  nc.vector.tensor_tensor(ctx[:, c2 * 512:(c2 + 1) * 512],
                                          cps[:], bc[:], op=OP.mult)
              for ob in range(4):
                  ops_ = psT.tile([128, 512], F32, tag="tail",
                                  name=f"ops_{qc}_{ob}")
                  for cc in range(2):
                      nc.tensor.matmul(
                          ops_[:],
                          wws_sb[:, cc * CO + ob * 128:
                                 cc * CO + ob * 128 + 128],
                          ctx[:, cc * 512:(cc + 1) * 512],
                          start=(cc == 0), stop=False)
                  # rank-1 mean restore: + wu^T x ones
                  nc.tensor.matmul(
                      ops_[:], wu_sb[:, ob * 128:(ob + 1) * 128],
                      ones_sb[:], start=False, stop=True)
                  outb = wpool.tile([128, 512], F32, tag="outb", bufs=3,
                                    name=f"outb_{qc}_{ob}")
                  _cp(eng if ob % 2 == 0 else
                      ("dve" if eng == "act" else "act"), outb[:], ops_[:])
                  nc.sync.dma_start(
                      out=out_d[ob * 128:(ob + 1) * 128,
                                qc * 512:(qc + 1) * 512],
                      in_=outb[:])

          def emit_tail_final(psT, psJ, o82ps, qp):
              """Last tail: both column-blocks interleaved, 4 psum slots
              (psT's 2 plus the now-idle psJ's 2)."""
              slot_i = [0]

              def _slot(name):
                  slot_i[0] += 1
                  if slot_i[0] % 2 == 0:
                      return psT.tile([128, 512], F32, tag="tail", name=name)
                  t = psJ.tile([128, 1024], F32, tag="psL", name=name)
                  return t[:, 0:512]

              qcs = [qp * 2, qp * 2 + 1]
              rcs, o82s, bcs, ctxs = [], [], [], []
              for h in range(2):
                  rc = wpool.tile([1, 512], F32R, tag="rc", bufs=2,
                                  name=f"rc_{qcs[h]}")
                  nc.vector.reciprocal(rc[:], o82ps[h][0:1, :])
                  rcs.append(rc)
              for h in range(2):
                  o82 = wpool.tile([82, 512], F32R, tag="o82sb", bufs=2,
                                   name=f"o82_{qcs[h]}")
                  nc.scalar.copy(o82[:], o82ps[h][:])
                  o82s.append(o82)
              bpss = []
              for h in range(2):
                  bps = _slot(f"bps_{qcs[h]}")
                  nc.tensor.matmul(bps, ones_sb[0:1, 0:128], rcs[h][:],
                                   start=True, stop=True)
                  bpss.append(bps)
              for h in range(2):
                  bc = wpool.tile([128, 512], F32, tag="bc", bufs=2,
                                  name=f"bc_{qcs[h]}")
                  _cp("act" if h == 0 else "dve", bc[:], bpss[h])
                  bcs.append(bc)
              for h in range(2):
                  ctx = wpool.tile([128, 2 * 512], F32R, tag="ctx", bufs=2,
                                   name=f"ctx_{qcs[h]}")
                  ctxs.append(ctx)
              for c2 in range(2):
                  for h in range(2):
                      cps = _slot(f"cps_{qcs[h]}_{c2}")
                      nc.tensor.matmul(cps,
                                       v2t_sb[:, c2 * 128:(c2 + 1) * 128],
                                       o82s[h][0:82, :], start=True, stop=True)
                      nc.vector.tensor_tensor(
                          ctxs[h][:, c2 * 512:(c2 + 1) * 512], cps, bcs[h][:],
                          op=OP.mult)
              for ob in range(4):
                  for h in range(2):
                      qc = qcs[h]
                      ops_ = _slot(f"ops_{qc}_{ob}")
                      for cc in range(2):
                          nc.tensor.matmul(
                              ops_,
                              wws_sb[:, cc * CO + ob * 128:
                                     cc * CO + ob * 128 + 128],
                              ctxs[h][:, cc * 512:(cc + 1) * 512],
                              start=(cc == 0), stop=False)
                      nc.tensor.matmul(
                          ops_, wu_sb[:, ob * 128:(ob + 1) * 128],
                          ones_sb[:], start=False, stop=True)
                      outb = wpool.tile([128, 512], F32, tag="outb", bufs=3,
                                        name=f"outb_{qc}_{ob}")
                      _cp("act" if (ob + h) % 2 == 0 else "dve",
                          outb[:], ops_)
                      nc.sync.dma_start(
                          out=out_d[ob * 128:(ob + 1) * 128,
                                    qc * 512:(qc + 1) * 512],
                          in_=outb[:])

          # ---------- emission schedule ----------
          with tc.tile_pool(name="psJ", bufs=2, space="PSUM") as psJ, \
               tc.tile_pool(name="psO", bufs=2, space="PSUM") as psO:
              o82_qp0 = [psO.tile([82, 512], F32, tag="o82",
                                  name=f"o82ps_0_{h}") for h in range(2)]
              with tc.tile_pool(name="psHead", bufs=2, space="PSUM") as psH:
                  emit_loads_early()
                  emit_conv(psH, 0)
                  emit_conv(psH, 1)
                  emit_pooled_path(psH)
                  emit_lg2(psH, 0)
                  emit_lg2(psH, 1)
                  # lag-one interleave: after slab s, pairs 2(s-1), 2(s-1)+1
                  for s in range(2, 8):
                      emit_pair(psJ, o82_qp0, 0, 2 * (s - 2))
                      emit_pair(psJ, o82_qp0, 0, 2 * (s - 2) + 1)
                      emit_conv(psH, s)
                      emit_lg2(psH, s)
                  emit_mask()
                  emit_pair(psJ, o82_qp0, 0, 12)
                  emit_pair(psJ, o82_qp0, 0, 13)
                  emit_tail_consts(psH)
                  emit_pair(psJ, o82_qp0, 0, 14)
                  emit_pair(psJ, o82_qp0, 0, 15, last=True)
              with tc.tile_pool(name="psT", bufs=2, space="PSUM") as psT:
                  o82_qp1 = [psO.tile([82, 512], F32, tag="o82",
                                      name=f"o82ps_1_{h}") for h in range(2)]
                  # qp0's tail rides between early qp1 pairs so the in-order
                  # ACT/DVE queues don't stall qp1's exps behind it
                  for pairi in range(3):
                      emit_pair(psJ, o82_qp1, 1, pairi)
                  emit_tail_h(psT, o82_qp0, 0, 0, "act")
                  for pairi in range(3, 6):
                      emit_pair(psJ, o82_qp1, 1, pairi)
                  emit_tail_h(psT, o82_qp0, 0, 1, "dve")
                  for pairi in range(6, 16):
                      emit_pair(psJ, o82_qp1, 1, pairi, last=(pairi == 15))
                  emit_tail_final(psT, psJ, o82_qp1, 1)

    nc.finalize()
    return nc


def _get_program(reps=1):
    if ("nc", reps) not in _CACHE:
        _CACHE[("nc", reps)] = _build_program(reps)
    return _CACHE[("nc", reps)]


def _host_inputs(data_input, Wk, bk, gamma, beta, Wv, bv, Wv2, bv2, Ww, bw):
    f = np.float32
    for name, bias in (("bv", bv), ("bv2", bv2), ("bw", bw)):
        if not np.allclose(np.asarray(bias), 0.0):
            raise NotImplementedError(f"{name} != 0 not supported")
    s = (np.asarray(gamma, f) / np.sqrt(f(1.0) + f(1e-5))).astype(f)
    wk_s = (np.asarray(Wk, f) * s[:, None]) * f(SK)     # [CI, CIN]
    bk2s = ((np.asarray(bk, f) * s + np.asarray(beta, f)) * f(SK)).astype(f)

    # wk8 packed layout: [p, blk*512 + pair*256 + slot*128 + oc]
    # cin = pair*256 + slot*128 + p ; oc_global = blk*128 + oc
    wk8 = np.zeros((128, 1024), NPF8)
    wkT = np.ascontiguousarray(wk_s.T)                  # [CIN, CI]
    for blk in range(2):
        for pr in range(2):
            for sl in range(2):
                cin0 = pr * 256 + sl * 128
                col0 = blk * 512 + pr * 256 + sl * 128
                wk8[:, col0:col0 + 128] = wkT[
                    cin0:cin0 + 128, blk * 128:blk * 128 + 128].astype(NPF8)

    wvT = np.ascontiguousarray(np.asarray(Wv, f).T)
    wv2T = np.ascontiguousarray((np.asarray(Wv2, f) / f(49.0)).T)
    wvO = np.ascontiguousarray(np.asarray(Wv, f))
    wws = np.ascontiguousarray(np.asarray(Ww, f).T * f(SS / SS2))
    xs = np.ascontiguousarray(np.asarray(data_input, f).reshape(B, CIN, N))
    ones1 = np.ones((1, 512), f)
    c8 = np.full((128, 128), SS, NPF8)
    x8s = [np.ascontiguousarray(xs[b].astype(NPF8)) for b in range(B)]
    xpools = []
    for b in range(B):
        xp = np.zeros((CIN, 82), f)
        xp[:, 1:] = xs[b].reshape(CIN, 9, 7, 9, 7).sum(axis=(2, 4)).reshape(
            CIN, KK)
        xpools.append(xp)
    bk2p = np.ascontiguousarray(bk2s.reshape(2, 128).T)

    in_maps = []
    for c in range(8):
        b = c % 4
        q0 = (c // 4) * Q0STEP
        xr = np.ascontiguousarray(np.roll(x8s[b], -q0, axis=1))
        in_maps.append({
            "x8": xr, "xpool": xpools[b], "wk8": wk8, "wvT": wvT,
            "wv2T": wv2T, "wvO": wvO, "wws": wws, "bk2s": bk2p,
            "ones1": ones1, "c8ones": c8,
        })
    return in_maps


def kernel(data_input, Wk, bk, gamma, beta, Wv, bv, Wv2, bv2, Ww, bw):
    f = np.float32
    in_maps = _host_inputs(data_input, Wk, bk, gamma, beta, Wv, bv, Wv2,
                           bv2, Ww, bw)
    nc = _get_program()
    res = run_bass_kernel_spmd(nc, in_maps, list(range(8)))

    full = np.empty((B, CO, N), f)
    for b in range(B):
        full[b, :, :Q0STEP] = res.results[b]["out"][:, :Q0STEP]
        full[b, :, Q0STEP:] = res.results[4 + b]["out"][:, :QCNT]
    return full.reshape(B, CO, H, W)


# revision 16
# speedup vs baseline: 1.9723x; 1.0170x over previous
"""Trainium2 Bass kernel for EmbededNonLocalLayer (fp8 DoubleRow version).

Distribution: 8 cores = 4 batches x 2 query-halves. Each core holds its
batch's full keys; its query half sits at columns [0:2048) of a rolled x.

Math (per core), with host scales SK=16 (qk path), SW=64 (w2 path),
SS=32 (r1 ones), SS2=1024 (centered simv):
  qk8   = fp8(wk8^T x8 + bk2*SK)              [256, 4096]  (conv, DoubleRow)
  v2    = Wv2 @ (Wv @ xpool)/49               [256, 82]    (fp32, col0=0)
  w28   = fp8(Wv^T v2 * SW)                   [512, 82]    (val^T v2 == x^T w2)
  lgt2  = x8^T w28                            per 128-key block (DoubleRow)
  simv  = softmax_k(lgt2 * S/SW); dsimv8 = fp8((simv - 1/81)*SS2), col0 = SS
  E8    = fp8(exp(qk8^T qk8 * S/SK^2))        ACT exp or DVE Schraudolph bits
  o82   = dsimv8^T E8  (DoubleRow, PSUM accum over 32 key blocks)
          row0 = SS*r1 (softmax denom), rows 1:82 = SS2 * (dsimv^T E)
  ctx   = (v2t^T o82) * (1/row0 bcast);  out = (Ww*SS/SS2)^T ctx + wu^T x 1s
          (wu = Ww @ v2.sum/81 restores the centered-simv mean term)
"""

import sys

sys.path.insert(0, "/opt/trn_rl_repo")

import numpy as np
import ml_dtypes

import concourse.bacc as bacc
import concourse.bass as bass
import concourse.mybir as mybir
from concourse.bass_utils import run_bass_kernel_spmd
from concourse.tile import TileContext

F32 = mybir.dt.float32
F32R = mybir.dt.float32r
FP8 = mybir.dt.float8e4
U8 = mybir.dt.uint8
AF = mybir.ActivationFunctionType
AX = mybir.AxisListType
DR = mybir.MatmulPerfMode.DoubleRow
OP = mybir.AluOpType
NPF8 = ml_dtypes.float8_e4m3

B, CIN, H, W = 4, 512, 63, 63
N = H * W            # 3969
NPAD = 4096
CI, CO = 256, 512
KK = 81
SCALE = 0.0625       # 1/sqrt(CI)
QCNT = 1985
QP = 2048
Q0STEP = 1984
MB = NPAD // 128     # 32 key blocks
SLABS = 8            # 512-column x slabs

SK = 16.0            # qk fp8 scale
SW = 64.0            # w2 fp8 scale
SS = 32.0            # ones column scale (r1 row)
SS2 = 1024.0         # centered-simv scale
EXP_SCALE = SCALE / (SK * SK)
LG2_SCALE = SCALE / SW
SIGMA = 0.35
A_SCH = 8.0 / np.log(2.0) * EXP_SCALE
B_SCH = 8.0 * 7.0 + SIGMA

# (qp, mb) units whose exp runs on DVE via Schraudolph bits; rest on ACT.
DVE_EXP = {(qp, mb) for qp in range(2) for mb in range(MB)
           if mb % 5 in (1, 3)}

_CACHE = {}


def _build_program(reps=1):
    nc = bacc.Bacc()

    x8_d = nc.dram_tensor("x8", [CIN, N], FP8, kind="ExternalInput")
    xp_d = nc.dram_tensor("xpool", [CIN, 82], F32R, kind="ExternalInput")
    wk8_d = nc.dram_tensor("wk8", [128, 1024], FP8, kind="ExternalInput")
    wv_d = nc.dram_tensor("wvT", [CIN, CI], F32R, kind="ExternalInput")
    wv2_d = nc.dram_tensor("wv2T", [CI, CI], F32R, kind="ExternalInput")
    wvO_d = nc.dram_tensor("wvO", [CI, CIN], F32R, kind="ExternalInput")
    wws_d = nc.dram_tensor("wws", [CI, CO], F32R, kind="ExternalInput")
    bk2s_d = nc.dram_tensor("bk2s", [128, 2], F32, kind="ExternalInput")
    ones_d = nc.dram_tensor("ones1", [1, 512], F32R, kind="ExternalInput")
    c8_d = nc.dram_tensor("c8ones", [128, 128], FP8, kind="ExternalInput")
    out_d = nc.dram_tensor("out", [CO, QP], F32, kind="ExternalOutput")

    with TileContext(nc) as tc, \
         nc.allow_low_precision(reason="fp8 attention validated numerically"):
      for _rep in range(reps):
        with tc.tile_pool(name=f"const{_rep}", bufs=1) as cpool, \
             tc.tile_pool(name=f"work{_rep}", bufs=1) as wpool:
          ones_sb = cpool.tile([1, 512], F32R)
          wk8_sb = cpool.tile([128, 1024], FP8)
          wv_sb = cpool.tile([128, 4 * CI], F32R)
          wv2_sb = cpool.tile([128, 2 * CI], F32R)
          wvO_sb = cpool.tile([128, 2 * CIN], F32R)
          wws_sb = cpool.tile([128, 2 * CO], F32R)
          bk2s_sb = cpool.tile([128, 2], F32)
          xp_sb = cpool.tile([128, 4 * 82], F32R)
          x8_sb = cpool.tile([128, 4 * NPAD], FP8)
          qk8_sb = cpool.tile([128, 2 * NPAD], FP8)
          dsimv8_sb = cpool.tile([128, MB * 82], FP8)
          pooled_sb = cpool.tile([128, 2 * 82], F32R)
          v2_sb = cpool.tile([128, 2 * 82], F32R)
          v2t_sb = cpool.tile([82, CI], F32R)
          w28_sb = cpool.tile([128, 4 * 82], FP8)
          r2_sb = cpool.tile([128, MB], F32)
          r2i2_sb = cpool.tile([128, MB], F32)
          v2s_sb = cpool.tile([128, 2], F32R)
          wu_sb = cpool.tile([1, CO], F32R)
          mask_sb = cpool.tile([128, 1], F32)

          x8_4 = x8_sb.rearrange("p (c n) -> p c n", c=4)
          qk3 = qk8_sb.rearrange("p (t n) -> p t n", t=2)
          w28_4 = w28_sb.rearrange("p (c k) -> p c k", c=4)
          dsim3 = dsimv8_sb.rearrange("p (m c) -> p m c", m=MB)

          # ---------- emission helpers ----------
          def _slab_dma(s):
              n0 = s * 512
              rl = min(512, N - n0)
              nc.sync.dma_start(
                  out=x8_4[:, :, n0:n0 + rl],
                  in_=x8_d.rearrange("(c p) n -> p c n", c=4)[:, :, n0:n0 + rl])

          def emit_loads_early():
              nc.sync.dma_start(out=wk8_sb[:], in_=wk8_d[:])
              _slab_dma(0)
              _slab_dma(1)
              nc.sync.dma_start(out=bk2s_sb[:], in_=bk2s_d[:])
              nc.sync.dma_start(
                  out=xp_sb.rearrange("p (c k) -> p c k", c=4),
                  in_=xp_d.rearrange("(c p) k -> p c k", c=4))
              nc.sync.dma_start(
                  out=wv_sb.rearrange("p (c k) -> p c k", c=4),
                  in_=wv_d.rearrange("(c p) k -> p c k", c=4))
              nc.sync.dma_start(
                  out=wv2_sb.rearrange("p (c k) -> p c k", c=2),
                  in_=wv2_d.rearrange("(c p) k -> p c k", c=2))
              nc.sync.dma_start(
                  out=wvO_sb.rearrange("p (c k) -> p c k", c=2),
                  in_=wvO_d.rearrange("(c p) k -> p c k", c=2))
              _slab_dma(2)
              _slab_dma(3)
              nc.sync.dma_start(
                  out=wws_sb.rearrange("p (c k) -> p c k", c=2),
                  in_=wws_d.rearrange("(c p) k -> p c k", c=2))
              nc.sync.dma_start(out=ones_sb[:], in_=ones_d[:])
              nc.sync.dma_start(
                  out=dsim3[:, :, 0:1],
                  in_=c8_d[:, 0:MB].rearrange("p (m c) -> p m c", m=MB))
              for s in range(4, 8):
                  _slab_dma(s)
              for cc in range(4):
                  nc.gpsimd.memset(x8_4[:, cc:cc + 1, N:NPAD], 0.0)
              nc.gpsimd.memset(mask_sb[:], 0.0)
              nc.gpsimd.memset(mask_sb[0:1, :], 1.0)

          def emit_conv(psH, s):
              n0 = s * 512
              for blk in range(2):
                  ps = psH.tile([128, 512], F32, tag="ps", name=f"cv{s}_{blk}")
                  for pr in range(2):
                      lhsT = wk8_sb[:, blk * 512 + pr * 256:
                                    blk * 512 + pr * 256 + 256].rearrange(
                          "p (s o) -> p s o", s=2)
                      nc.tensor.matmul(
                          ps[:], lhsT,
                          x8_4[:, 2 * pr:2 * pr + 2, n0:n0 + 512],
                          start=(pr == 0), stop=(pr == 1), perf_mode=DR)
                  qslice = qk8_sb[:, blk * NPAD + n0:blk * NPAD + n0 + 512]
                  if blk == 0:
                      nc.scalar.activation(qslice, ps[:], AF.Identity,
                                           bias=bk2s_sb[:, blk:blk + 1])
                  else:
                      nc.vector.tensor_scalar(
                          out=qslice, in0=ps[:],
                          scalar1=bk2s_sb[:, blk:blk + 1], scalar2=None,
                          op0=OP.add)

          def emit_pooled_path(psH):
              for blk in range(2):
                  ps = psH.tile([128, 512], F32, tag="ps", name=f"pooled{blk}")
                  for cc in range(4):
                      nc.tensor.matmul(
                          ps[:, :82],
                          wv_sb[:, cc * CI + blk * 128:
                                cc * CI + blk * 128 + 128],
                          xp_sb[:, cc * 82:(cc + 1) * 82],
                          start=(cc == 0), stop=(cc == 3))
                  nc.vector.tensor_copy(pooled_sb[:, blk * 82:(blk + 1) * 82],
                                        ps[:, :82])
              for blk in range(2):
                  ps = psH.tile([128, 512], F32, tag="ps", name=f"v2_{blk}")
                  for cc in range(2):
                      nc.tensor.matmul(
                          ps[:, :82],
                          wv2_sb[:, cc * CI + blk * 128:
                                 cc * CI + blk * 128 + 128],
                          pooled_sb[:, cc * 82:(cc + 1) * 82],
                          start=(cc == 0), stop=(cc == 1))
                  nc.vector.tensor_copy(v2_sb[:, blk * 82:(blk + 1) * 82],
                                        ps[:, :82])
              # w2 = Wv^T v2 (contract ci), scaled into fp8
              for oc4 in range(4):
                  ps = psH.tile([128, 512], F32, tag="ps", name=f"w2_{oc4}")
                  for cc in range(2):
                      nc.tensor.matmul(
                          ps[:, :82],
                          wvO_sb[:, cc * CIN + oc4 * 128:
                                 cc * CIN + oc4 * 128 + 128],
                          v2_sb[:, cc * 82:(cc + 1) * 82],
                          start=(cc == 0), stop=(cc == 1))
                  nc.vector.tensor_scalar(
                      out=w28_sb[:, oc4 * 82:(oc4 + 1) * 82], in0=ps[:, :82],
                      scalar1=SW, scalar2=None, op0=OP.mult)

          def emit_tail_consts(psH):
              ps = psH.tile([128, 512], F32, tag="ps", name="v2t")
              for cc in range(2):
                  nc.tensor.matmul(ps[:82, :CI],
                                   pooled_sb[:, cc * 82:(cc + 1) * 82],
                                   wv2_sb[:, cc * CI:(cc + 1) * CI],
                                   start=(cc == 0), stop=(cc == 1))
              nc.vector.tensor_copy(v2t_sb[:], ps[:82, :CI])
              # v2s = rowsum(v2) * SS2/(SS*81); wu = v2s^T wws  -> [1, CO]
              nc.vector.reduce_sum(
                  v2s_sb[:], v2_sb.rearrange("p (c k) -> p c k", c=2),
                  axis=AX.X)
              nc.vector.tensor_scalar(
                  out=v2s_sb[:], in0=v2s_sb[:],
                  scalar1=float(SS2 / (SS * 81.0)), scalar2=None, op0=OP.mult)
              ps = psH.tile([128, 512], F32, tag="ps", name="wu")
              for cc in range(2):
                  nc.tensor.matmul(ps[0:1, :CO], v2s_sb[:, cc:cc + 1],
                                   wws_sb[:, cc * CO:(cc + 1) * CO],
                                   start=(cc == 0), stop=(cc == 1))
              nc.vector.tensor_copy(wu_sb[:], ps[0:1, :CO])

          def emit_lg2(psH, s):
              ps2 = psH.tile([128, 512], F32, tag="ps", name=f"lg{s}")
              for j in range(4):
                  mb = 4 * s + j
                  for pr in range(2):
                      nc.tensor.matmul(
                          ps2[:, j * 82:j * 82 + 82],
                          x8_4[:, 2 * pr:2 * pr + 2, mb * 128:mb * 128 + 128],
                          w28_4[:, 2 * pr:2 * pr + 2, :],
                          start=(pr == 0), stop=(pr == 1), perf_mode=DR)
              ex2 = wpool.tile([128, 4 * 82], F32, tag="ex2", bufs=2,
                               name=f"ex2_{s}")
              ps2v = ps2[:, 0:328].rearrange("p (g c) -> p g c", g=4)
              ex2v = ex2.rearrange("p (g c) -> p g c", g=4)
              nc.scalar.activation(ex2v[:, :, 1:82], ps2v[:, :, 1:82],
                                   AF.Exp, scale=LG2_SCALE)
              nc.vector.reduce_sum(r2_sb[:, 4 * s:4 * s + 4],
                                   ex2v[:, :, 1:82], axis=AX.X)
              nc.vector.reciprocal(r2i2_sb[:, 4 * s:4 * s + 4],
                                   r2_sb[:, 4 * s:4 * s + 4])
              nc.vector.tensor_scalar(
                  out=r2i2_sb[:, 4 * s:4 * s + 4],
                  in0=r2i2_sb[:, 4 * s:4 * s + 4],
                  scalar1=SS2, scalar2=None, op0=OP.mult)
              for j in range(4):
                  mb = 4 * s + j
                  nc.gpsimd.tensor_scalar(
                      out=dsim3[:, mb:mb + 1, 1:82],
                      in0=ex2v[:, j:j + 1, 1:82],
                      scalar1=r2i2_sb[:, mb:mb + 1],
                      scalar2=float(SS2 / 81.0),
                      op0=OP.mult, op1=OP.subtract)

          def emit_mask():
              nc.gpsimd.tensor_scalar(
                  out=dsim3[:, MB - 1:MB, :], in0=dsim3[:, MB - 1:MB, :],
                  scalar1=mask_sb[:, 0:1], scalar2=None, op0=OP.mult)

          pend_o82 = []

          def emit_o82(o82ps, pairi, E83):
              for h in range(2):
                  nc.tensor.matmul(
                      o82ps[h][:],
                      dsim3[:, 2 * pairi:2 * pairi + 2, :],
                      E83[:, :, h * 512:h * 512 + 512],
                      start=(pairi == 0), stop=(pairi == 15), perf_mode=DR)

          def emit_pair(psJ, o82ps, qp, pairi, last=False):
              """Emit psL+exp for pair `pairi`; the o82 accumulation is
              emitted one pair late so PE never stalls waiting on exp."""
              E8 = wpool.tile([128, 2048], FP8, tag="E8", bufs=4,
                              name=f"E8_{qp}_{pairi}")
              E83 = E8.rearrange("p (t n) -> p t n", t=2)
              for j in range(2):
                  mb = 2 * pairi + j
                  psL = psJ.tile([128, 1024], F32, tag="psL",
                                 name=f"psL_{qp}_{mb}")
                  for h in range(2):
                      nc.tensor.matmul(
                          psL[:, h * 512:h * 512 + 512],
                          qk3[:, :, mb * 128:mb * 128 + 128],
                          qk3[:, :, qp * 1024 + h * 512:
                              qp * 1024 + h * 512 + 512],
                          start=True, stop=True, perf_mode=DR)
                  dst = E8[:, j * 1024:j * 1024 + 1024]
                  if (qp, mb) in DVE_EXP:
                      nc.vector.tensor_scalar(
                          out=dst.bitcast(U8), in0=psL[:],
                          scalar1=float(A_SCH), scalar2=float(B_SCH),
                          op0=OP.mult, op1=OP.add)
                  else:
                      nc.scalar.activation(dst, psL[:], AF.Exp,
                                           scale=EXP_SCALE)
              pend_o82.append((pairi, E83))
              while len(pend_o82) > (0 if last else 1):
                  pi, e83 = pend_o82.pop(0)
                  emit_o82(o82ps, pi, e83)

          def _cp(engine, out, in_):
              if engine == "act":
                  nc.scalar.copy(out, in_)
              else:
                  nc.vector.tensor_copy(out, in_)

          def emit_tail_h(psT, o82ps, qp, h, eng):
              """Tail for one 512-query column block; copies on `eng`."""
              qc = qp * 2 + h
              o82 = wpool.tile([82, 512], F32R, tag="o82sb", bufs=2,
                               name=f"o82_{qc}")
              _cp(eng, o82[:], o82ps[h][:])
              rc = wpool.tile([1, 512], F32R, tag="rc", bufs=2,
                              name=f"rc_{qc}")
              nc.vector.reciprocal(rc[:], o82ps[h][0:1, :])
              bps = psT.tile([128, 512], F32, tag="tail", name=f"bps_{qc}")
              nc.tensor.matmul(bps[:], ones_sb[0:1, 0:128], rc[:],
                               start=True, stop=True)
              bc = wpool.tile([128, 512], F32, tag="bc", bufs=2,
                              name=f"bc_{qc}")
              _cp(eng, bc[:], bps[:])
              ctx = wpool.tile([128, 2 * 512], F32R, tag="ctx", bufs=2,
                               name=f"ctx_{qc}")
              for c2 in range(2):
                  cps = psT.tile([128, 512], F32, tag="tail",
                                 name=f"cps_{qc}_{c2}")
                  nc.tensor.matmul(cps[:],
                                   v2t_sb[:, c2 * 128:(c2 + 1) * 128],
                                   o82[0:82, :], start=True, stop=True)
                  # fold the softmax normalization into ctx
                  nc.vector.tensor_tensor(ctx[:, c2 * 512:(c2 + 1) * 512],
                                          cps[:], bc[:], op=OP.mult)
              for ob in range(4):
                  ops_ = psT.tile([128, 512], F32, tag="tail",
                                  name=f"ops_{qc}_{ob}")
                  for cc in range(2):
                      nc.tensor.matmul(
                          ops_[:],
                          wws_sb[:, cc * CO + ob * 128:
                                 cc * CO + ob * 128 + 128],
                          ctx[:, cc * 512:(cc + 1) * 512],
                          start=(cc == 0), stop=False)
                  # rank-1 mean restore: + wu^T x ones
                  nc.tensor.matmul(
                      ops_[:], wu_sb[:, ob * 128:(ob + 1) * 128],
                      ones_sb[:], start=False, stop=True)
                  outb = wpool.tile([128, 512], F32, tag="outb", bufs=8,
                                    name=f"outb_{qc}_{ob}")
                  _cp(eng if ob % 2 == 0 else
                      ("dve" if eng == "act" else "act"), outb[:], ops_[:])
                  nc.sync.dma_start(
                      out=out_d[ob * 128:(ob + 1) * 128,
                                qc * 512:(qc + 1) * 512],
                      in_=outb[:])

          def emit_tail_final(psT, psJ, o82ps, qp):
              """Last tail: both column-blocks interleaved, 4 psum slots
              (psT's 2 plus the now-idle psJ's 2)."""
              slot_i = [0]

              def _slot(name):
                  slot_i[0] += 1
                  if slot_i[0] % 2 == 0:
                      return psT.tile([128, 512], F32, tag="tail", name=name)
                  t = psJ.tile([128, 1024], F32, tag="psL", name=name)
                  return t[:, 0:512]

              qcs = [qp * 2, qp * 2 + 1]
              rcs, o82s, bcs, ctxs = [], [], [], []
              for h in range(2):
                  rc = wpool.tile([1, 512], F32R, tag="rc", bufs=2,
                                  name=f"rc_{qcs[h]}")
                  nc.vector.reciprocal(rc[:], o82ps[h][0:1, :])
                  rcs.append(rc)
              for h in range(2):
                  o82 = wpool.tile([82, 512], F32R, tag="o82sb", bufs=2,
                                   name=f"o82_{qcs[h]}")
                  nc.scalar.copy(o82[:], o82ps[h][:])
                  o82s.append(o82)
              bpss = []
              for h in range(2):
                  bps = _slot(f"bps_{qcs[h]}")
                  nc.tensor.matmul(bps, ones_sb[0:1, 0:128], rcs[h][:],
                                   start=True, stop=True)
                  bpss.append(bps)
              for h in range(2):
                  bc = wpool.tile([128, 512], F32, tag="bc", bufs=2,
                                  name=f"bc_{qcs[h]}")
                  _cp("act" if h == 0 else "dve", bc[:], bpss[h])
                  bcs.append(bc)
              for h in range(2):
                  ctx = wpool.tile([128, 2 * 512], F32R, tag="ctx", bufs=2,
                                   name=f"ctx_{qcs[h]}")
                  ctxs.append(ctx)
              for c2 in range(2):
                  for h in range(2):
                      cps = _slot(f"cps_{qcs[h]}_{c2}")
                      nc.tensor.matmul(cps,
                                       v2t_sb[:, c2 * 128:(c2 + 1) * 128],
                                       o82s[h][0:82, :], start=True, stop=True)
                      nc.vector.tensor_tensor(
                          ctxs[h][:, c2 * 512:(c2 + 1) * 512], cps, bcs[h][:],
                          op=OP.mult)
              for ob in range(4):
                  for h in range(2):
                      qc = qcs[h]
                      ops_ = _slot(f"ops_{qc}_{ob}")
                      for cc in range(2):
                          nc.tensor.matmul(
                              ops_,
                              wws_sb[:, cc * CO + ob * 128:
                                     cc * CO + ob * 128 + 128],
                              ctxs[h][:, cc * 512:(cc + 1) * 512],
                              start=(cc == 0), stop=False)
                      nc.tensor.matmul(
                          ops_, wu_sb[:, ob * 128:(ob + 1) * 128],
                          ones_sb[:], start=False, stop=True)
                      outb = wpool.tile([128, 512], F32, tag="outb", bufs=8,
                                        name=f"outb_{qc}_{ob}")
                      _cp("act" if (ob + h) % 2 == 0 else "dve",
                          outb[:], ops_)
                      nc.sync.dma_start(
                          out=out_d[ob * 128:(ob + 1) * 128,
                                    qc * 512:(qc + 1) * 512],
                          in_=outb[:])

          # ---------- emission schedule ----------
          with tc.tile_pool(name="psJ", bufs=2, space="PSUM") as psJ, \
               tc.tile_pool(name="psO", bufs=2, space="PSUM") as psO:
              o82_qp0 = [psO.tile([82, 512], F32, tag="o82",
                                  name=f"o82ps_0_{h}") for h in range(2)]
              with tc.tile_pool(name="psHead", bufs=2, space="PSUM") as psH:
                  emit_loads_early()
                  emit_conv(psH, 0)
                  emit_conv(psH, 1)
                  emit_pooled_path(psH)
                  emit_lg2(psH, 0)
                  emit_lg2(psH, 1)
                  # lag-one interleave: after slab s, pairs 2(s-1), 2(s-1)+1
                  for s in range(2, 8):
                      emit_pair(psJ, o82_qp0, 0, 2 * (s - 2))
                      emit_pair(psJ, o82_qp0, 0, 2 * (s - 2) + 1)
                      emit_conv(psH, s)
                      emit_lg2(psH, s)
                  emit_mask()
                  emit_pair(psJ, o82_qp0, 0, 12)
                  emit_pair(psJ, o82_qp0, 0, 13)
                  emit_tail_consts(psH)
                  emit_pair(psJ, o82_qp0, 0, 14)
                  emit_pair(psJ, o82_qp0, 0, 15, last=True)
              with tc.tile_pool(name="psT", bufs=2, space="PSUM") as psT:
                  o82_qp1 = [psO.tile([82, 512], F32, tag="o82",
                                      name=f"o82ps_1_{h}") for h in range(2)]
                  # qp0's tail rides between early qp1 pairs so the in-order
                  # ACT/DVE queues don't stall qp1's exps behind it
                  for pairi in range(3):
                      emit_pair(psJ, o82_qp1, 1, pairi)
                  emit_tail_h(psT, o82_qp0, 0, 0, "act")
                  for pairi in range(3, 6):
                      emit_pair(psJ, o82_qp1, 1, pairi)
                  emit_tail_h(psT, o82_qp0, 0, 1, "dve")
                  for pairi in range(6, 16):
                      emit_pair(psJ, o82_qp1, 1, pairi, last=(pairi == 15))
                  emit_tail_final(psT, psJ, o82_qp1, 1)

    nc.finalize()
    return nc


def _get_program(reps=1):
    if ("nc", reps) not in _CACHE:
        _CACHE[("nc", reps)] = _build_program(reps)
    return _CACHE[("nc", reps)]


def _host_inputs(data_input, Wk, bk, gamma, beta, Wv, bv, Wv2, bv2, Ww, bw):
    f = np.float32
    for name, bias in (("bv", bv), ("bv2", bv2), ("bw", bw)):
        if not np.allclose(np.asarray(bias), 0.0):
            raise NotImplementedError(f"{name} != 0 not supported")
    s = (np.asarray(gamma, f) / np.sqrt(f(1.0) + f(1e-5))).astype(f)
    wk_s = (np.asarray(Wk, f) * s[:, None]) * f(SK)     # [CI, CIN]
    bk2s = ((np.asarray(bk, f) * s + np.asarray(beta, f)) * f(SK)).astype(f)

    # wk8 packed layout: [p, blk*512 + pair*256 + slot*128 + oc]
    # cin = pair*256 + slot*128 + p ; oc_global = blk*128 + oc
    wk8 = np.zeros((128, 1024), NPF8)
    wkT = np.ascontiguousarray(wk_s.T)                  # [CIN, CI]
    for blk in range(2):
        for pr in range(2):
            for sl in range(2):
                cin0 = pr * 256 + sl * 128
                col0 = blk * 512 + pr * 256 + sl * 128
                wk8[:, col0:col0 + 128] = wkT[
                    cin0:cin0 + 128, blk * 128:blk * 128 + 128].astype(NPF8)

    wvT = np.ascontiguousarray(np.asarray(Wv, f).T)
    wv2T = np.ascontiguousarray((np.asarray(Wv2, f) / f(49.0)).T)
    wvO = np.ascontiguousarray(np.asarray(Wv, f))
    wws = np.ascontiguousarray(np.asarray(Ww, f).T * f(SS / SS2))
    xs = np.ascontiguousarray(np.asarray(data_input, f).reshape(B, CIN, N))
    ones1 = np.ones((1, 512), f)
    c8 = np.full((128, 128), SS, NPF8)
    x8s = [np.ascontiguousarray(xs[b].astype(NPF8)) for b in range(B)]
    xpools = []
    for b in range(B):
        xp = np.zeros((CIN, 82), f)
        xp[:, 1:] = xs[b].reshape(CIN, 9, 7, 9, 7).sum(axis=(2, 4)).reshape(
            CIN, KK)
        xpools.append(xp)
    bk2p = np.ascontiguousarray(bk2s.reshape(2, 128).T)

    in_maps = []
    for c in range(8):
        b = c % 4
        q0 = (c // 4) * Q0STEP
        xr = np.ascontiguousarray(np.roll(x8s[b], -q0, axis=1))
        in_maps.append({
            "x8": xr, "xpool": xpools[b], "wk8": wk8, "wvT": wvT,
            "wv2T": wv2T, "wvO": wvO, "wws": wws, "bk2s": bk2p,
            "ones1": ones1, "c8ones": c8,
        })
    return in_maps


def kernel(data_input, Wk, bk, gamma, beta, Wv, bv, Wv2, bv2, Ww, bw):
    f = np.float32
    in_maps = _host_inputs(data_input, Wk, bk, gamma, beta, Wv, bv, Wv2,
                           bv2, Ww, bw)
    nc = _get_program()
    res = run_bass_kernel_spmd(nc, in_maps, list(range(8)))

    full = np.empty((B, CO, N), f)
    for b in range(B):
        full[b, :, :Q0STEP] = res.results[b]["out"][:, :Q0STEP]
        full[b, :, Q0STEP:] = res.results[4 + b]["out"][:, :QCNT]
    return full.reshape(B, CO, H, W)


# revision 18
# speedup vs baseline: 2.0224x; 1.0254x over previous
"""Trainium2 Bass kernel for EmbededNonLocalLayer (fp8 DoubleRow version).

Distribution: 8 cores = 4 batches x 2 query-halves. Each core holds its
batch's full keys; its query half sits at columns [0:2048) of a rolled x.

Math (per core), with host scales SK=16 (qk path), SW=64 (w2 path),
SS=32 (r1 ones), SS2=1024 (centered simv):
  qk8   = fp8(wk8^T x8 + bk2*SK)              [256, 4096]  (conv, DoubleRow)
  v2    = Wv2 @ (Wv @ xpool)/49               [256, 82]    (fp32, col0=0)
  w28   = fp8(Wv^T v2 * SW)                   [512, 82]    (val^T v2 == x^T w2)
  lgt2  = x8^T w28                            per 128-key block (DoubleRow)
  simv  = softmax_k(lgt2 * S/SW); dsimv8 = fp8((simv - 1/81)*SS2), col0 = SS
  E8    = fp8(exp(qk8^T qk8 * S/SK^2))        ACT exp or DVE Schraudolph bits
  o82   = dsimv8^T E8  (DoubleRow, PSUM accum over 32 key blocks)
          row0 = SS*r1 (softmax denom), rows 1:82 = SS2 * (dsimv^T E)
  ctx   = (v2t^T o82) * (1/row0 bcast);  out = (Ww*SS/SS2)^T ctx + wu^T x 1s
          (wu = Ww @ v2.sum/81 restores the centered-simv mean term)
"""

import sys

sys.path.insert(0, "/opt/trn_rl_repo")

import numpy as np
import ml_dtypes

import concourse.bacc as bacc
import concourse.bass as bass
import concourse.mybir as mybir
from concourse.bass_utils import run_bass_kernel_spmd
from concourse.tile import TileContext

F32 = mybir.dt.float32
F32R = mybir.dt.float32r
FP8 = mybir.dt.float8e4
U8 = mybir.dt.uint8
AF = mybir.ActivationFunctionType
AX = mybir.AxisListType
DR = mybir.MatmulPerfMode.DoubleRow
OP = mybir.AluOpType
NPF8 = ml_dtypes.float8_e4m3

B, CIN, H, W = 4, 512, 63, 63
N = H * W            # 3969
NPAD = 4096
CI, CO = 256, 512
KK = 81
SCALE = 0.0625       # 1/sqrt(CI)
QCNT = 1985
QP = 2048
Q0STEP = 1984
MB = NPAD // 128     # 32 key blocks
SLABS = 8            # 512-column x slabs

SK = 16.0            # qk fp8 scale
SW = 64.0            # w2 fp8 scale
SS = 32.0            # ones column scale (r1 row)
SS2 = 1024.0         # centered-simv scale
EXP_SCALE = SCALE / (SK * SK)
LG2_SCALE = SCALE / SW
SIGMA = 0.35
A_SCH = 8.0 / np.log(2.0) * EXP_SCALE
B_SCH = 8.0 * 7.0 + SIGMA

# query-column widths per (qp, h): qp1's second half holds only the 449
# real query columns (1985 total); the out tail beyond QCNT is never read.
WH = {(0, 0): 512, (0, 1): 512, (1, 0): 512, (1, 1): 449}
QW = {0: 1024, 1: 961}

# (qp, mb) units whose exp runs on DVE via Schraudolph bits; rest on ACT.
DVE_EXP = {(qp, mb) for qp in range(2) for mb in range(MB)
           if mb % 5 in (1, 3)}

_CACHE = {}


def _build_program(reps=1):
    nc = bacc.Bacc()

    x8_d = nc.dram_tensor("x8", [CIN, N], FP8, kind="ExternalInput")
    xp_d = nc.dram_tensor("xpool", [CIN, 82], F32R, kind="ExternalInput")
    wk8_d = nc.dram_tensor("wk8", [128, 1024], FP8, kind="ExternalInput")
    wv_d = nc.dram_tensor("wvT", [CIN, CI], F32R, kind="ExternalInput")
    wv2_d = nc.dram_tensor("wv2T", [CI, CI], F32R, kind="ExternalInput")
    wvO_d = nc.dram_tensor("wvO", [CI, CIN], F32R, kind="ExternalInput")
    wws_d = nc.dram_tensor("wws", [CI, CO], F32R, kind="ExternalInput")
    bk2s_d = nc.dram_tensor("bk2s", [128, 2], F32, kind="ExternalInput")
    ones_d = nc.dram_tensor("ones1", [1, 512], F32R, kind="ExternalInput")
    c8_d = nc.dram_tensor("c8ones", [128, 128], FP8, kind="ExternalInput")
    out_d = nc.dram_tensor("out", [CO, QP], F32, kind="ExternalOutput")

    with TileContext(nc) as tc, \
         nc.allow_low_precision(reason="fp8 attention validated numerically"):
      for _rep in range(reps):
        with tc.tile_pool(name=f"const{_rep}", bufs=1) as cpool, \
             tc.tile_pool(name=f"work{_rep}", bufs=1) as wpool:
          ones_sb = cpool.tile([1, 512], F32R)
          wk8_sb = cpool.tile([128, 1024], FP8)
          wv_sb = cpool.tile([128, 4 * CI], F32R)
          wv2_sb = cpool.tile([128, 2 * CI], F32R)
          wvO_sb = cpool.tile([128, 2 * CIN], F32R)
          wws_sb = cpool.tile([128, 2 * CO], F32R)
          bk2s_sb = cpool.tile([128, 2], F32)
          xp_sb = cpool.tile([128, 4 * 82], F32R)
          x8_sb = cpool.tile([128, 4 * NPAD], FP8)
          qk8_sb = cpool.tile([128, 2 * NPAD], FP8)
          dsimv8_sb = cpool.tile([128, MB * 82], FP8)
          pooled_sb = cpool.tile([128, 2 * 82], F32R)
          v2_sb = cpool.tile([128, 2 * 82], F32R)
          v2t_sb = cpool.tile([82, CI], F32R)
          w28_sb = cpool.tile([128, 4 * 82], FP8)
          r2_sb = cpool.tile([128, MB], F32)
          r2i2_sb = cpool.tile([128, MB], F32)
          v2s_sb = cpool.tile([128, 2], F32R)
          wu_sb = cpool.tile([1, CO], F32R)
          mask_sb = cpool.tile([128, 1], F32)

          x8_4 = x8_sb.rearrange("p (c n) -> p c n", c=4)
          qk3 = qk8_sb.rearrange("p (t n) -> p t n", t=2)
          w28_4 = w28_sb.rearrange("p (c k) -> p c k", c=4)
          dsim3 = dsimv8_sb.rearrange("p (m c) -> p m c", m=MB)

          # ---------- emission helpers ----------
          def _slab_dma(s):
              n0 = s * 512
              rl = min(512, N - n0)
              nc.sync.dma_start(
                  out=x8_4[:, :, n0:n0 + rl],
                  in_=x8_d.rearrange("(c p) n -> p c n", c=4)[:, :, n0:n0 + rl])

          def emit_loads_early():
              nc.sync.dma_start(out=wk8_sb[:], in_=wk8_d[:])
              _slab_dma(0)
              _slab_dma(1)
              nc.sync.dma_start(out=bk2s_sb[:], in_=bk2s_d[:])
              nc.sync.dma_start(
                  out=xp_sb.rearrange("p (c k) -> p c k", c=4),
                  in_=xp_d.rearrange("(c p) k -> p c k", c=4))
              nc.sync.dma_start(
                  out=wv_sb.rearrange("p (c k) -> p c k", c=4),
                  in_=wv_d.rearrange("(c p) k -> p c k", c=4))
              nc.sync.dma_start(
                  out=wv2_sb.rearrange("p (c k) -> p c k", c=2),
                  in_=wv2_d.rearrange("(c p) k -> p c k", c=2))
              nc.sync.dma_start(
                  out=wvO_sb.rearrange("p (c k) -> p c k", c=2),
                  in_=wvO_d.rearrange("(c p) k -> p c k", c=2))
              _slab_dma(2)
              _slab_dma(3)
              nc.sync.dma_start(
                  out=wws_sb.rearrange("p (c k) -> p c k", c=2),
                  in_=wws_d.rearrange("(c p) k -> p c k", c=2))
              nc.sync.dma_start(out=ones_sb[:], in_=ones_d[:])
              nc.sync.dma_start(
                  out=dsim3[:, :, 0:1],
                  in_=c8_d[:, 0:MB].rearrange("p (m c) -> p m c", m=MB))
              for s in range(4, 8):
                  _slab_dma(s)
              for cc in range(4):
                  nc.gpsimd.memset(x8_4[:, cc:cc + 1, N:NPAD], 0.0)
              nc.gpsimd.memset(mask_sb[:], 0.0)
              nc.gpsimd.memset(mask_sb[0:1, :], 1.0)

          def emit_conv(psH, s):
              n0 = s * 512
              for blk in range(2):
                  ps = psH.tile([128, 512], F32, tag="ps", name=f"cv{s}_{blk}")
                  for pr in range(2):
                      lhsT = wk8_sb[:, blk * 512 + pr * 256:
                                    blk * 512 + pr * 256 + 256].rearrange(
                          "p (s o) -> p s o", s=2)
                      nc.tensor.matmul(
                          ps[:], lhsT,
                          x8_4[:, 2 * pr:2 * pr + 2, n0:n0 + 512],
                          start=(pr == 0), stop=(pr == 1), perf_mode=DR)
                  qslice = qk8_sb[:, blk * NPAD + n0:blk * NPAD + n0 + 512]
                  if blk == 0:
                      nc.scalar.activation(qslice, ps[:], AF.Identity,
                                           bias=bk2s_sb[:, blk:blk + 1])
                  else:
                      nc.vector.tensor_scalar(
                          out=qslice, in0=ps[:],
                          scalar1=bk2s_sb[:, blk:blk + 1], scalar2=None,
                          op0=OP.add)

          def emit_pooled_path(psH):
              for blk in range(2):
                  ps = psH.tile([128, 512], F32, tag="ps", name=f"pooled{blk}")
                  for cc in range(4):
                      nc.tensor.matmul(
                          ps[:, :82],
                          wv_sb[:, cc * CI + blk * 128:
                                cc * CI + blk * 128 + 128],
                          xp_sb[:, cc * 82:(cc + 1) * 82],
                          start=(cc == 0), stop=(cc == 3))
                  nc.vector.tensor_copy(pooled_sb[:, blk * 82:(blk + 1) * 82],
                                        ps[:, :82])
              for blk in range(2):
                  ps = psH.tile([128, 512], F32, tag="ps", name=f"v2_{blk}")
                  for cc in range(2):
                      nc.tensor.matmul(
                          ps[:, :82],
                          wv2_sb[:, cc * CI + blk * 128:
                                 cc * CI + blk * 128 + 128],
                          pooled_sb[:, cc * 82:(cc + 1) * 82],
                          start=(cc == 0), stop=(cc == 1))
                  nc.vector.tensor_copy(v2_sb[:, blk * 82:(blk + 1) * 82],
                                        ps[:, :82])
              # w2 = Wv^T v2 (contract ci), scaled into fp8
              for oc4 in range(4):
                  ps = psH.tile([128, 512], F32, tag="ps", name=f"w2_{oc4}")
                  for cc in range(2):
                      nc.tensor.matmul(
                          ps[:, :82],
                          wvO_sb[:, cc * CIN + oc4 * 128:
                                 cc * CIN + oc4 * 128 + 128],
                          v2_sb[:, cc * 82:(cc + 1) * 82],
                          start=(cc == 0), stop=(cc == 1))
                  nc.vector.tensor_scalar(
                      out=w28_sb[:, oc4 * 82:(oc4 + 1) * 82], in0=ps[:, :82],
                      scalar1=SW, scalar2=None, op0=OP.mult)

          def emit_tail_consts(psH):
              ps = psH.tile([128, 512], F32, tag="ps", name="v2t")
              for cc in range(2):
                  nc.tensor.matmul(ps[:82, :CI],
                                   pooled_sb[:, cc * 82:(cc + 1) * 82],
                                   wv2_sb[:, cc * CI:(cc + 1) * CI],
                                   start=(cc == 0), stop=(cc == 1))
              nc.vector.tensor_copy(v2t_sb[:], ps[:82, :CI])
              # v2s = rowsum(v2) * SS2/(SS*81); wu = v2s^T wws  -> [1, CO]
              nc.vector.reduce_sum(
                  v2s_sb[:], v2_sb.rearrange("p (c k) -> p c k", c=2),
                  axis=AX.X)
              nc.vector.tensor_scalar(
                  out=v2s_sb[:], in0=v2s_sb[:],
                  scalar1=float(SS2 / (SS * 81.0)), scalar2=None, op0=OP.mult)
              ps = psH.tile([128, 512], F32, tag="ps", name="wu")
              for cc in range(2):
                  nc.tensor.matmul(ps[0:1, :CO], v2s_sb[:, cc:cc + 1],
                                   wws_sb[:, cc * CO:(cc + 1) * CO],
                                   start=(cc == 0), stop=(cc == 1))
              nc.vector.tensor_copy(wu_sb[:], ps[0:1, :CO])

          def emit_lg2(psH, s):
              ps2 = psH.tile([128, 512], F32, tag="ps", name=f"lg{s}")
              for j in range(4):
                  mb = 4 * s + j
                  for pr in range(2):
                      nc.tensor.matmul(
                          ps2[:, j * 82:j * 82 + 82],
                          x8_4[:, 2 * pr:2 * pr + 2, mb * 128:mb * 128 + 128],
                          w28_4[:, 2 * pr:2 * pr + 2, :],
                          start=(pr == 0), stop=(pr == 1), perf_mode=DR)
              ex2 = wpool.tile([128, 4 * 82], F32, tag="ex2", bufs=2,
                               name=f"ex2_{s}")
              ps2v = ps2[:, 0:328].rearrange("p (g c) -> p g c", g=4)
              ex2v = ex2.rearrange("p (g c) -> p g c", g=4)
              nc.scalar.activation(ex2v[:, :, 1:82], ps2v[:, :, 1:82],
                                   AF.Exp, scale=LG2_SCALE)
              nc.vector.reduce_sum(r2_sb[:, 4 * s:4 * s + 4],
                                   ex2v[:, :, 1:82], axis=AX.X)
              nc.vector.reciprocal(r2i2_sb[:, 4 * s:4 * s + 4],
                                   r2_sb[:, 4 * s:4 * s + 4])
              nc.vector.tensor_scalar(
                  out=r2i2_sb[:, 4 * s:4 * s + 4],
                  in0=r2i2_sb[:, 4 * s:4 * s + 4],
                  scalar1=SS2, scalar2=None, op0=OP.mult)
              for j in range(4):
                  mb = 4 * s + j
                  nc.gpsimd.tensor_scalar(
                      out=dsim3[:, mb:mb + 1, 1:82],
                      in0=ex2v[:, j:j + 1, 1:82],
                      scalar1=r2i2_sb[:, mb:mb + 1],
                      scalar2=float(SS2 / 81.0),
                      op0=OP.mult, op1=OP.subtract)

          def emit_mask():
              nc.gpsimd.tensor_scalar(
                  out=dsim3[:, MB - 1:MB, :], in0=dsim3[:, MB - 1:MB, :],
                  scalar1=mask_sb[:, 0:1], scalar2=None, op0=OP.mult)

          pend_o82 = []

          def emit_o82(o82ps, qp, pairi, E83):
              for h in range(2):
                  w = WH[(qp, h)]
                  nc.tensor.matmul(
                      o82ps[h][:, 0:w],
                      dsim3[:, 2 * pairi:2 * pairi + 2, :],
                      E83[:, :, h * 512:h * 512 + w],
                      start=(pairi == 0), stop=(pairi == 15), perf_mode=DR)

          def emit_pair(psJ, o82ps, qp, pairi, last=False):
              """Emit psL+exp for pair `pairi`; the o82 accumulation is
              emitted one pair late so PE never stalls waiting on exp."""
              qw = QW[qp]
              E8 = wpool.tile([128, 2048], FP8, tag="E8", bufs=4,
                              name=f"E8_{qp}_{pairi}")
              E83 = E8[:, 0:2 * qw].rearrange("p (t n) -> p t n", t=2)
              for j in range(2):
                  mb = 2 * pairi + j
                  psL = psJ.tile([128, 1024], F32, tag="psL",
                                 name=f"psL_{qp}_{mb}")
                  for h in range(2):
                      w = WH[(qp, h)]
                      nc.tensor.matmul(
                          psL[:, h * 512:h * 512 + w],
                          qk3[:, :, mb * 128:mb * 128 + 128],
                          qk3[:, :, qp * 1024 + h * 512:
                              qp * 1024 + h * 512 + w],
                          start=True, stop=True, perf_mode=DR)
                  dst = E8[:, j * qw:j * qw + qw]
                  if (qp, mb) in DVE_EXP:
                      nc.vector.tensor_scalar(
                          out=dst.bitcast(U8), in0=psL[:, 0:qw],
                          scalar1=float(A_SCH), scalar2=float(B_SCH),
                          op0=OP.mult, op1=OP.add)
                  else:
                      nc.scalar.activation(dst, psL[:, 0:qw], AF.Exp,
                                           scale=EXP_SCALE)
              pend_o82.append((pairi, E83))
              while len(pend_o82) > (0 if last else 1):
                  pi, e83 = pend_o82.pop(0)
                  emit_o82(o82ps, qp, pi, e83)

          def _cp(engine, out, in_):
              if engine == "act":
                  nc.scalar.copy(out, in_)
              else:
                  nc.vector.tensor_copy(out, in_)

          def emit_tail_h(psT, o82ps, qp, h, eng):
              """Tail for one 512-query column block; copies on `eng`."""
              qc = qp * 2 + h
              o82 = wpool.tile([82, 512], F32R, tag="o82sb", bufs=2,
                               name=f"o82_{qc}")
              _cp(eng, o82[:], o82ps[h][:])
              rc = wpool.tile([1, 512], F32R, tag="rc", bufs=2,
                              name=f"rc_{qc}")
              nc.vector.reciprocal(rc[:], o82ps[h][0:1, :])
              bps = psT.tile([128, 512], F32, tag="tail", name=f"bps_{qc}")
              nc.tensor.matmul(bps[:], ones_sb[0:1, 0:128], rc[:],
                               start=True, stop=True)
              bc = wpool.tile([128, 512], F32, tag="bc", bufs=2,
                              name=f"bc_{qc}")
              _cp(eng, bc[:], bps[:])
              ctx = wpool.tile([128, 2 * 512], F32R, tag="ctx", bufs=2,
                               name=f"ctx_{qc}")
              for c2 in range(2):
                  cps = psT.tile([128, 512], F32, tag="tail",
                                 name=f"cps_{qc}_{c2}")
                  nc.tensor.matmul(cps[:],
                                   v2t_sb[:, c2 * 128:(c2 + 1) * 128],
                                   o82[0:82, :], start=True, stop=True)
                  # fold the softmax normalization into ctx
                  nc.vector.tensor_tensor(ctx[:, c2 * 512:(c2 + 1) * 512],
                                          cps[:], bc[:], op=OP.mult)
              for ob in range(4):
                  ops_ = psT.tile([128, 512], F32, tag="tail",
                                  name=f"ops_{qc}_{ob}")
                  for cc in range(2):
                      nc.tensor.matmul(
                          ops_[:],
                          wws_sb[:, cc * CO + ob * 128:
                                 cc * CO + ob * 128 + 128],
                          ctx[:, cc * 512:(cc + 1) * 512],
                          start=(cc == 0), stop=False)
                  # rank-1 mean restore: + wu^T x ones
                  nc.tensor.matmul(
                      ops_[:], wu_sb[:, ob * 128:(ob + 1) * 128],
                      ones_sb[:], start=False, stop=True)
                  outb = wpool.tile([128, 512], F32, tag="outb", bufs=8,
                                    name=f"outb_{qc}_{ob}")
                  _cp(eng if ob % 2 == 0 else
                      ("dve" if eng == "act" else "act"), outb[:], ops_[:])
                  nc.sync.dma_start(
                      out=out_d[ob * 128:(ob + 1) * 128,
                                qc * 512:(qc + 1) * 512],
                      in_=outb[:])

          def emit_tail_final(psT, psJ, o82ps, qp):
              """Last tail: both column-blocks interleaved, 4 psum slots
              (psT's 2 plus the now-idle psJ's 2)."""
              slot_i = [0]

              def _slot(name):
                  slot_i[0] += 1
                  if slot_i[0] % 2 == 0:
                      return psT.tile([128, 512], F32, tag="tail", name=name)
                  t = psJ.tile([128, 1024], F32, tag="psL", name=name)
                  return t

              qcs = [qp * 2, qp * 2 + 1]
              ws = [WH[(qp, 0)], WH[(qp, 1)]]
              rcs, o82s, bcs, ctxs = [], [], [], []
              for h in range(2):
                  rc = wpool.tile([1, 512], F32R, tag="rc", bufs=2,
                                  name=f"rc_{qcs[h]}")
                  nc.vector.reciprocal(rc[:, 0:ws[h]],
                                       o82ps[h][0:1, 0:ws[h]])
                  rcs.append(rc)
              for h in range(2):
                  o82 = wpool.tile([82, 512], F32R, tag="o82sb", bufs=2,
                                   name=f"o82_{qcs[h]}")
                  nc.scalar.copy(o82[:, 0:ws[h]], o82ps[h][:, 0:ws[h]])
                  o82s.append(o82)
              bpss = []
              for h in range(2):
                  bps = _slot(f"bps_{qcs[h]}")
                  nc.tensor.matmul(bps[:, 0:ws[h]], ones_sb[0:1, 0:128],
                                   rcs[h][:, 0:ws[h]],
                                   start=True, stop=True)
                  bpss.append(bps)
              for h in range(2):
                  bc = wpool.tile([128, 512], F32, tag="bc", bufs=2,
                                  name=f"bc_{qcs[h]}")
                  _cp("act" if h == 0 else "dve", bc[:, 0:ws[h]],
                      bpss[h][:, 0:ws[h]])
                  bcs.append(bc)
              for h in range(2):
                  ctx = wpool.tile([128, 2 * 512], F32R, tag="ctx", bufs=2,
                                   name=f"ctx_{qcs[h]}")
                  ctxs.append(ctx)
              for c2 in range(2):
                  for h in range(2):
                      cps = _slot(f"cps_{qcs[h]}_{c2}")
                      nc.tensor.matmul(cps[:, 0:ws[h]],
                                       v2t_sb[:, c2 * 128:(c2 + 1) * 128],
                                       o82s[h][0:82, 0:ws[h]],
                                       start=True, stop=True)
                      nc.vector.tensor_tensor(
                          ctxs[h][:, c2 * 512:c2 * 512 + ws[h]],
                          cps[:, 0:ws[h]], bcs[h][:, 0:ws[h]], op=OP.mult)
              for ob in range(4):
                  for h in range(2):
                      qc = qcs[h]
                      w = ws[h]
                      ops_ = _slot(f"ops_{qc}_{ob}")
                      for cc in range(2):
                          nc.tensor.matmul(
                              ops_[:, 0:w],
                              wws_sb[:, cc * CO + ob * 128:
                                     cc * CO + ob * 128 + 128],
                              ctxs[h][:, cc * 512:cc * 512 + w],
                              start=(cc == 0), stop=False)
                      nc.tensor.matmul(
                          ops_[:, 0:w], wu_sb[:, ob * 128:(ob + 1) * 128],
                          ones_sb[0:1, 0:w], start=False, stop=True)
                      outb = wpool.tile([128, 512], F32, tag="outb", bufs=8,
                                        name=f"outb_{qc}_{ob}")
                      _cp("act" if (ob + h) % 2 == 0 else "dve",
                          outb[:, 0:w], ops_[:, 0:w])
                      nc.sync.dma_start(
                          out=out_d[ob * 128:(ob + 1) * 128,
                                    qc * 512:qc * 512 + w],
                          in_=outb[:, 0:w])

          # ---------- emission schedule ----------
          with tc.tile_pool(name="psJ", bufs=2, space="PSUM") as psJ, \
               tc.tile_pool(name="psO", bufs=2, space="PSUM") as psO:
              o82_qp0 = [psO.tile([82, 512], F32, tag="o82",
                                  name=f"o82ps_0_{h}") for h in range(2)]
              with tc.tile_pool(name="psHead", bufs=2, space="PSUM") as psH:
                  emit_loads_early()
                  emit_conv(psH, 0)
                  emit_conv(psH, 1)
                  emit_pooled_path(psH)
                  emit_lg2(psH, 0)
                  emit_lg2(psH, 1)
                  # lag-one interleave: after slab s, pairs 2(s-1), 2(s-1)+1
                  for s in range(2, 8):
                      emit_pair(psJ, o82_qp0, 0, 2 * (s - 2))
                      emit_pair(psJ, o82_qp0, 0, 2 * (s - 2) + 1)
                      emit_conv(psH, s)
                      emit_lg2(psH, s)
                  emit_mask()
                  emit_pair(psJ, o82_qp0, 0, 12)
                  emit_pair(psJ, o82_qp0, 0, 13)
                  emit_tail_consts(psH)
                  emit_pair(psJ, o82_qp0, 0, 14)
                  emit_pair(psJ, o82_qp0, 0, 15, last=True)
              with tc.tile_pool(name="psT", bufs=2, space="PSUM") as psT:
                  o82_qp1 = [psO.tile([82, 512], F32, tag="o82",
                                      name=f"o82ps_1_{h}") for h in range(2)]
                  # qp0's tail rides between early qp1 pairs so the in-order
                  # ACT/DVE queues don't stall qp1's exps behind it
                  for pairi in range(3):
                      emit_pair(psJ, o82_qp1, 1, pairi)
                  emit_tail_h(psT, o82_qp0, 0, 0, "act")
                  for pairi in range(3, 6):
                      emit_pair(psJ, o82_qp1, 1, pairi)
                  emit_tail_h(psT, o82_qp0, 0, 1, "dve")
                  for pairi in range(6, 16):
                      emit_pair(psJ, o82_qp1, 1, pairi, last=(pairi == 15))
                  emit_tail_final(psT, psJ, o82_qp1, 1)

    nc.finalize()
    return nc


def _get_program(reps=1):
    if ("nc", reps) not in _CACHE:
        _CACHE[("nc", reps)] = _build_program(reps)
    return _CACHE[("nc", reps)]


def _host_inputs(data_input, Wk, bk, gamma, beta, Wv, bv, Wv2, bv2, Ww, bw):
    f = np.float32
    for name, bias in (("bv", bv), ("bv2", bv2), ("bw", bw)):
        if not np.allclose(np.asarray(bias), 0.0):
            raise NotImplementedError(f"{name} != 0 not supported")
    s = (np.asarray(gamma, f) / np.sqrt(f(1.0) + f(1e-5))).astype(f)
    wk_s = (np.asarray(Wk, f) * s[:, None]) * f(SK)     # [CI, CIN]
    bk2s = ((np.asarray(bk, f) * s + np.asarray(beta, f)) * f(SK)).astype(f)

    # wk8 packed layout: [p, blk*512 + pair*256 + slot*128 + oc]
    # cin = pair*256 + slot*128 + p ; oc_global = blk*128 + oc
    wk8 = np.zeros((128, 1024), NPF8)
    wkT = np.ascontiguousarray(wk_s.T)                  # [CIN, CI]
    for blk in range(2):
        for pr in range(2):
            for sl in range(2):
                cin0 = pr * 256 + sl * 128
                col0 = blk * 512 + pr * 256 + sl * 128
                wk8[:, col0:col0 + 128] = wkT[
                    cin0:cin0 + 128, blk * 128:blk * 128 + 128].astype(NPF8)

    wvT = np.ascontiguousarray(np.asarray(Wv, f).T)
    wv2T = np.ascontiguousarray((np.asarray(Wv2, f) / f(49.0)).T)
    wvO = np.ascontiguousarray(np.asarray(Wv, f))
    wws = np.ascontiguousarray(np.asarray(Ww, f).T * f(SS / SS2))
    xs = np.ascontiguousarray(np.asarray(data_input, f).reshape(B, CIN, N))
    ones1 = np.ones((1, 512), f)
    c8 = np.full((128, 128), SS, NPF8)
    x8s = [np.ascontiguousarray(xs[b].astype(NPF8)) for b in range(B)]
    xpools = []
    for b in range(B):
        xp = np.zeros((CIN, 82), f)
        xp[:, 1:] = xs[b].reshape(CIN, 9, 7, 9, 7).sum(axis=(2, 4)).reshape(
            CIN, KK)
        xpools.append(xp)
    bk2p = np.ascontiguousarray(bk2s.reshape(2, 128).T)

    in_maps = []
    for c in range(8):
        b = c % 4
        q0 = (c // 4) * Q0STEP
        xr = np.ascontiguousarray(np.roll(x8s[b], -q0, axis=1))
        in_maps.append({
            "x8": xr, "xpool": xpools[b], "wk8": wk8, "wvT": wvT,
            "wv2T": wv2T, "wvO": wvO, "wws": wws, "bk2s": bk2p,
            "ones1": ones1, "c8ones": c8,
        })
    return in_maps


def kernel(data_input, Wk, bk, gamma, beta, Wv, bv, Wv2, bv2, Ww, bw):
    f = np.float32
    in_maps = _host_inputs(data_input, Wk, bk, gamma, beta, Wv, bv, Wv2,
                           bv2, Ww, bw)
    nc = _get_program()
    res = run_bass_kernel_spmd(nc, in_maps, list(range(8)))

    full = np.empty((B, CO, N), f)
    for b in range(B):
        full[b, :, :Q0STEP] = res.results[b]["out"][:, :Q0STEP]
        full[b, :, Q0STEP:] = res.results[4 + b]["out"][:, :QCNT]
    return full.reshape(B, CO, H, W)
